# revision 12
# baseline (speedup 1.0000x reference)
"""Trainium2 Bass kernel for nn_MoEBottleneckA (MoE bottleneck block).

Strategy: data-parallel over batch (64 -> 8 samples per core, 8 cores),
weights replicated. Training-mode BatchNorm stats are synchronized with
small AllGather collectives (sync-BN). All matmuls in bf16 (fp32 PSUM
accumulation); BN statistics and normalization in fp32.

Per core:
  conv1 (1x1, 1024->256) as 8-chunk K-accumulated matmuls, gated by g1,
  BN1 partial stats fused into the PSUM->SBUF epilogues; AllGather;
  BN1 apply (+ReLU) in place; conv2 (3x3 SAME) as 18 shifted matmuls on a
  zero-padded 30x30 activation; gate g2 + BN2 stats; AllGather; BN2 apply
  with fused S2 = sum(act2) accumulation; conv3 (1x1, 256->1024) pass 1
  computes sum(y^2) partials (mean comes free via W3 @ S2); AllGather;
  conv3 pass 2 recomputes y, adds the residual inside PSUM via a
  diag(1/s3) matmul of x, and a single scalar-engine Relu(scale,bias)
  epilogue writes the final fp32 output.
"""
import sys

for _p in ("/opt/trn_rl_repo", "/root/.axon_site/_ro/trn_rl_repo"):
    if _p not in sys.path:
        sys.path.append(_p)

import numpy as np
import ml_dtypes

import concourse.bass as bass
import concourse.mybir as mybir
import concourse.tile as tile
from concourse import bacc
from concourse.masks import make_identity

N_CORES = 8
B, CIN, H, W = 64, 1024, 28, 28
MID, COUT, EMB = 256, 1024, 64
EPS = 1e-5
HWP = H * W          # 784
NH = 392             # matmul free-dim chunk (2 per 784)
BL = B // N_CORES    # samples per core
NTOT = float(B * HWP)

f32 = mybir.dt.float32
bf16 = mybir.dt.bfloat16
AF = mybir.ActivationFunctionType
ALU = mybir.AluOpType
X = mybir.AxisListType.X

bfnp = ml_dtypes.bfloat16


def _emit(nc, tc, ctx, bl, use_cc=True, phases=99, sub=99):
    """Emit the whole program. bl = samples per core."""
    ntot = float(N_CORES * bl * HWP)
    rg = [list(range(N_CORES))]

    # ---------------- DRAM I/O ----------------
    x_ext = nc.dram_tensor("x", [bl, CIN, HWP], f32, kind="ExternalInput")
    embt_ext = nc.dram_tensor("embt", [EMB, bl], bf16, kind="ExternalInput")
    w1_ext = nc.dram_tensor("w1t", [128, 8, MID], bf16, kind="ExternalInput")
    w2_ext = nc.dram_tensor("w2t", [128, 2, 9, MID], bf16, kind="ExternalInput")
    w3_ext = nc.dram_tensor("w3t", [128, 2, COUT], bf16, kind="ExternalInput")
    wg1_ext = nc.dram_tensor("wg1", [EMB, MID], bf16, kind="ExternalInput")
    wg2_ext = nc.dram_tensor("wg2", [EMB, MID], bf16, kind="ExternalInput")
    bg1_ext = nc.dram_tensor("bg1", [128, 2], f32, kind="ExternalInput")
    bg2_ext = nc.dram_tensor("bg2", [128, 2], f32, kind="ExternalInput")
    bn1g_ext = nc.dram_tensor("bn1g", [128, 2], f32, kind="ExternalInput")
    bn1b_ext = nc.dram_tensor("bn1b", [128, 2], f32, kind="ExternalInput")
    bn2g_ext = nc.dram_tensor("bn2g", [128, 2], f32, kind="ExternalInput")
    bn2b_ext = nc.dram_tensor("bn2b", [128, 2], f32, kind="ExternalInput")
    bn3g_ext = nc.dram_tensor("bn3g", [128, 8], f32, kind="ExternalInput")
    bn3b_ext = nc.dram_tensor("bn3b", [128, 8], f32, kind="ExternalInput")

    out_ext = nc.dram_tensor("out", [bl, COUT, HWP], f32, kind="ExternalOutput")
    g1_ext = nc.dram_tensor("g1", [bl, MID], f32, kind="ExternalOutput")
    g2_ext = nc.dram_tensor("g2", [bl, MID], f32, kind="ExternalOutput")

    # internal DRAM bounce buffers for the three sync-BN collectives
    cc1_in = nc.dram_tensor("cc1_in", [2 * MID], f32)
    cc1_out = nc.dram_tensor("cc1_out", [N_CORES, 2 * MID], f32, addr_space="Shared")
    cc2_in = nc.dram_tensor("cc2_in", [2 * MID], f32)
    cc2_out = nc.dram_tensor("cc2_out", [N_CORES, 2 * MID], f32, addr_space="Shared")
    cc3_in = nc.dram_tensor("cc3_in", [COUT + MID], f32)
    cc3_out = nc.dram_tensor("cc3_out", [N_CORES, COUT + MID], f32,
                             addr_space="Shared")

    # ---------------- pools ----------------
    singles = ctx.enter_context(tc.tile_pool(name="singles", bufs=1))
    xload = ctx.enter_context(tc.tile_pool(name="xload", bufs=4))
    ostage_p = ctx.enter_context(tc.tile_pool(name="ostage", bufs=4))
    scratch_p = ctx.enter_context(tc.tile_pool(name="scratch", bufs=3))
    psA = ctx.enter_context(tc.tile_pool(name="psA", bufs=3, space="PSUM"))
    psB = ctx.enter_context(tc.tile_pool(name="psB", bufs=2, space="PSUM"))

    # ---------------- persistent SBUF ----------------
    w1s = singles.tile([128, 8, MID], bf16)
    w2s = singles.tile([128, 2, 9, MID], bf16)
    w3s = singles.tile([128, 2, COUT], bf16)
    wg1s = singles.tile([EMB, MID], bf16)
    wg2s = singles.tile([EMB, MID], bf16)
    embts = singles.tile([EMB, bl], bf16)
    nc.sync.dma_start(out=w1s, in_=w1_ext[:, :, :])
    nc.sync.dma_start(out=w2s, in_=w2_ext[:, :, :, :])
    nc.sync.dma_start(out=w3s, in_=w3_ext[:, :, :])
    nc.sync.dma_start(out=wg1s, in_=wg1_ext[:, :])
    nc.sync.dma_start(out=wg2s, in_=wg2_ext[:, :])
    nc.sync.dma_start(out=embts, in_=embt_ext[:, :])

    bg1 = singles.tile([128, 2], f32)
    bg2 = singles.tile([128, 2], f32)
    bn1g = singles.tile([128, 2], f32)
    bn1b = singles.tile([128, 2], f32)
    bn2g = singles.tile([128, 2], f32)
    bn2b = singles.tile([128, 2], f32)
    bn3g = singles.tile([128, 8], f32)
    bn3b = singles.tile([128, 8], f32)
    for t, e in ((bg1, bg1_ext), (bg2, bg2_ext), (bn1g, bn1g_ext),
                 (bn1b, bn1b_ext), (bn2g, bn2g_ext), (bn2b, bn2b_ext),
                 (bn3g, bn3g_ext), (bn3b, bn3b_ext)):
        nc.sync.dma_start(out=t, in_=e[:, :])

    xres = singles.tile([128, 8, bl, HWP], bf16)
    act1 = singles.tile([128, 2, bl, 900], bf16)   # zero-padded 30x30
    act2 = singles.tile([128, 2, bl, HWP], bf16)
    nc.vector.memset(act1, 0.0)

    ident = singles.tile([128, 128], bf16)
    make_identity(nc, ident)

    # stats accumulators
    sum1 = singles.tile([128, 2 * bl], f32)
    sq1 = singles.tile([128, 2 * bl], f32)
    sum2 = singles.tile([128, 2 * bl], f32)
    sq2 = singles.tile([128, 2 * bl], f32)
    sq3 = singles.tile([128, 8 * bl], f32)

    # ---------------- gates ----------------
    g1s = singles.tile([128, 2, bl], f32)
    g2s = singles.tile([128, 2, bl], f32)
    for gs, wgs, bgs, gext in ((g1s, wg1s, bg1, g1_ext),
                               (g2s, wg2s, bg2, g2_ext)):
        for mo in range(2):
            gps = psB.tile([128, bl], f32, tag="small", name=f"gps_{mo}")
            nc.tensor.matmul(gps[:, :], lhsT=wgs[:, mo * 128:(mo + 1) * 128],
                             rhs=embts, start=True, stop=True)
            nc.scalar.activation(out=gs[:, mo, :], in_=gps[:, :], func=AF.Relu,
                                 bias=bgs[:, mo:mo + 1], scale=1.0)
        for mo in range(2):
            nc.sync.dma_start(
                out=gext[:, mo * 128:(mo + 1) * 128].rearrange("b ki -> ki b"),
                in_=gs[:, mo, :])

    def stats_combine(cc_out, width, kinds):
        """DMA gathered per-core partials back and sum over cores.

        kinds: list of (offset, n_chunks) into the flat per-core buffer.
        Returns list of [128, n_chunks] fp32 tiles (global sums).
        """
        outs = []
        for idx, (off, nch) in enumerate(kinds):
            comb = scratch_p.tile([128, nch, N_CORES], f32, tag="comb",
                                  name=f"comb_{idx}")
            for c in range(N_CORES):
                nc.sync.dma_start(
                    out=comb[:, :, c],
                    in_=cc_out[c, off:off + nch * 128].rearrange(
                        "(mo ki) -> ki mo", ki=128))
            g = singles.tile([128, nch], f32, name=f"glob_{idx}_{off}_{width}")
            nc.vector.tensor_reduce(out=g, in_=comb, axis=X, op=ALU.add)
            outs.append(g)
        return outs

    def bn_scale_bias(gsum, gsq, gamma, beta, nch, tag):
        """From global sum / sum-sq -> (s, t) with s = gamma*rsqrt(var+eps),
        t = beta - mean*s. All [128, nch] fp32."""
        m = singles.tile([128, nch], f32, name=f"m_{tag}")
        ey2 = singles.tile([128, nch], f32, name=f"ey2_{tag}")
        nc.vector.tensor_scalar_mul(m, gsum, 1.0 / ntot)
        nc.vector.tensor_scalar_mul(ey2, gsq, 1.0 / ntot)
        var = singles.tile([128, nch], f32, name=f"var_{tag}")
        nc.vector.tensor_tensor(out=var, in0=m, in1=m, op=ALU.mult)
        nc.vector.tensor_tensor(out=var, in0=ey2, in1=var, op=ALU.subtract)
        std = singles.tile([128, nch], f32, name=f"std_{tag}")
        nc.vector.tensor_scalar_add(var, var, EPS)
        nc.scalar.activation(out=std, in_=var, func=AF.Sqrt)
        rstd = singles.tile([128, nch], f32, name=f"rstd_{tag}")
        nc.vector.reciprocal(rstd, std)
        s = singles.tile([128, nch], f32, name=f"s_{tag}")
        t = singles.tile([128, nch], f32, name=f"t_{tag}")
        nc.vector.tensor_tensor(out=s, in0=gamma, in1=rstd, op=ALU.mult)
        nc.vector.tensor_tensor(out=t, in0=m, in1=s, op=ALU.mult)
        nc.vector.tensor_tensor(out=t, in0=beta, in1=t, op=ALU.subtract)
        return s, t

    if phases < 1:
        return
    # ---------------- phase 1: load x, conv1, bn1 stats ----------------
    for b in range(bl):
        for ko in range(8):
            xst = xload.tile([128, HWP], f32, tag="xst", name=f"xst_{b}_{ko}")
            nc.sync.dma_start(out=xst, in_=x_ext[b, ko * 128:(ko + 1) * 128, :])
            nc.gpsimd.tensor_copy(out=xres[:, ko, b, :], in_=xst)
        if sub < 1:
            continue
        for mo in range(2):
            pt = psA.tile([128, 2, 512], f32, tag="mm", name=f"c1_{b}_{mo}")
            for n in range(2):
                for ko in range(8):
                    nc.tensor.matmul(
                        pt[:, n, :NH],
                        lhsT=w1s[:, ko, mo * 128:(mo + 1) * 128],
                        rhs=xres[:, ko, b, n * NH:(n + 1) * NH],
                        start=(ko == 0), stop=(ko == 7))
            if sub < 2:
                sc0 = scratch_p.tile([128, HWP], bf16, tag="sq",
                                     name=f"cp1_{b}_{mo}")
                nc.vector.tensor_copy(out=sc0.rearrange(
                    "p (u c) -> p u c", u=2), in_=pt[:, :, :NH])
                continue
            # gated PSUM->SBUF into padded act1 interior, fused sum
            dst = act1[:, mo, b, :].rearrange("p (r c) -> p r c", c=30)[
                :, 1:29, 1:29].rearrange("p (u r) c -> p u r c", u=2)
            src = pt[:, :, :NH].rearrange("p u (r c) -> p u r c", c=28)
            idx = mo * bl + b
            if sub < 3:
                nc.vector.tensor_copy(out=dst, in_=src)
                continue
            nc.vector.tensor_scalar(
                out=dst, in0=src, scalar1=g1s[:, mo, b:b + 1], scalar2=None,
                op0=ALU.mult, op1=ALU.add, accum_out=sum1[:, idx:idx + 1])
            if sub < 4:
                continue
            sc = scratch_p.tile([128, HWP], bf16, tag="sq", name=f"sq1_{b}_{mo}")
            dst3 = act1[:, mo, b, :].rearrange("p (r c) -> p r c", c=30)[
                :, 1:29, 1:29]
            nc.scalar.activation(
                out=sc.rearrange("p (r c) -> p r c", c=28),
                in_=dst3, func=AF.Square,
                accum_out=sq1[:, idx:idx + 1])

    if phases < 2:
        return
    # bn1 sync
    s1loc = singles.tile([128, 2], f32)
    q1loc = singles.tile([128, 2], f32)
    nc.vector.tensor_reduce(out=s1loc, in_=sum1.rearrange(
        "p (mo b) -> p mo b", b=bl), axis=X, op=ALU.add)
    nc.vector.tensor_reduce(out=q1loc, in_=sq1.rearrange(
        "p (mo b) -> p mo b", b=bl), axis=X, op=ALU.add)
    nc.sync.dma_start(
        out=cc1_in[0:MID].rearrange("(mo ki) -> ki mo", ki=128), in_=s1loc)
    nc.sync.dma_start(
        out=cc1_in[MID:2 * MID].rearrange("(mo ki) -> ki mo", ki=128), in_=q1loc)
    if use_cc:
        nc.gpsimd.collective_compute(
            "AllGather", ALU.bypass, replica_groups=rg,
            ins=[cc1_in[:]], outs=[cc1_out[:, :]])
    else:
        for _c in range(N_CORES):
            nc.gpsimd.dma_start(out=cc1_out[_c, :], in_=cc1_in[:])
    gsum1, gsq1 = stats_combine(cc1_out, 2 * MID, [(0, 2), (MID, 2)])
    s1, t1 = bn_scale_bias(gsum1, gsq1, bn1g, bn1b, 2, "bn1")

    # bn1 apply (+ReLU), in place on act1 interior
    for mo in range(2):
        v = act1[:, mo, :, :].rearrange("p b (r c) -> p b r c", c=30)[
            :, :, 1:29, 1:29]
        nc.scalar.activation(out=v, in_=v, func=AF.Relu,
                             bias=t1[:, mo:mo + 1], scale=s1[:, mo:mo + 1])

    if phases < 3:
        return
    # ---------------- phase 2: conv2 3x3, bn2 stats ----------------
    for b in range(bl):
        for mo in range(2):
            pt = psA.tile([128, 2, 512], f32, tag="mm", name=f"c2_{b}_{mo}")
            for rc in range(2):
                k = 0
                for ko in range(2):
                    a1v = act1[:, ko, b, :].rearrange("p (r c) -> p r c", c=30)
                    for tap in range(9):
                        dy, dx = tap // 3, tap % 3
                        rhs = a1v[:, 14 * rc + dy:14 * rc + dy + 14, dx:dx + 28]
                        nc.tensor.matmul(
                            pt[:, rc, :NH],
                            lhsT=w2s[:, ko, tap, mo * 128:(mo + 1) * 128],
                            rhs=rhs, start=(k == 0), stop=(k == 17))
                        k += 1
            dst = act2[:, mo, b, :].rearrange("p (u r c) -> p u r c", u=2, c=28)
            src = pt[:, :, :NH].rearrange("p u (r c) -> p u r c", c=28)
            idx = mo * bl + b
            nc.vector.tensor_scalar(
                out=dst, in0=src, scalar1=g2s[:, mo, b:b + 1], scalar2=None,
                op0=ALU.mult, op1=ALU.add, accum_out=sum2[:, idx:idx + 1])
            sc = scratch_p.tile([128, HWP], bf16, tag="sq", name=f"sq2_{b}_{mo}")
            nc.scalar.activation(
                out=sc.rearrange("p (u c) -> p u c", u=2),
                in_=act2[:, mo, b, :].rearrange("p (u c) -> p u c", u=2),
                func=AF.Square, accum_out=sq2[:, idx:idx + 1])

    if phases < 4:
        return
    # bn2 sync
    s2loc = singles.tile([128, 2], f32)
    q2loc = singles.tile([128, 2], f32)
    nc.vector.tensor_reduce(out=s2loc, in_=sum2.rearrange(
        "p (mo b) -> p mo b", b=bl), axis=X, op=ALU.add)
    nc.vector.tensor_reduce(out=q2loc, in_=sq2.rearrange(
        "p (mo b) -> p mo b", b=bl), axis=X, op=ALU.add)
    nc.sync.dma_start(
        out=cc2_in[0:MID].rearrange("(mo ki) -> ki mo", ki=128), in_=s2loc)
    nc.sync.dma_start(
        out=cc2_in[MID:2 * MID].rearrange("(mo ki) -> ki mo", ki=128), in_=q2loc)
    if use_cc:
        nc.gpsimd.collective_compute(
            "AllGather", ALU.bypass, replica_groups=rg,
            ins=[cc2_in[:]], outs=[cc2_out[:, :]])
    else:
        for _c in range(N_CORES):
            nc.gpsimd.dma_start(out=cc2_out[_c, :], in_=cc2_in[:])
    gsum2, gsq2 = stats_combine(cc2_out, 2 * MID, [(0, 2), (MID, 2)])
    s2, t2 = bn_scale_bias(gsum2, gsq2, bn2g, bn2b, 2, "bn2")

    # bn2 apply (+ReLU) with fused S2 = sum(act2) accumulation
    S2cols = singles.tile([128, 2], f32)
    for mo in range(2):
        v = act2[:, mo, :, :]
        nc.scalar.activation(out=v, in_=v, func=AF.Relu,
                             bias=t2[:, mo:mo + 1], scale=s2[:, mo:mo + 1],
                             accum_out=S2cols[:, mo:mo + 1])

    if phases < 5:
        return
    # ---------------- phase 3: conv3 pass 1 (stats only) ----------------
    for b in range(bl):
        for mo in range(8):
            pt = psA.tile([128, 2, 512], f32, tag="mm", name=f"c3a_{b}_{mo}")
            for n in range(2):
                for ko in range(2):
                    nc.tensor.matmul(
                        pt[:, n, :NH],
                        lhsT=w3s[:, ko, mo * 128:(mo + 1) * 128],
                        rhs=act2[:, ko, b, n * NH:(n + 1) * NH],
                        start=(ko == 0), stop=(ko == 1))
            sc = scratch_p.tile([128, HWP], bf16, tag="sq", name=f"sq3_{b}_{mo}")
            idx = mo * bl + b
            nc.scalar.activation(
                out=sc.rearrange("p (u c) -> p u c", u=2),
                in_=pt[:, :, :NH], func=AF.Square,
                accum_out=sq3[:, idx:idx + 1])

    if phases < 6:
        return
    # bn3 sync: sum(y^2) partials + S2 partials in one AllGather
    q3loc = singles.tile([128, 8], f32)
    nc.vector.tensor_reduce(out=q3loc, in_=sq3.rearrange(
        "p (mo b) -> p mo b", b=bl), axis=X, op=ALU.add)
    nc.sync.dma_start(
        out=cc3_in[0:COUT].rearrange("(mo ki) -> ki mo", ki=128), in_=q3loc)
    nc.sync.dma_start(
        out=cc3_in[COUT:COUT + MID].rearrange("(ko ki) -> ki ko", ki=128),
        in_=S2cols)
    if use_cc:
        nc.gpsimd.collective_compute(
            "AllGather", ALU.bypass, replica_groups=rg,
            ins=[cc3_in[:]], outs=[cc3_out[:, :]])
    else:
        for _c in range(N_CORES):
            nc.gpsimd.dma_start(out=cc3_out[_c, :], in_=cc3_in[:])
    gsq3, gS2 = stats_combine(cc3_out, COUT + MID, [(0, 8), (COUT, 2)])

    # mean3 via W3 @ S2  (exploits linearity of the 1x1 conv)
    S2b = singles.tile([128, 2], bf16)
    nc.vector.tensor_copy(out=S2b, in_=gS2)
    pm = psB.tile([128, 8], f32, tag="small")
    for mo in range(8):
        for ko in range(2):
            nc.tensor.matmul(pm[:, mo:mo + 1],
                             lhsT=w3s[:, ko, mo * 128:(mo + 1) * 128],
                             rhs=S2b[:, ko:ko + 1],
                             start=(ko == 0), stop=(ko == 1))
    gsum3 = singles.tile([128, 8], f32)
    nc.vector.tensor_copy(out=gsum3, in_=pm)
    s3, t3 = bn_scale_bias(gsum3, gsq3, bn3g, bn3b, 8, "bn3")

    # diag(1/s3) in bf16 for the in-PSUM residual add
    invs3 = singles.tile([128, 8], f32)
    nc.vector.reciprocal(invs3, s3)
    diag3 = singles.tile([128, 8, 128], bf16)
    for mo in range(8):
        nc.vector.tensor_scalar_mul(diag3[:, mo, :], ident,
                                    invs3[:, mo:mo + 1])

    if phases < 7:
        return
    # ---------------- phase 4: conv3 pass 2 + residual + out ----------------
    for b in range(bl):
        for mo in range(8):
            pt = psA.tile([128, 2, 512], f32, tag="mm", name=f"c3b_{b}_{mo}")
            for n in range(2):
                for ko in range(2):
                    nc.tensor.matmul(
                        pt[:, n, :NH],
                        lhsT=w3s[:, ko, mo * 128:(mo + 1) * 128],
                        rhs=act2[:, ko, b, n * NH:(n + 1) * NH],
                        start=(ko == 0), stop=False)
                nc.tensor.matmul(
                    pt[:, n, :NH],
                    lhsT=diag3[:, mo, :],
                    rhs=xres[:, mo, b, n * NH:(n + 1) * NH],
                    start=False, stop=True)
            ost = ostage_p.tile([128, HWP], f32, tag="ost", name=f"ost_{b}_{mo}")
            nc.scalar.activation(
                out=ost.rearrange("p (u c) -> p u c", u=2),
                in_=pt[:, :, :NH], func=AF.Relu,
                scale=s3[:, mo:mo + 1], bias=t3[:, mo:mo + 1])
            nc.sync.dma_start(out=out_ext[b, mo * 128:(mo + 1) * 128, :],
                              in_=ost)


def build(bl=BL, use_cc=True, phases=99, sub=99):
    nc = bacc.Bacc("TRN2", target_bir_lowering=False, debug=False,
                   num_devices=N_CORES)
    from contextlib import ExitStack
    with tile.TileContext(nc) as tc, ExitStack() as ctx:
        _emit(nc, tc, ctx, bl, use_cc=use_cc, phases=phases, sub=sub)
    nc.compile()
    return nc


def prep_weights(inputs):
    """Host-side reshape/cast of the (small) replicated weights."""
    w1 = np.asarray(inputs["conv1_w"], np.float32).reshape(MID, CIN)
    w2 = np.asarray(inputs["conv2_w"], np.float32)
    w3 = np.asarray(inputs["conv3_w"], np.float32).reshape(COUT, MID)

    w1t = np.ascontiguousarray(
        w1.reshape(MID, 8, 128).transpose(2, 1, 0)).astype(bfnp)
    # w2t[ki, ko, tap, m] = w2[m, ko*128+ki, dy, dx]
    w2t = np.ascontiguousarray(
        w2.reshape(MID, 2, 128, 9).transpose(2, 1, 3, 0)).astype(bfnp)
    w3t = np.ascontiguousarray(
        w3.reshape(COUT, 2, 128).transpose(2, 1, 0)).astype(bfnp)

    def chan_tile(v, nch):
        return np.ascontiguousarray(
            np.asarray(v, np.float32).reshape(nch, 128).T)

    return {
        "w1t": w1t, "w2t": w2t, "w3t": w3t,
        "wg1": np.asarray(inputs["w_gate1"], np.float32).astype(bfnp),
        "wg2": np.asarray(inputs["w_gate2"], np.float32).astype(bfnp),
        "bg1": chan_tile(inputs["b_gate1"], 2),
        "bg2": chan_tile(inputs["b_gate2"], 2),
        "bn1g": chan_tile(inputs["bn1_g"], 2),
        "bn1b": chan_tile(inputs["bn1_b"], 2),
        "bn2g": chan_tile(inputs["bn2_g"], 2),
        "bn2b": chan_tile(inputs["bn2_b"], 2),
        "bn3g": chan_tile(inputs["bn3_g"], 8),
        "bn3b": chan_tile(inputs["bn3_b"], 8),
    }


def make_in_maps(inputs, bl=BL):
    x = np.asarray(inputs["x"], np.float32).reshape(B, CIN, HWP)
    emb = np.asarray(inputs["embeddings"], np.float32)
    w = prep_weights(inputs)
    in_maps = []
    for c in range(N_CORES):
        sl = slice(c * bl, (c + 1) * bl)
        m = dict(w)
        m["x"] = x[sl]
        m["embt"] = np.ascontiguousarray(emb[sl].T).astype(bfnp)
        in_maps.append(m)
    return in_maps


_built = {}


def _get_nc():
    if "nc" not in _built:
        _built["nc"] = build(BL)
    return _built["nc"]


def kernel(**inputs):
    from concourse.bass_utils import run_bass_kernel_spmd
    nc = _get_nc()
    in_maps = make_in_maps(inputs)
    res = run_bass_kernel_spmd(nc, in_maps, list(range(N_CORES)))
    outs = [r["out"].reshape(BL, COUT, H, W) for r in res.results]
    g1s = [r["g1"] for r in res.results]
    g2s = [r["g2"] for r in res.results]
    out = np.concatenate(outs, axis=0)
    g1 = np.concatenate(g1s, axis=0)
    g2 = np.concatenate(g2s, axis=0)
    return out, g1, g2


if __name__ == "__main__":
    nc = build(BL)
    print("built + compiled ok")


# revision 13
# speedup vs baseline: 1.1101x; 1.1101x over previous
"""Trainium2 Bass kernel for nn_MoEBottleneckA (MoE bottleneck block).

Strategy: data-parallel over batch (64 -> 8 samples per core, 8 cores),
weights replicated. Training-mode BatchNorm stats are synchronized with
small AllGather collectives (sync-BN). All matmuls in bf16 (fp32 PSUM
accumulation); BN statistics and normalization in fp32.

Per core:
  conv1 (1x1, 1024->256) as 8-chunk K-accumulated matmuls, gated by g1,
  BN1 partial stats fused into the PSUM->SBUF epilogues; AllGather;
  BN1 apply (+ReLU) in place; conv2 (3x3 SAME) as 18 shifted matmuls on a
  zero-padded 30x30 activation; gate g2 + BN2 stats; AllGather; BN2 apply
  with fused S2 = sum(act2) accumulation; conv3 (1x1, 256->1024) pass 1
  computes sum(y^2) partials (mean comes free via W3 @ S2); AllGather;
  conv3 pass 2 recomputes y, adds the residual inside PSUM via a
  diag(1/s3) matmul of x, and a single scalar-engine Relu(scale,bias)
  epilogue writes the final fp32 output.
"""
import sys

for _p in ("/opt/trn_rl_repo", "/root/.axon_site/_ro/trn_rl_repo"):
    if _p not in sys.path:
        sys.path.append(_p)

import numpy as np
import ml_dtypes

import concourse.bass as bass
import concourse.mybir as mybir
import concourse.tile as tile
from concourse import bacc
from concourse.masks import make_identity

N_CORES = 8
B, CIN, H, W = 64, 1024, 28, 28
MID, COUT, EMB = 256, 1024, 64
EPS = 1e-5
HWP = H * W          # 784
NH = 392             # matmul free-dim chunk (2 per 784)
BL = B // N_CORES    # samples per core
NTOT = float(B * HWP)

f32 = mybir.dt.float32
bf16 = mybir.dt.bfloat16
AF = mybir.ActivationFunctionType
ALU = mybir.AluOpType
X = mybir.AxisListType.X

bfnp = ml_dtypes.bfloat16


def _emit(nc, tc, ctx, bl, use_cc=True, phases=99, sub=99):
    """Emit the whole program. bl = samples per core."""
    ntot = float(N_CORES * bl * HWP)
    rg = [list(range(N_CORES))]

    # ---------------- DRAM I/O ----------------
    x_ext = nc.dram_tensor("x", [bl, CIN, HWP], f32, kind="ExternalInput")
    embt_ext = nc.dram_tensor("embt", [EMB, bl], bf16, kind="ExternalInput")
    w1_ext = nc.dram_tensor("w1t", [128, 8, MID], bf16, kind="ExternalInput")
    w2_ext = nc.dram_tensor("w2t", [128, 2, 9, MID], bf16, kind="ExternalInput")
    w3_ext = nc.dram_tensor("w3t", [128, 2, COUT], bf16, kind="ExternalInput")
    wg1_ext = nc.dram_tensor("wg1", [EMB, MID], bf16, kind="ExternalInput")
    wg2_ext = nc.dram_tensor("wg2", [EMB, MID], bf16, kind="ExternalInput")
    bg1_ext = nc.dram_tensor("bg1", [128, 2], f32, kind="ExternalInput")
    bg2_ext = nc.dram_tensor("bg2", [128, 2], f32, kind="ExternalInput")
    bn1g_ext = nc.dram_tensor("bn1g", [128, 2], f32, kind="ExternalInput")
    bn1b_ext = nc.dram_tensor("bn1b", [128, 2], f32, kind="ExternalInput")
    bn2g_ext = nc.dram_tensor("bn2g", [128, 2], f32, kind="ExternalInput")
    bn2b_ext = nc.dram_tensor("bn2b", [128, 2], f32, kind="ExternalInput")
    bn3g_ext = nc.dram_tensor("bn3g", [128, 8], f32, kind="ExternalInput")
    bn3b_ext = nc.dram_tensor("bn3b", [128, 8], f32, kind="ExternalInput")

    out_ext = nc.dram_tensor("out", [bl, COUT, HWP], f32, kind="ExternalOutput")
    g1_ext = nc.dram_tensor("g1", [bl, MID], f32, kind="ExternalOutput")
    g2_ext = nc.dram_tensor("g2", [bl, MID], f32, kind="ExternalOutput")

    # internal DRAM bounce buffers for the three sync-BN collectives
    cc1_in = nc.dram_tensor("cc1_in", [2 * MID], f32)
    cc1_out = nc.dram_tensor("cc1_out", [N_CORES, 2 * MID], f32, addr_space="Shared")
    cc2_in = nc.dram_tensor("cc2_in", [2 * MID], f32)
    cc2_out = nc.dram_tensor("cc2_out", [N_CORES, 2 * MID], f32, addr_space="Shared")
    cc3_in = nc.dram_tensor("cc3_in", [COUT + MID], f32)
    cc3_out = nc.dram_tensor("cc3_out", [N_CORES, COUT + MID], f32,
                             addr_space="Shared")

    # ---------------- pools ----------------
    singles = ctx.enter_context(tc.tile_pool(name="singles", bufs=1))
    xload = ctx.enter_context(tc.tile_pool(name="xload", bufs=4))
    ostage_p = ctx.enter_context(tc.tile_pool(name="ostage", bufs=4))
    scratch_p = ctx.enter_context(tc.tile_pool(name="scratch", bufs=3))
    psA = ctx.enter_context(tc.tile_pool(name="psA", bufs=3, space="PSUM"))
    psB = ctx.enter_context(tc.tile_pool(name="psB", bufs=2, space="PSUM"))

    # ---------------- persistent SBUF ----------------
    w1s = singles.tile([128, 8, MID], bf16)
    w2s = singles.tile([128, 2, 9, MID], bf16)
    w3s = singles.tile([128, 2, COUT], bf16)
    wg1s = singles.tile([EMB, MID], bf16)
    wg2s = singles.tile([EMB, MID], bf16)
    embts = singles.tile([EMB, bl], bf16)
    nc.sync.dma_start(out=w1s, in_=w1_ext[:, :, :])
    nc.sync.dma_start(out=w2s, in_=w2_ext[:, :, :, :])
    nc.sync.dma_start(out=w3s, in_=w3_ext[:, :, :])
    nc.sync.dma_start(out=wg1s, in_=wg1_ext[:, :])
    nc.sync.dma_start(out=wg2s, in_=wg2_ext[:, :])
    nc.sync.dma_start(out=embts, in_=embt_ext[:, :])

    bg1 = singles.tile([128, 2], f32)
    bg2 = singles.tile([128, 2], f32)
    bn1g = singles.tile([128, 2], f32)
    bn1b = singles.tile([128, 2], f32)
    bn2g = singles.tile([128, 2], f32)
    bn2b = singles.tile([128, 2], f32)
    bn3g = singles.tile([128, 8], f32)
    bn3b = singles.tile([128, 8], f32)
    for t, e in ((bg1, bg1_ext), (bg2, bg2_ext), (bn1g, bn1g_ext),
                 (bn1b, bn1b_ext), (bn2g, bn2g_ext), (bn2b, bn2b_ext),
                 (bn3g, bn3g_ext), (bn3b, bn3b_ext)):
        nc.sync.dma_start(out=t, in_=e[:, :])

    xres = singles.tile([128, 8, bl, HWP], bf16)
    act1 = singles.tile([128, 2, bl, 900], bf16)   # zero-padded 30x30
    act2 = singles.tile([128, 2, bl, HWP], bf16)
    nc.vector.memset(act1, 0.0)

    ident = singles.tile([128, 128], bf16)
    make_identity(nc, ident)

    # stats accumulators
    sum1 = singles.tile([128, 2 * bl], f32)
    sq1 = singles.tile([128, 2 * bl], f32)
    sum2 = singles.tile([128, 2 * bl], f32)
    sq2 = singles.tile([128, 2 * bl], f32)
    sq3 = singles.tile([128, 8 * bl], f32)

    # ---------------- gates ----------------
    g1s = singles.tile([128, 2, bl], f32)
    g2s = singles.tile([128, 2, bl], f32)
    for gs, wgs, bgs, gext in ((g1s, wg1s, bg1, g1_ext),
                               (g2s, wg2s, bg2, g2_ext)):
        for mo in range(2):
            gps = psB.tile([128, bl], f32, tag="small", name=f"gps_{mo}")
            nc.tensor.matmul(gps[:, :], lhsT=wgs[:, mo * 128:(mo + 1) * 128],
                             rhs=embts, start=True, stop=True)
            nc.scalar.activation(out=gs[:, mo, :], in_=gps[:, :], func=AF.Relu,
                                 bias=bgs[:, mo:mo + 1], scale=1.0)
        for mo in range(2):
            nc.sync.dma_start(
                out=gext[:, mo * 128:(mo + 1) * 128].rearrange("b ki -> ki b"),
                in_=gs[:, mo, :])

    def stats_combine(cc_out, width, kinds):
        """DMA gathered per-core partials back and sum over cores.

        kinds: list of (offset, n_chunks) into the flat per-core buffer.
        Returns list of [128, n_chunks] fp32 tiles (global sums).
        """
        outs = []
        for idx, (off, nch) in enumerate(kinds):
            comb = scratch_p.tile([128, nch, N_CORES], f32, tag="comb",
                                  name=f"comb_{idx}")
            for c in range(N_CORES):
                nc.sync.dma_start(
                    out=comb[:, :, c],
                    in_=cc_out[c, off:off + nch * 128].rearrange(
                        "(mo ki) -> ki mo", ki=128))
            g = singles.tile([128, nch], f32, name=f"glob_{idx}_{off}_{width}")
            nc.vector.tensor_reduce(out=g, in_=comb, axis=X, op=ALU.add)
            outs.append(g)
        return outs

    def bn_scale_bias(gsum, gsq, gamma, beta, nch, tag):
        """From global sum / sum-sq -> (s, t) with s = gamma*rsqrt(var+eps),
        t = beta - mean*s. All [128, nch] fp32."""
        m = singles.tile([128, nch], f32, name=f"m_{tag}")
        ey2 = singles.tile([128, nch], f32, name=f"ey2_{tag}")
        nc.vector.tensor_scalar_mul(m, gsum, 1.0 / ntot)
        nc.vector.tensor_scalar_mul(ey2, gsq, 1.0 / ntot)
        var = singles.tile([128, nch], f32, name=f"var_{tag}")
        nc.vector.tensor_tensor(out=var, in0=m, in1=m, op=ALU.mult)
        nc.vector.tensor_tensor(out=var, in0=ey2, in1=var, op=ALU.subtract)
        std = singles.tile([128, nch], f32, name=f"std_{tag}")
        nc.vector.tensor_scalar_add(var, var, EPS)
        nc.scalar.activation(out=std, in_=var, func=AF.Sqrt)
        rstd = singles.tile([128, nch], f32, name=f"rstd_{tag}")
        nc.vector.reciprocal(rstd, std)
        s = singles.tile([128, nch], f32, name=f"s_{tag}")
        t = singles.tile([128, nch], f32, name=f"t_{tag}")
        nc.vector.tensor_tensor(out=s, in0=gamma, in1=rstd, op=ALU.mult)
        nc.vector.tensor_tensor(out=t, in0=m, in1=s, op=ALU.mult)
        nc.vector.tensor_tensor(out=t, in0=beta, in1=t, op=ALU.subtract)
        return s, t

    if phases < 1:
        return
    # ---------------- phase 1: load x, conv1, bn1 stats ----------------
    for b in range(bl):
        for ko in range(8):
            xst = xload.tile([128, HWP], f32, tag="xst", name=f"xst_{b}_{ko}")
            nc.sync.dma_start(out=xst, in_=x_ext[b, ko * 128:(ko + 1) * 128, :])
            nc.vector.tensor_copy(out=xres[:, ko, b, :], in_=xst)
        if sub < 1:
            continue
        for mo in range(2):
            pt = psA.tile([128, 2, 512], f32, tag="mm", name=f"c1_{b}_{mo}")
            for n in range(2):
                for ko in range(8):
                    nc.tensor.matmul(
                        pt[:, n, :NH],
                        lhsT=w1s[:, ko, mo * 128:(mo + 1) * 128],
                        rhs=xres[:, ko, b, n * NH:(n + 1) * NH],
                        start=(ko == 0), stop=(ko == 7))
            if sub < 2:
                sc0 = scratch_p.tile([128, HWP], bf16, tag="sq",
                                     name=f"cp1_{b}_{mo}")
                nc.vector.tensor_copy(out=sc0.rearrange(
                    "p (u c) -> p u c", u=2), in_=pt[:, :, :NH])
                continue
            # gated PSUM->SBUF into padded act1 interior, fused sum
            dst = act1[:, mo, b, :].rearrange("p (r c) -> p r c", c=30)[
                :, 1:29, 1:29].rearrange("p (u r) c -> p u r c", u=2)
            src = pt[:, :, :NH].rearrange("p u (r c) -> p u r c", c=28)
            idx = mo * bl + b
            if sub < 3:
                nc.vector.tensor_copy(out=dst, in_=src)
                continue
            nc.vector.tensor_scalar(
                out=dst, in0=src, scalar1=g1s[:, mo, b:b + 1], scalar2=None,
                op0=ALU.mult, op1=ALU.add, accum_out=sum1[:, idx:idx + 1])
            if sub < 4:
                continue
            sc = scratch_p.tile([128, HWP], bf16, tag="sq", name=f"sq1_{b}_{mo}")
            dst3 = act1[:, mo, b, :].rearrange("p (r c) -> p r c", c=30)[
                :, 1:29, 1:29]
            nc.scalar.activation(
                out=sc.rearrange("p (r c) -> p r c", c=28),
                in_=dst3, func=AF.Square,
                accum_out=sq1[:, idx:idx + 1])

    if phases < 2:
        return
    # bn1 sync
    s1loc = singles.tile([128, 2], f32)
    q1loc = singles.tile([128, 2], f32)
    nc.vector.tensor_reduce(out=s1loc, in_=sum1.rearrange(
        "p (mo b) -> p mo b", b=bl), axis=X, op=ALU.add)
    nc.vector.tensor_reduce(out=q1loc, in_=sq1.rearrange(
        "p (mo b) -> p mo b", b=bl), axis=X, op=ALU.add)
    nc.sync.dma_start(
        out=cc1_in[0:MID].rearrange("(mo ki) -> ki mo", ki=128), in_=s1loc)
    nc.sync.dma_start(
        out=cc1_in[MID:2 * MID].rearrange("(mo ki) -> ki mo", ki=128), in_=q1loc)
    if use_cc:
        nc.gpsimd.collective_compute(
            "AllGather", ALU.bypass, replica_groups=rg,
            ins=[cc1_in[:]], outs=[cc1_out[:, :]])
    else:
        for _c in range(N_CORES):
            nc.gpsimd.dma_start(out=cc1_out[_c, :], in_=cc1_in[:])
    gsum1, gsq1 = stats_combine(cc1_out, 2 * MID, [(0, 2), (MID, 2)])
    s1, t1 = bn_scale_bias(gsum1, gsq1, bn1g, bn1b, 2, "bn1")

    # bn1 apply (+ReLU), in place on act1 interior (chunked per sample so
    # conv2 for sample b starts as soon as its two chunks are normalized)
    for b in range(bl):
        for mo in range(2):
            v = act1[:, mo, b, :].rearrange("p (r c) -> p r c", c=30)[
                :, 1:29, 1:29]
            nc.scalar.activation(out=v, in_=v, func=AF.Relu,
                                 bias=t1[:, mo:mo + 1], scale=s1[:, mo:mo + 1])

    if phases < 3:
        return
    # ---------------- phase 2: conv2 3x3, bn2 stats ----------------
    for b in range(bl):
        for mo in range(2):
            pt = psA.tile([128, 2, 512], f32, tag="mm", name=f"c2_{b}_{mo}")
            for rc in range(2):
                k = 0
                for ko in range(2):
                    a1v = act1[:, ko, b, :].rearrange("p (r c) -> p r c", c=30)
                    for tap in range(9):
                        dy, dx = tap // 3, tap % 3
                        rhs = a1v[:, 14 * rc + dy:14 * rc + dy + 14, dx:dx + 28]
                        nc.tensor.matmul(
                            pt[:, rc, :NH],
                            lhsT=w2s[:, ko, tap, mo * 128:(mo + 1) * 128],
                            rhs=rhs, start=(k == 0), stop=(k == 17))
                        k += 1
            dst = act2[:, mo, b, :].rearrange("p (u r c) -> p u r c", u=2, c=28)
            src = pt[:, :, :NH].rearrange("p u (r c) -> p u r c", c=28)
            idx = mo * bl + b
            nc.vector.tensor_scalar(
                out=dst, in0=src, scalar1=g2s[:, mo, b:b + 1], scalar2=None,
                op0=ALU.mult, op1=ALU.add, accum_out=sum2[:, idx:idx + 1])
            sc = scratch_p.tile([128, HWP], bf16, tag="sq", name=f"sq2_{b}_{mo}")
            nc.scalar.activation(
                out=sc.rearrange("p (u c) -> p u c", u=2),
                in_=act2[:, mo, b, :].rearrange("p (u c) -> p u c", u=2),
                func=AF.Square, accum_out=sq2[:, idx:idx + 1])

    if phases < 4:
        return
    # bn2 sync
    s2loc = singles.tile([128, 2], f32)
    q2loc = singles.tile([128, 2], f32)
    nc.vector.tensor_reduce(out=s2loc, in_=sum2.rearrange(
        "p (mo b) -> p mo b", b=bl), axis=X, op=ALU.add)
    nc.vector.tensor_reduce(out=q2loc, in_=sq2.rearrange(
        "p (mo b) -> p mo b", b=bl), axis=X, op=ALU.add)
    nc.sync.dma_start(
        out=cc2_in[0:MID].rearrange("(mo ki) -> ki mo", ki=128), in_=s2loc)
    nc.sync.dma_start(
        out=cc2_in[MID:2 * MID].rearrange("(mo ki) -> ki mo", ki=128), in_=q2loc)
    if use_cc:
        nc.gpsimd.collective_compute(
            "AllGather", ALU.bypass, replica_groups=rg,
            ins=[cc2_in[:]], outs=[cc2_out[:, :]])
    else:
        for _c in range(N_CORES):
            nc.gpsimd.dma_start(out=cc2_out[_c, :], in_=cc2_in[:])
    gsum2, gsq2 = stats_combine(cc2_out, 2 * MID, [(0, 2), (MID, 2)])
    s2, t2 = bn_scale_bias(gsum2, gsq2, bn2g, bn2b, 2, "bn2")

    # bn2 apply (+ReLU) with fused S2 = sum(act2) accumulation
    S2acc = singles.tile([128, 2 * bl], f32)
    for b in range(bl):
        for mo in range(2):
            v = act2[:, mo, b, :]
            idx = mo * bl + b
            nc.scalar.activation(out=v, in_=v, func=AF.Relu,
                                 bias=t2[:, mo:mo + 1], scale=s2[:, mo:mo + 1],
                                 accum_out=S2acc[:, idx:idx + 1])
    S2cols = singles.tile([128, 2], f32)
    nc.vector.tensor_reduce(out=S2cols, in_=S2acc.rearrange(
        "p (mo b) -> p mo b", b=bl), axis=X, op=ALU.add)

    if phases < 5:
        return
    # ---------------- phase 3: conv3 pass 1 (stats only) ----------------
    for b in range(bl):
        for mo in range(8):
            pt = psA.tile([128, 2, 512], f32, tag="mm", name=f"c3a_{b}_{mo}")
            for n in range(2):
                for ko in range(2):
                    nc.tensor.matmul(
                        pt[:, n, :NH],
                        lhsT=w3s[:, ko, mo * 128:(mo + 1) * 128],
                        rhs=act2[:, ko, b, n * NH:(n + 1) * NH],
                        start=(ko == 0), stop=(ko == 1))
            sc = scratch_p.tile([128, HWP], bf16, tag="sq", name=f"sq3_{b}_{mo}")
            idx = mo * bl + b
            nc.scalar.activation(
                out=sc.rearrange("p (u c) -> p u c", u=2),
                in_=pt[:, :, :NH], func=AF.Square,
                accum_out=sq3[:, idx:idx + 1])

    if phases < 6:
        return
    # bn3 sync: sum(y^2) partials + S2 partials in one AllGather
    q3loc = singles.tile([128, 8], f32)
    nc.vector.tensor_reduce(out=q3loc, in_=sq3.rearrange(
        "p (mo b) -> p mo b", b=bl), axis=X, op=ALU.add)
    nc.sync.dma_start(
        out=cc3_in[0:COUT].rearrange("(mo ki) -> ki mo", ki=128), in_=q3loc)
    nc.sync.dma_start(
        out=cc3_in[COUT:COUT + MID].rearrange("(ko ki) -> ki ko", ki=128),
        in_=S2cols)
    if use_cc:
        nc.gpsimd.collective_compute(
            "AllGather", ALU.bypass, replica_groups=rg,
            ins=[cc3_in[:]], outs=[cc3_out[:, :]])
    else:
        for _c in range(N_CORES):
            nc.gpsimd.dma_start(out=cc3_out[_c, :], in_=cc3_in[:])
    gsq3, gS2 = stats_combine(cc3_out, COUT + MID, [(0, 8), (COUT, 2)])

    # mean3 via W3 @ S2  (exploits linearity of the 1x1 conv)
    S2b = singles.tile([128, 2], bf16)
    nc.vector.tensor_copy(out=S2b, in_=gS2)
    pm = psB.tile([128, 8], f32, tag="small")
    for mo in range(8):
        for ko in range(2):
            nc.tensor.matmul(pm[:, mo:mo + 1],
                             lhsT=w3s[:, ko, mo * 128:(mo + 1) * 128],
                             rhs=S2b[:, ko:ko + 1],
                             start=(ko == 0), stop=(ko == 1))
    gsum3 = singles.tile([128, 8], f32)
    nc.vector.tensor_copy(out=gsum3, in_=pm)
    s3, t3 = bn_scale_bias(gsum3, gsq3, bn3g, bn3b, 8, "bn3")

    # diag(1/s3) in bf16 for the in-PSUM residual add
    invs3 = singles.tile([128, 8], f32)
    nc.vector.reciprocal(invs3, s3)
    diag3 = singles.tile([128, 8, 128], bf16)
    for mo in range(8):
        nc.vector.tensor_scalar_mul(diag3[:, mo, :], ident,
                                    invs3[:, mo:mo + 1])

    if phases < 7:
        return
    # ---------------- phase 4: conv3 pass 2 + residual + out ----------------
    for b in range(bl):
        for mo in range(8):
            pt = psA.tile([128, 2, 512], f32, tag="mm", name=f"c3b_{b}_{mo}")
            for n in range(2):
                for ko in range(2):
                    nc.tensor.matmul(
                        pt[:, n, :NH],
                        lhsT=w3s[:, ko, mo * 128:(mo + 1) * 128],
                        rhs=act2[:, ko, b, n * NH:(n + 1) * NH],
                        start=(ko == 0), stop=False)
                nc.tensor.matmul(
                    pt[:, n, :NH],
                    lhsT=diag3[:, mo, :],
                    rhs=xres[:, mo, b, n * NH:(n + 1) * NH],
                    start=False, stop=True)
            ost = ostage_p.tile([128, HWP], f32, tag="ost", name=f"ost_{b}_{mo}")
            if (b * 8 + mo) % 2 == 0:
                nc.scalar.activation(
                    out=ost.rearrange("p (u c) -> p u c", u=2),
                    in_=pt[:, :, :NH], func=AF.Relu,
                    scale=s3[:, mo:mo + 1], bias=t3[:, mo:mo + 1])
            else:
                nc.vector.tensor_scalar(
                    out=ost.rearrange("p (u c) -> p u c", u=2),
                    in0=pt[:, :, :NH], scalar1=s3[:, mo:mo + 1],
                    scalar2=t3[:, mo:mo + 1], op0=ALU.mult, op1=ALU.add)
                nc.vector.tensor_scalar_max(ost, ost, 0.0)
            nc.sync.dma_start(out=out_ext[b, mo * 128:(mo + 1) * 128, :],
                              in_=ost)


def build(bl=BL, use_cc=True, phases=99, sub=99):
    nc = bacc.Bacc("TRN2", target_bir_lowering=False, debug=False,
                   num_devices=N_CORES)
    from contextlib import ExitStack
    with tile.TileContext(nc) as tc, ExitStack() as ctx:
        _emit(nc, tc, ctx, bl, use_cc=use_cc, phases=phases, sub=sub)
    nc.compile()
    return nc


def prep_weights(inputs):
    """Host-side reshape/cast of the (small) replicated weights."""
    w1 = np.asarray(inputs["conv1_w"], np.float32).reshape(MID, CIN)
    w2 = np.asarray(inputs["conv2_w"], np.float32)
    w3 = np.asarray(inputs["conv3_w"], np.float32).reshape(COUT, MID)

    w1t = np.ascontiguousarray(
        w1.reshape(MID, 8, 128).transpose(2, 1, 0)).astype(bfnp)
    # w2t[ki, ko, tap, m] = w2[m, ko*128+ki, dy, dx]
    w2t = np.ascontiguousarray(
        w2.reshape(MID, 2, 128, 9).transpose(2, 1, 3, 0)).astype(bfnp)
    w3t = np.ascontiguousarray(
        w3.reshape(COUT, 2, 128).transpose(2, 1, 0)).astype(bfnp)

    def chan_tile(v, nch):
        return np.ascontiguousarray(
            np.asarray(v, np.float32).reshape(nch, 128).T)

    return {
        "w1t": w1t, "w2t": w2t, "w3t": w3t,
        "wg1": np.asarray(inputs["w_gate1"], np.float32).astype(bfnp),
        "wg2": np.asarray(inputs["w_gate2"], np.float32).astype(bfnp),
        "bg1": chan_tile(inputs["b_gate1"], 2),
        "bg2": chan_tile(inputs["b_gate2"], 2),
        "bn1g": chan_tile(inputs["bn1_g"], 2),
        "bn1b": chan_tile(inputs["bn1_b"], 2),
        "bn2g": chan_tile(inputs["bn2_g"], 2),
        "bn2b": chan_tile(inputs["bn2_b"], 2),
        "bn3g": chan_tile(inputs["bn3_g"], 8),
        "bn3b": chan_tile(inputs["bn3_b"], 8),
    }


def make_in_maps(inputs, bl=BL):
    x = np.asarray(inputs["x"], np.float32).reshape(B, CIN, HWP)
    emb = np.asarray(inputs["embeddings"], np.float32)
    w = prep_weights(inputs)
    in_maps = []
    for c in range(N_CORES):
        sl = slice(c * bl, (c + 1) * bl)
        m = dict(w)
        m["x"] = x[sl]
        m["embt"] = np.ascontiguousarray(emb[sl].T).astype(bfnp)
        in_maps.append(m)
    return in_maps


_built = {}


def _get_nc():
    if "nc" not in _built:
        _built["nc"] = build(BL)
    return _built["nc"]


def kernel(**inputs):
    from concourse.bass_utils import run_bass_kernel_spmd
    nc = _get_nc()
    in_maps = make_in_maps(inputs)
    res = run_bass_kernel_spmd(nc, in_maps, list(range(N_CORES)))
    outs = [r["out"].reshape(BL, COUT, H, W) for r in res.results]
    g1s = [r["g1"] for r in res.results]
    g2s = [r["g2"] for r in res.results]
    out = np.concatenate(outs, axis=0)
    g1 = np.concatenate(g1s, axis=0)
    g2 = np.concatenate(g2s, axis=0)
    return out, g1, g2


if __name__ == "__main__":
    nc = build(BL)
    print("built + compiled ok")


# revision 17
# speedup vs baseline: 1.2449x; 1.1214x over previous
"""Trainium2 Bass kernel for nn_MoEBottleneckA (MoE bottleneck block).

Strategy: data-parallel over batch (64 -> 8 samples per core, 8 cores),
weights replicated. Training-mode BatchNorm stats are synchronized with
small AllGather collectives (sync-BN). All matmuls in bf16 (fp32 PSUM
accumulation); BN statistics and normalization in fp32.

Per core:
  conv1 (1x1, 1024->256) as 8-chunk K-accumulated matmuls, gated by g1,
  BN1 partial stats fused into the PSUM->SBUF epilogues; AllGather;
  BN1 apply (+ReLU) in place; conv2 (3x3 SAME) as 18 shifted matmuls on a
  zero-padded 30x30 activation; gate g2 + BN2 stats; AllGather; BN2 apply
  with fused S2 = sum(act2) accumulation; conv3 (1x1, 256->1024) pass 1
  computes sum(y^2) partials (mean comes free via W3 @ S2); AllGather;
  conv3 pass 2 recomputes y, adds the residual inside PSUM via a
  diag(1/s3) matmul of x, and Relu(scale,bias) epilogues (split between
  the scalar and vector engines) write the final fp32 output.

Cross-core partial sums travel as [128, n] contiguous blocks; the
per-core reduction of the gathered [8, 128*n] block is done on the
tensor engine (ones-vector matmul), which keeps the sync window short.
"""
import sys

for _p in ("/opt/trn_rl_repo", "/root/.axon_site/_ro/trn_rl_repo"):
    if _p not in sys.path:
        sys.path.append(_p)

import numpy as np
import ml_dtypes

import concourse.bass as bass
import concourse.mybir as mybir
import concourse.tile as tile
from concourse import bacc
from concourse.masks import make_identity

N_CORES = 8
B, CIN, H, W = 64, 1024, 28, 28
MID, COUT, EMB = 256, 1024, 64
EPS = 1e-5
HWP = H * W          # 784
NH = 392             # matmul free-dim chunk (2 per 784)
BL = B // N_CORES    # samples per core
NTOT = float(B * HWP)

f32 = mybir.dt.float32
bf16 = mybir.dt.bfloat16
AF = mybir.ActivationFunctionType
ALU = mybir.AluOpType
X = mybir.AxisListType.X

bfnp = ml_dtypes.bfloat16


def _emit(nc, tc, ctx, bl):
    ntot = float(N_CORES * bl * HWP)
    rg = [list(range(N_CORES))]

    # ---------------- DRAM I/O ----------------
    x_ext = nc.dram_tensor("x", [bl, CIN, HWP], f32, kind="ExternalInput")
    embt_ext = nc.dram_tensor("embt", [EMB, bl], bf16, kind="ExternalInput")
    w1_ext = nc.dram_tensor("w1t", [128, 8, MID], bf16, kind="ExternalInput")
    w2_ext = nc.dram_tensor("w2t", [128, 2, 9, MID], bf16, kind="ExternalInput")
    w3_ext = nc.dram_tensor("w3t", [128, 2, COUT], bf16, kind="ExternalInput")
    wg1_ext = nc.dram_tensor("wg1", [EMB, MID], bf16, kind="ExternalInput")
    wg2_ext = nc.dram_tensor("wg2", [EMB, MID], bf16, kind="ExternalInput")
    bg1_ext = nc.dram_tensor("bg1", [128, 2], f32, kind="ExternalInput")
    bg2_ext = nc.dram_tensor("bg2", [128, 2], f32, kind="ExternalInput")
    bn1g_ext = nc.dram_tensor("bn1g", [128, 2], f32, kind="ExternalInput")
    bn1b_ext = nc.dram_tensor("bn1b", [128, 2], f32, kind="ExternalInput")
    bn2g_ext = nc.dram_tensor("bn2g", [128, 2], f32, kind="ExternalInput")
    bn2b_ext = nc.dram_tensor("bn2b", [128, 2], f32, kind="ExternalInput")
    bn3g_ext = nc.dram_tensor("bn3g", [128, 8], f32, kind="ExternalInput")
    bn3b_ext = nc.dram_tensor("bn3b", [128, 8], f32, kind="ExternalInput")

    out_ext = nc.dram_tensor("out", [bl, COUT, HWP], f32, kind="ExternalOutput")
    g1_ext = nc.dram_tensor("g1", [bl, MID], f32, kind="ExternalOutput")
    g2_ext = nc.dram_tensor("g2", [bl, MID], f32, kind="ExternalOutput")

    # internal DRAM bounce buffers for the sync-BN collectives
    ccw_in = nc.dram_tensor("ccw_in", [8], f32)
    ccw_out = nc.dram_tensor("ccw_out", [N_CORES, 8], f32, addr_space="Shared")
    cc1_in = nc.dram_tensor("cc1_in", [128, 4], f32)
    cc1_out = nc.dram_tensor("cc1_out", [N_CORES, 128 * 4], f32,
                             addr_space="Shared")
    cc2_in = nc.dram_tensor("cc2_in", [128, 4], f32)
    cc2_out = nc.dram_tensor("cc2_out", [N_CORES, 128 * 4], f32,
                             addr_space="Shared")
    cc3_in = nc.dram_tensor("cc3_in", [128, 10], f32)
    cc3_out = nc.dram_tensor("cc3_out", [N_CORES, 128 * 10], f32,
                             addr_space="Shared")

    # ---------------- pools ----------------
    singles = ctx.enter_context(tc.tile_pool(name="singles", bufs=1))
    xload = ctx.enter_context(tc.tile_pool(name="xload", bufs=4))
    ostage_p = ctx.enter_context(tc.tile_pool(name="ostage", bufs=3))
    scratch_p = ctx.enter_context(tc.tile_pool(name="scratch", bufs=3))
    psA = ctx.enter_context(tc.tile_pool(name="psA", bufs=3, space="PSUM"))
    psB = ctx.enter_context(tc.tile_pool(name="psB", bufs=2, space="PSUM"))

    # ---------------- persistent SBUF ----------------
    xres = singles.tile([128, 8, bl, HWP], bf16)
    act1 = singles.tile([128, 2, bl, 900], bf16)   # zero-padded 30x30
    act2 = singles.tile([128, 2, bl, HWP], bf16)

    # x for the first samples first: conv1 starts as soon as possible
    w1s = singles.tile([128, 8, MID], bf16)
    xsts = {}
    for b in range(min(2, bl)):
        for ko in range(8):
            xst = xload.tile([128, HWP], f32, tag="xst", name=f"xst_{b}_{ko}")
            nc.sync.dma_start(out=xst, in_=x_ext[b, ko * 128:(ko + 1) * 128, :])
            xsts[(b, ko)] = xst
    nc.sync.dma_start(out=w1s, in_=w1_ext[:, :, :])

    # warm up the collectives firmware with a dummy 8-float AllGather
    ones8 = singles.tile([8, 1], f32)
    nc.vector.memset(ones8, 1.0)
    nc.sync.dma_start(out=ccw_in[:], in_=ones8[:, 0])
    nc.gpsimd.collective_compute(
        "AllGather", ALU.bypass, replica_groups=rg,
        ins=[ccw_in[:]], outs=[ccw_out[:, :]])

    w2s = singles.tile([128, 2, 9, MID], bf16)
    w3s = singles.tile([128, 2, COUT], bf16)
    wg1s = singles.tile([EMB, MID], bf16)
    wg2s = singles.tile([EMB, MID], bf16)
    embts = singles.tile([EMB, bl], bf16)
    nc.sync.dma_start(out=wg1s, in_=wg1_ext[:, :])
    nc.sync.dma_start(out=wg2s, in_=wg2_ext[:, :])
    nc.sync.dma_start(out=embts, in_=embt_ext[:, :])
    nc.sync.dma_start(out=w2s, in_=w2_ext[:, :, :, :])
    nc.sync.dma_start(out=w3s, in_=w3_ext[:, :, :])

    bg1 = singles.tile([128, 2], f32)
    bg2 = singles.tile([128, 2], f32)
    bn1g = singles.tile([128, 2], f32)
    bn1b = singles.tile([128, 2], f32)
    bn2g = singles.tile([128, 2], f32)
    bn2b = singles.tile([128, 2], f32)
    bn3g = singles.tile([128, 8], f32)
    bn3b = singles.tile([128, 8], f32)
    for t, e in ((bg1, bg1_ext), (bg2, bg2_ext), (bn1g, bn1g_ext),
                 (bn1b, bn1b_ext), (bn2g, bn2g_ext), (bn2b, bn2b_ext),
                 (bn3g, bn3g_ext), (bn3b, bn3b_ext)):
        nc.sync.dma_start(out=t, in_=e[:, :])

    nc.vector.memset(act1, 0.0)

    ident = singles.tile([128, 128], bf16)
    make_identity(nc, ident)

    # stats accumulators
    sum1 = singles.tile([128, 2 * bl], f32)
    sq1 = singles.tile([128, 2 * bl], f32)
    sum2 = singles.tile([128, 2 * bl], f32)
    sq2 = singles.tile([128, 2 * bl], f32)
    sq3 = singles.tile([128, 8 * bl], f32)
    S2acc = singles.tile([128, 2 * bl], f32)

    # ---------------- gates ----------------
    g1s = singles.tile([128, 2, bl], f32)
    g2s = singles.tile([128, 2, bl], f32)
    for gs, wgs, bgs, gext in ((g1s, wg1s, bg1, g1_ext),
                               (g2s, wg2s, bg2, g2_ext)):
        for mo in range(2):
            gps = psB.tile([128, bl], f32, tag="small", name=f"gps_{mo}")
            nc.tensor.matmul(gps[:, :], lhsT=wgs[:, mo * 128:(mo + 1) * 128],
                             rhs=embts, start=True, stop=True)
            nc.scalar.activation(out=gs[:, mo, :], in_=gps[:, :], func=AF.Relu,
                                 bias=bgs[:, mo:mo + 1], scale=1.0)
        for mo in range(2):
            nc.sync.dma_start(
                out=gext[:, mo * 128:(mo + 1) * 128].rearrange("b ki -> ki b"),
                in_=gs[:, mo, :])

    def cross_core_sum(tag, part, nch, cc_in, cc_out):
        """AllGather a [128, nch] fp32 partial block, then sum over the 8
        cores with ones-vector matmuls on the tensor engine.
        Returns a [128, nch] fp32 tile of global sums."""
        nc.sync.dma_start(out=cc_in[:, :], in_=part)
        nc.gpsimd.collective_compute(
            "AllGather", ALU.bypass, replica_groups=rg,
            ins=[cc_in[:, :]], outs=[cc_out[:, :]])
        gath = scratch_p.tile([8, 128 * nch], f32, tag=f"gath{nch}",
                              name=f"gath_{tag}", bufs=1)
        nc.sync.dma_start(out=gath, in_=cc_out[:, :])
        gv = gath.rearrange("c (ki n) -> c ki n", n=nch)
        ps = psB.tile([128, nch], f32, tag="small", name=f"ccps_{tag}")
        for j in range(nch):
            nc.tensor.matmul(ps[:, j:j + 1], lhsT=gv[:, :, j], rhs=ones8,
                             start=True, stop=True)
        g = singles.tile([128, nch], f32, name=f"glob_{tag}")
        nc.vector.tensor_copy(out=g, in_=ps)
        return g

    def bn_scale_bias(glob, soff, qoff, gamma, beta, nch, tag):
        """glob: [128, *] with sums at soff and sum-sq at qoff.
        Returns (s, t): s = gamma*rsqrt(var+eps), t = beta - mean*s."""
        m = singles.tile([128, nch], f32, name=f"m_{tag}")
        ey2 = singles.tile([128, nch], f32, name=f"ey2_{tag}")
        nc.vector.tensor_scalar_mul(m, glob[:, soff:soff + nch], 1.0 / ntot)
        nc.vector.tensor_scalar_mul(ey2, glob[:, qoff:qoff + nch], 1.0 / ntot)
        var = singles.tile([128, nch], f32, name=f"var_{tag}")
        nc.vector.tensor_tensor(out=var, in0=m, in1=m, op=ALU.mult)
        nc.vector.tensor_tensor(out=var, in0=ey2, in1=var, op=ALU.subtract)
        std = singles.tile([128, nch], f32, name=f"std_{tag}")
        nc.vector.tensor_scalar_add(var, var, EPS)
        nc.scalar.activation(out=std, in_=var, func=AF.Sqrt)
        rstd = singles.tile([128, nch], f32, name=f"rstd_{tag}")
        nc.vector.reciprocal(rstd, std)
        s = singles.tile([128, nch], f32, name=f"s_{tag}")
        t = singles.tile([128, nch], f32, name=f"t_{tag}")
        nc.vector.tensor_tensor(out=s, in0=gamma, in1=rstd, op=ALU.mult)
        nc.vector.tensor_tensor(out=t, in0=m, in1=s, op=ALU.mult)
        nc.vector.tensor_tensor(out=t, in0=beta, in1=t, op=ALU.subtract)
        return s, t

    # ---------------- phase 1: load x, conv1, bn1 stats ----------------
    for b in range(bl):
        for ko in range(8):
            if (b, ko) in xsts:
                xst = xsts[(b, ko)]
            else:
                xst = xload.tile([128, HWP], f32, tag="xst",
                                 name=f"xst_{b}_{ko}")
                nc.sync.dma_start(out=xst,
                                  in_=x_ext[b, ko * 128:(ko + 1) * 128, :])
            nc.vector.tensor_copy(out=xres[:, ko, b, :], in_=xst)
        pts = [psA.tile([128, 2, 512], f32, tag="mm", name=f"c1_{b}_{mo}")
               for mo in range(2)]
        for ko in range(8):
            for mo in range(2):
                for n in range(2):
                    nc.tensor.matmul(
                        pts[mo][:, n, :NH],
                        lhsT=w1s[:, ko, mo * 128:(mo + 1) * 128],
                        rhs=xres[:, ko, b, n * NH:(n + 1) * NH],
                        start=(ko == 0), stop=(ko == 7))
        for mo in range(2):
            # gated PSUM->SBUF into padded act1 interior, fused sum
            dst = act1[:, mo, b, :].rearrange("p (r c) -> p r c", c=30)[
                :, 1:29, 1:29].rearrange("p (u r) c -> p u r c", u=2)
            src = pts[mo][:, :, :NH].rearrange("p u (r c) -> p u r c", c=28)
            idx = mo * bl + b
            nc.vector.tensor_scalar(
                out=dst, in0=src, scalar1=g1s[:, mo, b:b + 1], scalar2=None,
                op0=ALU.mult, op1=ALU.add, accum_out=sum1[:, idx:idx + 1])
            sc = scratch_p.tile([128, HWP], bf16, tag="sq", name=f"sq1_{b}_{mo}")
            dst3 = act1[:, mo, b, :].rearrange("p (r c) -> p r c", c=30)[
                :, 1:29, 1:29]
            nc.scalar.activation(
                out=sc.rearrange("p (r c) -> p r c", c=28),
                in_=dst3, func=AF.Square,
                accum_out=sq1[:, idx:idx + 1])

    # bn1 sync
    p1 = singles.tile([128, 4], f32)
    nc.vector.tensor_reduce(out=p1[:, 0:2], in_=sum1.rearrange(
        "p (mo b) -> p mo b", b=bl), axis=X, op=ALU.add)
    nc.vector.tensor_reduce(out=p1[:, 2:4], in_=sq1.rearrange(
        "p (mo b) -> p mo b", b=bl), axis=X, op=ALU.add)
    glob1 = cross_core_sum("bn1", p1, 4, cc1_in, cc1_out)
    s1, t1 = bn_scale_bias(glob1, 0, 2, bn1g, bn1b, 2, "bn1")

    # bn1 apply (+ReLU), in place, chunked per sample
    for b in range(bl):
        for mo in range(2):
            v = act1[:, mo, b, :].rearrange("p (r c) -> p r c", c=30)[
                :, 1:29, 1:29]
            nc.scalar.activation(out=v, in_=v, func=AF.Relu,
                                 bias=t1[:, mo:mo + 1], scale=s1[:, mo:mo + 1])

    # ---------------- phase 2: conv2 3x3, bn2 stats ----------------
    for mo in range(2):
        for bp in range((bl + 1) // 2):
            pair = tuple(range(2 * bp, min(2 * bp + 2, bl)))
            pts = {b: psA.tile([128, 2, 512], f32, tag="mm",
                               name=f"c2_{b}_{mo}") for b in pair}
            a1v = {b: act1[:, :, b, :].rearrange("p k (r c) -> p k r c", c=30)
                   for b in pair}
            k = 0
            for ko in range(2):
                for tap in range(9):
                    dy, dx = tap // 3, tap % 3
                    for b in pair:
                        for rc in range(2):
                            rhs = a1v[b][:, ko,
                                         14 * rc + dy:14 * rc + dy + 14,
                                         dx:dx + 28]
                            nc.tensor.matmul(
                                pts[b][:, rc, :NH],
                                lhsT=w2s[:, ko, tap, mo * 128:(mo + 1) * 128],
                                rhs=rhs, start=(k == 0), stop=(k == 17))
                    k += 1
            for b in pair:
                dst = act2[:, mo, b, :].rearrange("p (u r c) -> p u r c",
                                                  u=2, c=28)
                src = pts[b][:, :, :NH].rearrange("p u (r c) -> p u r c", c=28)
                idx = mo * bl + b
                nc.vector.tensor_scalar(
                    out=dst, in0=src, scalar1=g2s[:, mo, b:b + 1],
                    scalar2=None, op0=ALU.mult, op1=ALU.add,
                    accum_out=sum2[:, idx:idx + 1])
                sc = scratch_p.tile([128, HWP], bf16, tag="sq",
                                    name=f"sq2_{b}_{mo}")
                nc.scalar.activation(
                    out=sc.rearrange("p (u c) -> p u c", u=2),
                    in_=act2[:, mo, b, :].rearrange("p (u c) -> p u c", u=2),
                    func=AF.Square, accum_out=sq2[:, idx:idx + 1])

    # bn2 sync
    p2 = singles.tile([128, 4], f32)
    nc.vector.tensor_reduce(out=p2[:, 0:2], in_=sum2.rearrange(
        "p (mo b) -> p mo b", b=bl), axis=X, op=ALU.add)
    nc.vector.tensor_reduce(out=p2[:, 2:4], in_=sq2.rearrange(
        "p (mo b) -> p mo b", b=bl), axis=X, op=ALU.add)
    glob2 = cross_core_sum("bn2", p2, 4, cc2_in, cc2_out)
    s2, t2 = bn_scale_bias(glob2, 0, 2, bn2g, bn2b, 2, "bn2")

    # bn2 apply (+ReLU) with fused S2 = sum(act2) accumulation
    for b in range(bl):
        for mo in range(2):
            v = act2[:, mo, b, :]
            idx = mo * bl + b
            nc.scalar.activation(out=v, in_=v, func=AF.Relu,
                                 bias=t2[:, mo:mo + 1], scale=s2[:, mo:mo + 1],
                                 accum_out=S2acc[:, idx:idx + 1])

    # ---------------- phase 3: conv3 pass 1 (stats only) ----------------
    for mo in range(8):
        for bp in range((bl + 1) // 2):
            pair = tuple(range(2 * bp, min(2 * bp + 2, bl)))
            pts = {b: psA.tile([128, 2, 512], f32, tag="mm",
                               name=f"c3a_{b}_{mo}") for b in pair}
            for ko in range(2):
                for b in pair:
                    for n in range(2):
                        nc.tensor.matmul(
                            pts[b][:, n, :NH],
                            lhsT=w3s[:, ko, mo * 128:(mo + 1) * 128],
                            rhs=act2[:, ko, b, n * NH:(n + 1) * NH],
                            start=(ko == 0), stop=(ko == 1))
            for b in pair:
                sc = scratch_p.tile([128, HWP], bf16, tag="sq",
                                    name=f"sq3_{b}_{mo}")
                idx = mo * bl + b
                nc.scalar.activation(
                    out=sc.rearrange("p (u c) -> p u c", u=2),
                    in_=pts[b][:, :, :NH], func=AF.Square,
                    accum_out=sq3[:, idx:idx + 1])

    # bn3 sync: sum(y^2) partials + S2 partials in one AllGather
    p3 = singles.tile([128, 10], f32)
    nc.vector.tensor_reduce(out=p3[:, 0:8], in_=sq3.rearrange(
        "p (mo b) -> p mo b", b=bl), axis=X, op=ALU.add)
    nc.vector.tensor_reduce(out=p3[:, 8:10], in_=S2acc.rearrange(
        "p (mo b) -> p mo b", b=bl), axis=X, op=ALU.add)
    glob3 = cross_core_sum("bn3", p3, 10, cc3_in, cc3_out)

    # mean3 via W3 @ S2  (exploits linearity of the 1x1 conv)
    S2b = singles.tile([128, 2], bf16)
    nc.vector.tensor_copy(out=S2b, in_=glob3[:, 8:10])
    pm = psB.tile([128, 8], f32, tag="small")
    for mo in range(8):
        for ko in range(2):
            nc.tensor.matmul(pm[:, mo:mo + 1],
                             lhsT=w3s[:, ko, mo * 128:(mo + 1) * 128],
                             rhs=S2b[:, ko:ko + 1],
                             start=(ko == 0), stop=(ko == 1))
    g3full = singles.tile([128, 16], f32)
    nc.vector.tensor_copy(out=g3full[:, 0:8], in_=pm)
    nc.vector.tensor_copy(out=g3full[:, 8:16], in_=glob3[:, 0:8])
    s3, t3 = bn_scale_bias(g3full, 0, 8, bn3g, bn3b, 8, "bn3")

    # diag(1/s3) in bf16 for the in-PSUM residual add
    invs3 = singles.tile([128, 8], f32)
    nc.vector.reciprocal(invs3, s3)
    diag3 = singles.tile([128, 8, 128], bf16)
    for mo in range(8):
        nc.vector.tensor_scalar_mul(diag3[:, mo, :], ident,
                                    invs3[:, mo:mo + 1])

    # ---------------- phase 4: conv3 pass 2 + residual + out ----------------
    for mo in range(8):
        for bp in range((bl + 1) // 2):
            pair = tuple(range(2 * bp, min(2 * bp + 2, bl)))
            pts = {b: psA.tile([128, 2, 512], f32, tag="mm",
                               name=f"c3b_{b}_{mo}") for b in pair}
            for ko in range(2):
                for b in pair:
                    for n in range(2):
                        nc.tensor.matmul(
                            pts[b][:, n, :NH],
                            lhsT=w3s[:, ko, mo * 128:(mo + 1) * 128],
                            rhs=act2[:, ko, b, n * NH:(n + 1) * NH],
                            start=(ko == 0), stop=False)
            for b in pair:
                for n in range(2):
                    nc.tensor.matmul(
                        pts[b][:, n, :NH],
                        lhsT=diag3[:, mo, :],
                        rhs=xres[:, mo, b, n * NH:(n + 1) * NH],
                        start=False, stop=True)
            for b in pair:
                ost = ostage_p.tile([128, HWP], f32, tag="ost",
                                    name=f"ost_{b}_{mo}")
                if (b * 8 + mo) % 2 == 0:
                    nc.scalar.activation(
                        out=ost.rearrange("p (u c) -> p u c", u=2),
                        in_=pts[b][:, :, :NH], func=AF.Relu,
                        scale=s3[:, mo:mo + 1], bias=t3[:, mo:mo + 1])
                else:
                    nc.vector.tensor_scalar(
                        out=ost.rearrange("p (u c) -> p u c", u=2),
                        in0=pts[b][:, :, :NH], scalar1=s3[:, mo:mo + 1],
                        scalar2=t3[:, mo:mo + 1], op0=ALU.mult, op1=ALU.add)
                    nc.vector.tensor_scalar_max(ost, ost, 0.0)
                nc.sync.dma_start(out=out_ext[b, mo * 128:(mo + 1) * 128, :],
                                  in_=ost)


def build(bl=BL):
    nc = bacc.Bacc("TRN2", target_bir_lowering=False, debug=False,
                   num_devices=N_CORES)
    from contextlib import ExitStack
    with tile.TileContext(nc) as tc, ExitStack() as ctx:
        _emit(nc, tc, ctx, bl)
    nc.compile()
    return nc


def prep_weights(inputs):
    """Host-side reshape/cast of the (small) replicated weights."""
    w1 = np.asarray(inputs["conv1_w"], np.float32).reshape(MID, CIN)
    w2 = np.asarray(inputs["conv2_w"], np.float32)
    w3 = np.asarray(inputs["conv3_w"], np.float32).reshape(COUT, MID)

    w1t = np.ascontiguousarray(
        w1.reshape(MID, 8, 128).transpose(2, 1, 0)).astype(bfnp)
    w2t = np.ascontiguousarray(
        w2.reshape(MID, 2, 128, 9).transpose(2, 1, 3, 0)).astype(bfnp)
    w3t = np.ascontiguousarray(
        w3.reshape(COUT, 2, 128).transpose(2, 1, 0)).astype(bfnp)

    def chan_tile(v, nch):
        return np.ascontiguousarray(
            np.asarray(v, np.float32).reshape(nch, 128).T)

    return {
        "w1t": w1t, "w2t": w2t, "w3t": w3t,
        "wg1": np.asarray(inputs["w_gate1"], np.float32).astype(bfnp),
        "wg2": np.asarray(inputs["w_gate2"], np.float32).astype(bfnp),
        "bg1": chan_tile(inputs["b_gate1"], 2),
        "bg2": chan_tile(inputs["b_gate2"], 2),
        "bn1g": chan_tile(inputs["bn1_g"], 2),
        "bn1b": chan_tile(inputs["bn1_b"], 2),
        "bn2g": chan_tile(inputs["bn2_g"], 2),
        "bn2b": chan_tile(inputs["bn2_b"], 2),
        "bn3g": chan_tile(inputs["bn3_g"], 8),
        "bn3b": chan_tile(inputs["bn3_b"], 8),
    }


def make_in_maps(inputs, bl=BL):
    x = np.asarray(inputs["x"], np.float32).reshape(B, CIN, HWP)
    emb = np.asarray(inputs["embeddings"], np.float32)
    w = prep_weights(inputs)
    in_maps = []
    for c in range(N_CORES):
        sl = slice(c * bl, (c + 1) * bl)
        m = dict(w)
        m["x"] = x[sl]
        m["embt"] = np.ascontiguousarray(emb[sl].T).astype(bfnp)
        in_maps.append(m)
    return in_maps


_built = {}


def _get_nc():
    if "nc" not in _built:
        _built["nc"] = build(BL)
    return _built["nc"]


def kernel(**inputs):
    from concourse.bass_utils import run_bass_kernel_spmd
    nc = _get_nc()
    in_maps = make_in_maps(inputs)
    res = run_bass_kernel_spmd(nc, in_maps, list(range(N_CORES)))
    outs = [r["out"].reshape(BL, COUT, H, W) for r in res.results]
    g1s = [r["g1"].reshape(BL, MID) for r in res.results]
    g2s = [r["g2"].reshape(BL, MID) for r in res.results]
    out = np.concatenate(outs, axis=0)
    g1 = np.concatenate(g1s, axis=0)
    g2 = np.concatenate(g2s, axis=0)
    return out, g1, g2


if __name__ == "__main__":
    nc = build(BL)
    print("built + compiled ok")


# revision 19
# speedup vs baseline: 1.2649x; 1.0160x over previous
"""Trainium2 Bass kernel for nn_MoEBottleneckA (MoE bottleneck block).

Strategy: data-parallel over batch (64 -> 8 samples per core, 8 cores),
weights replicated. Training-mode BatchNorm stats are synchronized with
small AllGather collectives (sync-BN). All matmuls in bf16 (fp32 PSUM
accumulation); BN statistics and normalization in fp32.

Per core:
  conv1 (1x1, 1024->256) as 8-chunk K-accumulated matmuls, gated by g1,
  BN1 partial stats fused into the PSUM->SBUF epilogues; AllGather;
  BN1 apply (+ReLU) in place; conv2 (3x3 SAME) as 18 shifted matmuls on a
  zero-padded 30x30 activation; gate g2 + BN2 stats; AllGather; BN2 apply
  with fused S2 = sum(act2) accumulation; conv3 (1x1, 256->1024) pass 1
  computes sum(y^2) partials (mean comes free via W3 @ S2); AllGather;
  conv3 pass 2 recomputes y, adds the residual inside PSUM via a
  diag(1/s3) matmul of x, and Relu(scale,bias) epilogues (split between
  the scalar and vector engines) write the final fp32 output.

Cross-core partial sums travel as [128, n] contiguous blocks; the
per-core reduction of the gathered [8, 128*n] block is done on the
tensor engine (ones-vector matmul), which keeps the sync window short.
"""
import sys

for _p in ("/opt/trn_rl_repo", "/root/.axon_site/_ro/trn_rl_repo"):
    if _p not in sys.path:
        sys.path.append(_p)

import numpy as np
import ml_dtypes

import concourse.bass as bass
import concourse.mybir as mybir
import concourse.tile as tile
from concourse import bacc
from concourse.masks import make_identity

N_CORES = 8
B, CIN, H, W = 64, 1024, 28, 28
MID, COUT, EMB = 256, 1024, 64
EPS = 1e-5
HWP = H * W          # 784
NH = 392             # matmul free-dim chunk (2 per 784)
BL = B // N_CORES    # samples per core
NTOT = float(B * HWP)

f32 = mybir.dt.float32
bf16 = mybir.dt.bfloat16
AF = mybir.ActivationFunctionType
ALU = mybir.AluOpType
X = mybir.AxisListType.X

bfnp = ml_dtypes.bfloat16


def _emit(nc, tc, ctx, bl):
    ntot = float(N_CORES * bl * HWP)
    rg = [list(range(N_CORES))]

    # ---------------- DRAM I/O ----------------
    x_ext = nc.dram_tensor("x", [bl, CIN, HWP], f32, kind="ExternalInput")
    embt_ext = nc.dram_tensor("embt", [EMB, bl], bf16, kind="ExternalInput")
    w1_ext = nc.dram_tensor("w1t", [128, 8, MID], bf16, kind="ExternalInput")
    w2_ext = nc.dram_tensor("w2t", [128, 2, 9, MID], bf16, kind="ExternalInput")
    w3_ext = nc.dram_tensor("w3t", [128, 2, COUT], bf16, kind="ExternalInput")
    wg1_ext = nc.dram_tensor("wg1", [EMB, MID], bf16, kind="ExternalInput")
    wg2_ext = nc.dram_tensor("wg2", [EMB, MID], bf16, kind="ExternalInput")
    bg1_ext = nc.dram_tensor("bg1", [128, 2], f32, kind="ExternalInput")
    bg2_ext = nc.dram_tensor("bg2", [128, 2], f32, kind="ExternalInput")
    bn1g_ext = nc.dram_tensor("bn1g", [128, 2], f32, kind="ExternalInput")
    bn1b_ext = nc.dram_tensor("bn1b", [128, 2], f32, kind="ExternalInput")
    bn2g_ext = nc.dram_tensor("bn2g", [128, 2], f32, kind="ExternalInput")
    bn2b_ext = nc.dram_tensor("bn2b", [128, 2], f32, kind="ExternalInput")
    bn3g_ext = nc.dram_tensor("bn3g", [128, 8], f32, kind="ExternalInput")
    bn3b_ext = nc.dram_tensor("bn3b", [128, 8], f32, kind="ExternalInput")

    out_ext = nc.dram_tensor("out", [bl, COUT, HWP], f32, kind="ExternalOutput")
    g1_ext = nc.dram_tensor("g1", [bl, MID], f32, kind="ExternalOutput")
    g2_ext = nc.dram_tensor("g2", [bl, MID], f32, kind="ExternalOutput")

    # internal DRAM bounce buffers for the sync-BN collectives
    ccw_in = nc.dram_tensor("ccw_in", [8], f32)
    ccw_out = nc.dram_tensor("ccw_out", [N_CORES, 8], f32, addr_space="Shared")
    cc1_in = nc.dram_tensor("cc1_in", [128, 4], f32)
    cc1_out = nc.dram_tensor("cc1_out", [N_CORES, 128 * 4], f32,
                             addr_space="Shared")
    cc2_in = nc.dram_tensor("cc2_in", [128, 4], f32)
    cc2_out = nc.dram_tensor("cc2_out", [N_CORES, 128 * 4], f32,
                             addr_space="Shared")
    cc3_in = nc.dram_tensor("cc3_in", [128, 10], f32)
    cc3_out = nc.dram_tensor("cc3_out", [N_CORES, 128 * 10], f32,
                             addr_space="Shared")

    # ---------------- pools ----------------
    singles = ctx.enter_context(tc.tile_pool(name="singles", bufs=1))
    xload = ctx.enter_context(tc.tile_pool(name="xload", bufs=5))
    ostage_p = ctx.enter_context(tc.tile_pool(name="ostage", bufs=4))
    scratch_p = ctx.enter_context(tc.tile_pool(name="scratch", bufs=2))
    psA = ctx.enter_context(tc.tile_pool(name="psA", bufs=3, space="PSUM"))
    psB = ctx.enter_context(tc.tile_pool(name="psB", bufs=2, space="PSUM"))

    # ---------------- persistent SBUF ----------------
    xres = singles.tile([128, 8, bl, HWP], bf16)
    act1 = singles.tile([128, 2, bl, 900], bf16)   # zero-padded 30x30
    act2 = singles.tile([128, 2, bl, HWP], bf16)

    # x for the first samples first: conv1 starts as soon as possible
    w1s = singles.tile([128, 8, MID], bf16)
    xsts = {}
    for b in range(min(2, bl)):
        for ko in range(8):
            xst = xload.tile([128, HWP], f32, tag="xst", name=f"xst_{b}_{ko}")
            nc.sync.dma_start(out=xst, in_=x_ext[b, ko * 128:(ko + 1) * 128, :])
            xsts[(b, ko)] = xst
    nc.sync.dma_start(out=w1s, in_=w1_ext[:, :, :])

    # warm up the collectives firmware with a dummy 8-float AllGather
    ones8 = singles.tile([8, 1], f32)
    nc.vector.memset(ones8, 1.0)
    nc.sync.dma_start(out=ccw_in[:], in_=ones8[:, 0])
    nc.gpsimd.collective_compute(
        "AllGather", ALU.bypass, replica_groups=rg,
        ins=[ccw_in[:]], outs=[ccw_out[:, :]])

    w2s = singles.tile([128, 2, 9, MID], bf16)
    w3s = singles.tile([128, 2, COUT], bf16)
    wg1s = singles.tile([EMB, MID], bf16)
    wg2s = singles.tile([EMB, MID], bf16)
    embts = singles.tile([EMB, bl], bf16)
    nc.sync.dma_start(out=wg1s, in_=wg1_ext[:, :])
    nc.sync.dma_start(out=wg2s, in_=wg2_ext[:, :])
    nc.sync.dma_start(out=embts, in_=embt_ext[:, :])
    nc.sync.dma_start(out=w2s, in_=w2_ext[:, :, :, :])
    nc.sync.dma_start(out=w3s, in_=w3_ext[:, :, :])

    bg1 = singles.tile([128, 2], f32)
    bg2 = singles.tile([128, 2], f32)
    bn1g = singles.tile([128, 2], f32)
    bn1b = singles.tile([128, 2], f32)
    bn2g = singles.tile([128, 2], f32)
    bn2b = singles.tile([128, 2], f32)
    bn3g = singles.tile([128, 8], f32)
    bn3b = singles.tile([128, 8], f32)
    for t, e in ((bg1, bg1_ext), (bg2, bg2_ext), (bn1g, bn1g_ext),
                 (bn1b, bn1b_ext), (bn2g, bn2g_ext), (bn2b, bn2b_ext),
                 (bn3g, bn3g_ext), (bn3b, bn3b_ext)):
        nc.sync.dma_start(out=t, in_=e[:, :])

    nc.gpsimd.memset(act1, 0.0)

    ident = singles.tile([128, 128], bf16)
    make_identity(nc, ident)

    # stats accumulators
    sum1 = singles.tile([128, 2 * bl], f32)
    sq1 = singles.tile([128, 2 * bl], f32)
    sum2 = singles.tile([128, 2 * bl], f32)
    sq2 = singles.tile([128, 2 * bl], f32)
    sq3 = singles.tile([128, 8 * bl], f32)
    S2acc = singles.tile([128, 2 * bl], f32)

    # ---------------- gates ----------------
    g1s = singles.tile([128, 2, bl], f32)
    g2s = singles.tile([128, 2, bl], f32)
    for gs, wgs, bgs, gext in ((g1s, wg1s, bg1, g1_ext),
                               (g2s, wg2s, bg2, g2_ext)):
        for mo in range(2):
            gps = psB.tile([128, bl], f32, tag="small", name=f"gps_{mo}")
            nc.tensor.matmul(gps[:, :], lhsT=wgs[:, mo * 128:(mo + 1) * 128],
                             rhs=embts, start=True, stop=True)
            nc.scalar.activation(out=gs[:, mo, :], in_=gps[:, :], func=AF.Relu,
                                 bias=bgs[:, mo:mo + 1], scale=1.0)
        for mo in range(2):
            nc.sync.dma_start(
                out=gext[:, mo * 128:(mo + 1) * 128].rearrange("b ki -> ki b"),
                in_=gs[:, mo, :])

    def cross_core_sum(tag, part, nch, cc_in, cc_out):
        """AllGather a [128, nch] fp32 partial block, then sum over the 8
        cores with ones-vector matmuls on the tensor engine.
        Returns a [128, nch] fp32 tile of global sums."""
        nc.sync.dma_start(out=cc_in[:, :], in_=part)
        nc.gpsimd.collective_compute(
            "AllGather", ALU.bypass, replica_groups=rg,
            ins=[cc_in[:, :]], outs=[cc_out[:, :]])
        gath = scratch_p.tile([8, 128 * nch], f32, tag=f"gath{nch}",
                              name=f"gath_{tag}", bufs=1)
        nc.sync.dma_start(out=gath, in_=cc_out[:, :])
        gv = gath.rearrange("c (ki n) -> c ki n", n=nch)
        ps = psB.tile([128, nch], f32, tag="small", name=f"ccps_{tag}")
        for j in range(nch):
            nc.tensor.matmul(ps[:, j:j + 1], lhsT=gv[:, :, j], rhs=ones8,
                             start=True, stop=True)
        g = singles.tile([128, nch], f32, name=f"glob_{tag}")
        nc.vector.tensor_copy(out=g, in_=ps)
        return g

    def bn_scale_bias(glob, soff, qoff, gamma, beta, nch, tag):
        """glob: [128, *] with sums at soff and sum-sq at qoff.
        Returns (s, t): s = gamma*rsqrt(var+eps), t = beta - mean*s."""
        m = singles.tile([128, nch], f32, name=f"m_{tag}")
        ey2 = singles.tile([128, nch], f32, name=f"ey2_{tag}")
        nc.vector.tensor_scalar_mul(m, glob[:, soff:soff + nch], 1.0 / ntot)
        nc.vector.tensor_scalar_mul(ey2, glob[:, qoff:qoff + nch], 1.0 / ntot)
        var = singles.tile([128, nch], f32, name=f"var_{tag}")
        nc.vector.tensor_tensor(out=var, in0=m, in1=m, op=ALU.mult)
        nc.vector.tensor_tensor(out=var, in0=ey2, in1=var, op=ALU.subtract)
        std = singles.tile([128, nch], f32, name=f"std_{tag}")
        nc.vector.tensor_scalar_add(var, var, EPS)
        nc.scalar.activation(out=std, in_=var, func=AF.Sqrt)
        rstd = singles.tile([128, nch], f32, name=f"rstd_{tag}")
        nc.vector.reciprocal(rstd, std)
        s = singles.tile([128, nch], f32, name=f"s_{tag}")
        t = singles.tile([128, nch], f32, name=f"t_{tag}")
        nc.vector.tensor_tensor(out=s, in0=gamma, in1=rstd, op=ALU.mult)
        nc.vector.tensor_tensor(out=t, in0=m, in1=s, op=ALU.mult)
        nc.vector.tensor_tensor(out=t, in0=beta, in1=t, op=ALU.subtract)
        return s, t

    # ---------------- phase 1: load x, conv1, bn1 stats ----------------
    for b in range(bl):
        for ko in range(8):
            if (b, ko) in xsts:
                xst = xsts[(b, ko)]
            else:
                xst = xload.tile([128, HWP], f32, tag="xst",
                                 name=f"xst_{b}_{ko}")
                nc.sync.dma_start(out=xst,
                                  in_=x_ext[b, ko * 128:(ko + 1) * 128, :])
            nc.vector.tensor_copy(out=xres[:, ko, b, :], in_=xst)
        pts = [psA.tile([128, 2, 512], f32, tag="mm", name=f"c1_{b}_{mo}")
               for mo in range(2)]
        for ko in range(8):
            for mo in range(2):
                for n in range(2):
                    nc.tensor.matmul(
                        pts[mo][:, n, :NH],
                        lhsT=w1s[:, ko, mo * 128:(mo + 1) * 128],
                        rhs=xres[:, ko, b, n * NH:(n + 1) * NH],
                        start=(ko == 0), stop=(ko == 7))
        for mo in range(2):
            # gated PSUM->SBUF into padded act1 interior, fused sum
            dst = act1[:, mo, b, :].rearrange("p (r c) -> p r c", c=30)[
                :, 1:29, 1:29].rearrange("p (u r) c -> p u r c", u=2)
            src = pts[mo][:, :, :NH].rearrange("p u (r c) -> p u r c", c=28)
            idx = mo * bl + b
            nc.vector.tensor_scalar(
                out=dst, in0=src, scalar1=g1s[:, mo, b:b + 1], scalar2=None,
                op0=ALU.mult, op1=ALU.add, accum_out=sum1[:, idx:idx + 1])
            sc = ostage_p.tile([128, HWP], bf16, tag="ost", name=f"sq1_{b}_{mo}")
            dst3 = act1[:, mo, b, :].rearrange("p (r c) -> p r c", c=30)[
                :, 1:29, 1:29]
            nc.scalar.activation(
                out=sc.rearrange("p (r c) -> p r c", c=28),
                in_=dst3, func=AF.Square,
                accum_out=sq1[:, idx:idx + 1])

    # bn1 sync
    p1 = singles.tile([128, 4], f32)
    nc.vector.tensor_reduce(out=p1[:, 0:2], in_=sum1.rearrange(
        "p (mo b) -> p mo b", b=bl), axis=X, op=ALU.add)
    nc.vector.tensor_reduce(out=p1[:, 2:4], in_=sq1.rearrange(
        "p (mo b) -> p mo b", b=bl), axis=X, op=ALU.add)
    glob1 = cross_core_sum("bn1", p1, 4, cc1_in, cc1_out)
    s1, t1 = bn_scale_bias(glob1, 0, 2, bn1g, bn1b, 2, "bn1")

    # bn1 apply (+ReLU), in place, chunked per sample
    for b in range(bl):
        for mo in range(2):
            v = act1[:, mo, b, :].rearrange("p (r c) -> p r c", c=30)[
                :, 1:29, 1:29]
            nc.scalar.activation(out=v, in_=v, func=AF.Relu,
                                 bias=t1[:, mo:mo + 1], scale=s1[:, mo:mo + 1])

    # ---------------- phase 2: conv2 3x3, bn2 stats ----------------
    for mo in range(2):
        for bp in range((bl + 1) // 2):
            pair = tuple(range(2 * bp, min(2 * bp + 2, bl)))
            pts = {b: psA.tile([128, 2, 512], f32, tag="mm",
                               name=f"c2_{b}_{mo}") for b in pair}
            a1v = {b: act1[:, :, b, :].rearrange("p k (r c) -> p k r c", c=30)
                   for b in pair}
            k = 0
            for ko in range(2):
                for tap in range(9):
                    dy, dx = tap // 3, tap % 3
                    for b in pair:
                        for rc in range(2):
                            rhs = a1v[b][:, ko,
                                         14 * rc + dy:14 * rc + dy + 14,
                                         dx:dx + 28]
                            nc.tensor.matmul(
                                pts[b][:, rc, :NH],
                                lhsT=w2s[:, ko, tap, mo * 128:(mo + 1) * 128],
                                rhs=rhs, start=(k == 0), stop=(k == 17))
                    k += 1
            for b in pair:
                dst = act2[:, mo, b, :].rearrange("p (u r c) -> p u r c",
                                                  u=2, c=28)
                src = pts[b][:, :, :NH].rearrange("p u (r c) -> p u r c", c=28)
                idx = mo * bl + b
                nc.vector.tensor_scalar(
                    out=dst, in0=src, scalar1=g2s[:, mo, b:b + 1],
                    scalar2=None, op0=ALU.mult, op1=ALU.add,
                    accum_out=sum2[:, idx:idx + 1])
                sc = ostage_p.tile([128, HWP], bf16, tag="ost",
                                    name=f"sq2_{b}_{mo}")
                nc.scalar.activation(
                    out=sc.rearrange("p (u c) -> p u c", u=2),
                    in_=act2[:, mo, b, :].rearrange("p (u c) -> p u c", u=2),
                    func=AF.Square, accum_out=sq2[:, idx:idx + 1])

    # bn2 sync
    p2 = singles.tile([128, 4], f32)
    nc.vector.tensor_reduce(out=p2[:, 0:2], in_=sum2.rearrange(
        "p (mo b) -> p mo b", b=bl), axis=X, op=ALU.add)
    nc.vector.tensor_reduce(out=p2[:, 2:4], in_=sq2.rearrange(
        "p (mo b) -> p mo b", b=bl), axis=X, op=ALU.add)
    glob2 = cross_core_sum("bn2", p2, 4, cc2_in, cc2_out)
    s2, t2 = bn_scale_bias(glob2, 0, 2, bn2g, bn2b, 2, "bn2")

    # bn2 apply (+ReLU) with fused S2 = sum(act2) accumulation
    for b in range(bl):
        for mo in range(2):
            v = act2[:, mo, b, :]
            idx = mo * bl + b
            nc.scalar.activation(out=v, in_=v, func=AF.Relu,
                                 bias=t2[:, mo:mo + 1], scale=s2[:, mo:mo + 1],
                                 accum_out=S2acc[:, idx:idx + 1])

    # ---------------- phase 3: conv3 pass 1 (stats only) ----------------
    for mo in range(8):
        for bp in range((bl + 1) // 2):
            pair = tuple(range(2 * bp, min(2 * bp + 2, bl)))
            pts = {b: psA.tile([128, 2, 512], f32, tag="mm",
                               name=f"c3a_{b}_{mo}") for b in pair}
            for ko in range(2):
                for b in pair:
                    for n in range(2):
                        nc.tensor.matmul(
                            pts[b][:, n, :NH],
                            lhsT=w3s[:, ko, mo * 128:(mo + 1) * 128],
                            rhs=act2[:, ko, b, n * NH:(n + 1) * NH],
                            start=(ko == 0), stop=(ko == 1))
            for b in pair:
                sc = ostage_p.tile([128, HWP], bf16, tag="ost",
                                    name=f"sq3_{b}_{mo}")
                idx = mo * bl + b
                nc.scalar.activation(
                    out=sc.rearrange("p (u c) -> p u c", u=2),
                    in_=pts[b][:, :, :NH], func=AF.Square,
                    accum_out=sq3[:, idx:idx + 1])

    # bn3 sync: sum(y^2) partials + S2 partials in one AllGather
    p3 = singles.tile([128, 10], f32)
    nc.vector.tensor_reduce(out=p3[:, 0:8], in_=sq3.rearrange(
        "p (mo b) -> p mo b", b=bl), axis=X, op=ALU.add)
    nc.vector.tensor_reduce(out=p3[:, 8:10], in_=S2acc.rearrange(
        "p (mo b) -> p mo b", b=bl), axis=X, op=ALU.add)
    glob3 = cross_core_sum("bn3", p3, 10, cc3_in, cc3_out)

    # mean3 via W3 @ S2  (exploits linearity of the 1x1 conv)
    S2b = singles.tile([128, 2], bf16)
    nc.vector.tensor_copy(out=S2b, in_=glob3[:, 8:10])
    pm = psB.tile([128, 8], f32, tag="small")
    for mo in range(8):
        for ko in range(2):
            nc.tensor.matmul(pm[:, mo:mo + 1],
                             lhsT=w3s[:, ko, mo * 128:(mo + 1) * 128],
                             rhs=S2b[:, ko:ko + 1],
                             start=(ko == 0), stop=(ko == 1))
    g3full = singles.tile([128, 16], f32)
    nc.vector.tensor_copy(out=g3full[:, 0:8], in_=pm)
    nc.vector.tensor_copy(out=g3full[:, 8:16], in_=glob3[:, 0:8])
    s3, t3 = bn_scale_bias(g3full, 0, 8, bn3g, bn3b, 8, "bn3")

    # diag(1/s3) in bf16 for the in-PSUM residual add
    invs3 = singles.tile([128, 8], f32)
    nc.vector.reciprocal(invs3, s3)
    diag3 = singles.tile([128, 8, 128], bf16)
    for mo in range(8):
        nc.vector.tensor_scalar_mul(diag3[:, mo, :], ident,
                                    invs3[:, mo:mo + 1])

    # ---------------- phase 4: conv3 pass 2 + residual + out ----------------
    for mo in range(8):
        for bp in range((bl + 1) // 2):
            pair = tuple(range(2 * bp, min(2 * bp + 2, bl)))
            pts = {b: psA.tile([128, 2, 512], f32, tag="mm",
                               name=f"c3b_{b}_{mo}") for b in pair}
            for ko in range(2):
                for b in pair:
                    for n in range(2):
                        nc.tensor.matmul(
                            pts[b][:, n, :NH],
                            lhsT=w3s[:, ko, mo * 128:(mo + 1) * 128],
                            rhs=act2[:, ko, b, n * NH:(n + 1) * NH],
                            start=(ko == 0), stop=False)
            for b in pair:
                for n in range(2):
                    nc.tensor.matmul(
                        pts[b][:, n, :NH],
                        lhsT=diag3[:, mo, :],
                        rhs=xres[:, mo, b, n * NH:(n + 1) * NH],
                        start=False, stop=True)
            for b in pair:
                ost = ostage_p.tile([128, HWP], f32, tag="ost",
                                    name=f"ost_{b}_{mo}")
                if (b * 8 + mo) % 2 == 0:
                    nc.scalar.activation(
                        out=ost.rearrange("p (u c) -> p u c", u=2),
                        in_=pts[b][:, :, :NH], func=AF.Relu,
                        scale=s3[:, mo:mo + 1], bias=t3[:, mo:mo + 1])
                else:
                    nc.vector.tensor_scalar(
                        out=ost.rearrange("p (u c) -> p u c", u=2),
                        in0=pts[b][:, :, :NH], scalar1=s3[:, mo:mo + 1],
                        scalar2=t3[:, mo:mo + 1], op0=ALU.mult, op1=ALU.add)
                    nc.vector.tensor_scalar_max(ost, ost, 0.0)
                nc.sync.dma_start(out=out_ext[b, mo * 128:(mo + 1) * 128, :],
                                  in_=ost)


def build(bl=BL):
    nc = bacc.Bacc("TRN2", target_bir_lowering=False, debug=False,
                   num_devices=N_CORES)
    from contextlib import ExitStack
    with tile.TileContext(nc) as tc, ExitStack() as ctx:
        _emit(nc, tc, ctx, bl)
    nc.compile()
    return nc


def prep_weights(inputs):
    """Host-side reshape/cast of the (small) replicated weights."""
    w1 = np.asarray(inputs["conv1_w"], np.float32).reshape(MID, CIN)
    w2 = np.asarray(inputs["conv2_w"], np.float32)
    w3 = np.asarray(inputs["conv3_w"], np.float32).reshape(COUT, MID)

    w1t = np.ascontiguousarray(
        w1.reshape(MID, 8, 128).transpose(2, 1, 0)).astype(bfnp)
    w2t = np.ascontiguousarray(
        w2.reshape(MID, 2, 128, 9).transpose(2, 1, 3, 0)).astype(bfnp)
    w3t = np.ascontiguousarray(
        w3.reshape(COUT, 2, 128).transpose(2, 1, 0)).astype(bfnp)

    def chan_tile(v, nch):
        return np.ascontiguousarray(
            np.asarray(v, np.float32).reshape(nch, 128).T)

    return {
        "w1t": w1t, "w2t": w2t, "w3t": w3t,
        "wg1": np.asarray(inputs["w_gate1"], np.float32).astype(bfnp),
        "wg2": np.asarray(inputs["w_gate2"], np.float32).astype(bfnp),
        "bg1": chan_tile(inputs["b_gate1"], 2),
        "bg2": chan_tile(inputs["b_gate2"], 2),
        "bn1g": chan_tile(inputs["bn1_g"], 2),
        "bn1b": chan_tile(inputs["bn1_b"], 2),
        "bn2g": chan_tile(inputs["bn2_g"], 2),
        "bn2b": chan_tile(inputs["bn2_b"], 2),
        "bn3g": chan_tile(inputs["bn3_g"], 8),
        "bn3b": chan_tile(inputs["bn3_b"], 8),
    }


def make_in_maps(inputs, bl=BL):
    x = np.asarray(inputs["x"], np.float32).reshape(B, CIN, HWP)
    emb = np.asarray(inputs["embeddings"], np.float32)
    w = prep_weights(inputs)
    in_maps = []
    for c in range(N_CORES):
        sl = slice(c * bl, (c + 1) * bl)
        m = dict(w)
        m["x"] = x[sl]
        m["embt"] = np.ascontiguousarray(emb[sl].T).astype(bfnp)
        in_maps.append(m)
    return in_maps


_built = {}


def _get_nc():
    if "nc" not in _built:
        _built["nc"] = build(BL)
    return _built["nc"]


def kernel(**inputs):
    from concourse.bass_utils import run_bass_kernel_spmd
    nc = _get_nc()
    in_maps = make_in_maps(inputs)
    res = run_bass_kernel_spmd(nc, in_maps, list(range(N_CORES)))
    outs = [r["out"].reshape(BL, COUT, H, W) for r in res.results]
    g1s = [r["g1"].reshape(BL, MID) for r in res.results]
    g2s = [r["g2"].reshape(BL, MID) for r in res.results]
    out = np.concatenate(outs, axis=0)
    g1 = np.concatenate(g1s, axis=0)
    g2 = np.concatenate(g2s, axis=0)
    return out, g1, g2


if __name__ == "__main__":
    nc = build(BL)
    print("built + compiled ok")


# revision 20
# speedup vs baseline: 1.2793x; 1.0114x over previous
"""Trainium2 Bass kernel for nn_MoEBottleneckA (MoE bottleneck block).

Strategy: data-parallel over batch (64 -> 8 samples per core, 8 cores),
weights replicated. Training-mode BatchNorm stats are synchronized with
small AllGather collectives (sync-BN). All matmuls in bf16 (fp32 PSUM
accumulation); BN statistics and normalization in fp32.

Per core:
  conv1 (1x1, 1024->256) as 8-chunk K-accumulated matmuls, gated by g1,
  BN1 partial stats fused into the PSUM->SBUF epilogues; AllGather;
  BN1 apply (+ReLU) in place; conv2 (3x3 SAME) as 18 shifted matmuls on a
  zero-padded 30x30 activation; gate g2 + BN2 stats; AllGather; BN2 apply
  with fused S2 = sum(act2) accumulation; conv3 (1x1, 256->1024) pass 1
  computes sum(y^2) partials (mean comes free via W3 @ S2); AllGather;
  conv3 pass 2 recomputes y, adds the residual inside PSUM via a
  diag(1/s3) matmul of x, and Relu(scale,bias) epilogues (split between
  the scalar and vector engines) write the final fp32 output.

Cross-core partial sums travel as [128, n] contiguous blocks; the
per-core reduction of the gathered [8, 128*n] block is done on the
tensor engine (ones-vector matmul), which keeps the sync window short.
"""
import sys

for _p in ("/opt/trn_rl_repo", "/root/.axon_site/_ro/trn_rl_repo"):
    if _p not in sys.path:
        sys.path.append(_p)

import numpy as np
import ml_dtypes

import concourse.bass as bass
import concourse.mybir as mybir
import concourse.tile as tile
from concourse import bacc
from concourse.masks import make_identity

N_CORES = 8
B, CIN, H, W = 64, 1024, 28, 28
MID, COUT, EMB = 256, 1024, 64
EPS = 1e-5
HWP = H * W          # 784
NH = 392             # matmul free-dim chunk (2 per 784)
BL = B // N_CORES    # samples per core
NTOT = float(B * HWP)

f32 = mybir.dt.float32
bf16 = mybir.dt.bfloat16
AF = mybir.ActivationFunctionType
ALU = mybir.AluOpType
X = mybir.AxisListType.X

bfnp = ml_dtypes.bfloat16


def _emit(nc, tc, ctx, bl):
    ntot = float(N_CORES * bl * HWP)
    rg = [list(range(N_CORES))]

    # ---------------- DRAM I/O ----------------
    x_ext = nc.dram_tensor("x", [bl, CIN, HWP], f32, kind="ExternalInput")
    embt_ext = nc.dram_tensor("embt", [EMB, bl], bf16, kind="ExternalInput")
    w1_ext = nc.dram_tensor("w1t", [128, 8, MID], bf16, kind="ExternalInput")
    w2_ext = nc.dram_tensor("w2t", [128, 2, 9, MID], bf16, kind="ExternalInput")
    w3_ext = nc.dram_tensor("w3t", [128, 2, COUT], bf16, kind="ExternalInput")
    wg1_ext = nc.dram_tensor("wg1", [EMB, MID], bf16, kind="ExternalInput")
    wg2_ext = nc.dram_tensor("wg2", [EMB, MID], bf16, kind="ExternalInput")
    bg1_ext = nc.dram_tensor("bg1", [128, 2], f32, kind="ExternalInput")
    bg2_ext = nc.dram_tensor("bg2", [128, 2], f32, kind="ExternalInput")
    bn1g_ext = nc.dram_tensor("bn1g", [128, 2], f32, kind="ExternalInput")
    bn1b_ext = nc.dram_tensor("bn1b", [128, 2], f32, kind="ExternalInput")
    bn2g_ext = nc.dram_tensor("bn2g", [128, 2], f32, kind="ExternalInput")
    bn2b_ext = nc.dram_tensor("bn2b", [128, 2], f32, kind="ExternalInput")
    bn3g_ext = nc.dram_tensor("bn3g", [128, 8], f32, kind="ExternalInput")
    bn3b_ext = nc.dram_tensor("bn3b", [128, 8], f32, kind="ExternalInput")

    out_ext = nc.dram_tensor("out", [bl, COUT, HWP], f32, kind="ExternalOutput")
    g1_ext = nc.dram_tensor("g1", [bl, MID], f32, kind="ExternalOutput")
    g2_ext = nc.dram_tensor("g2", [bl, MID], f32, kind="ExternalOutput")

    # internal DRAM bounce buffers for the sync-BN collectives
    ccw_in = nc.dram_tensor("ccw_in", [8], f32)
    ccw_out = nc.dram_tensor("ccw_out", [N_CORES, 8], f32, addr_space="Shared")
    cc1_in = nc.dram_tensor("cc1_in", [128, 4], f32)
    cc1_out = nc.dram_tensor("cc1_out", [N_CORES, 128 * 4], f32,
                             addr_space="Shared")
    cc2_in = nc.dram_tensor("cc2_in", [128, 4], f32)
    cc2_out = nc.dram_tensor("cc2_out", [N_CORES, 128 * 4], f32,
                             addr_space="Shared")
    cc3_in = nc.dram_tensor("cc3_in", [128, 10], f32)
    cc3_out = nc.dram_tensor("cc3_out", [N_CORES, 128 * 10], f32,
                             addr_space="Shared")

    # ---------------- pools ----------------
    singles = ctx.enter_context(tc.tile_pool(name="singles", bufs=1))
    xload = ctx.enter_context(tc.tile_pool(name="xload", bufs=5))
    ostage_p = ctx.enter_context(tc.tile_pool(name="ostage", bufs=4))
    scratch_p = ctx.enter_context(tc.tile_pool(name="scratch", bufs=2))
    psA = ctx.enter_context(tc.tile_pool(name="psA", bufs=3, space="PSUM"))
    psB = ctx.enter_context(tc.tile_pool(name="psB", bufs=2, space="PSUM"))

    # ---------------- persistent SBUF ----------------
    xres = singles.tile([128, 8, bl, HWP], bf16)
    act1 = singles.tile([128, 2, bl, 900], bf16)   # zero-padded 30x30
    act2 = singles.tile([128, 2, bl, HWP], bf16)

    # warm up the collectives firmware with a dummy 8-float AllGather,
    # issued before everything else so its data plane doesn't contend
    # with the x loads
    ones8 = singles.tile([8, 1], f32)
    nc.vector.memset(ones8, 1.0)
    nc.sync.dma_start(out=ccw_in[:], in_=ones8[:, 0])
    nc.gpsimd.collective_compute(
        "AllGather", ALU.bypass, replica_groups=rg,
        ins=[ccw_in[:]], outs=[ccw_out[:, :]])

    # x for the first samples first: conv1 starts as soon as possible
    w1s = singles.tile([128, 8, MID], bf16)
    xsts = {}
    for b in range(min(2, bl)):
        for ko in range(8):
            xst = xload.tile([128, HWP], f32, tag="xst", name=f"xst_{b}_{ko}")
            eng = nc.sync if ko % 2 == 0 else nc.gpsimd
            eng.dma_start(out=xst, in_=x_ext[b, ko * 128:(ko + 1) * 128, :])
            xsts[(b, ko)] = xst
    nc.sync.dma_start(out=w1s, in_=w1_ext[:, :, :])

    w2s = singles.tile([128, 2, 9, MID], bf16)
    w3s = singles.tile([128, 2, COUT], bf16)
    wg1s = singles.tile([EMB, MID], bf16)
    wg2s = singles.tile([EMB, MID], bf16)
    embts = singles.tile([EMB, bl], bf16)
    nc.sync.dma_start(out=wg1s, in_=wg1_ext[:, :])
    nc.sync.dma_start(out=wg2s, in_=wg2_ext[:, :])
    nc.sync.dma_start(out=embts, in_=embt_ext[:, :])
    nc.sync.dma_start(out=w2s, in_=w2_ext[:, :, :, :])
    nc.sync.dma_start(out=w3s, in_=w3_ext[:, :, :])

    bg1 = singles.tile([128, 2], f32)
    bg2 = singles.tile([128, 2], f32)
    bn1g = singles.tile([128, 2], f32)
    bn1b = singles.tile([128, 2], f32)
    bn2g = singles.tile([128, 2], f32)
    bn2b = singles.tile([128, 2], f32)
    bn3g = singles.tile([128, 8], f32)
    bn3b = singles.tile([128, 8], f32)
    for t, e in ((bg1, bg1_ext), (bg2, bg2_ext), (bn1g, bn1g_ext),
                 (bn1b, bn1b_ext), (bn2g, bn2g_ext), (bn2b, bn2b_ext),
                 (bn3g, bn3g_ext), (bn3b, bn3b_ext)):
        nc.sync.dma_start(out=t, in_=e[:, :])

    nc.gpsimd.memset(act1, 0.0)

    ident = singles.tile([128, 128], bf16)
    make_identity(nc, ident)

    # stats accumulators
    sum1 = singles.tile([128, 2 * bl], f32)
    sq1 = singles.tile([128, 2 * bl], f32)
    sum2 = singles.tile([128, 2 * bl], f32)
    sq2 = singles.tile([128, 2 * bl], f32)
    sq3 = singles.tile([128, 8 * bl], f32)
    S2acc = singles.tile([128, 2 * bl], f32)

    # ---------------- gates ----------------
    g1s = singles.tile([128, 2, bl], f32)
    g2s = singles.tile([128, 2, bl], f32)
    for gs, wgs, bgs, gext in ((g1s, wg1s, bg1, g1_ext),
                               (g2s, wg2s, bg2, g2_ext)):
        for mo in range(2):
            gps = psB.tile([128, bl], f32, tag="small", name=f"gps_{mo}")
            nc.tensor.matmul(gps[:, :], lhsT=wgs[:, mo * 128:(mo + 1) * 128],
                             rhs=embts, start=True, stop=True)
            nc.scalar.activation(out=gs[:, mo, :], in_=gps[:, :], func=AF.Relu,
                                 bias=bgs[:, mo:mo + 1], scale=1.0)
        for mo in range(2):
            nc.sync.dma_start(
                out=gext[:, mo * 128:(mo + 1) * 128].rearrange("b ki -> ki b"),
                in_=gs[:, mo, :])

    def cross_core_sum(tag, part, nch, cc_in, cc_out):
        """AllGather a [128, nch] fp32 partial block, then sum over the 8
        cores with ones-vector matmuls on the tensor engine.
        Returns a [128, nch] fp32 tile of global sums."""
        nc.sync.dma_start(out=cc_in[:, :], in_=part)
        nc.gpsimd.collective_compute(
            "AllGather", ALU.bypass, replica_groups=rg,
            ins=[cc_in[:, :]], outs=[cc_out[:, :]])
        gath = scratch_p.tile([8, 128 * nch], f32, tag=f"gath{nch}",
                              name=f"gath_{tag}", bufs=1)
        nc.sync.dma_start(out=gath, in_=cc_out[:, :])
        gv = gath.rearrange("c (ki n) -> c ki n", n=nch)
        ps = psB.tile([128, nch], f32, tag="small", name=f"ccps_{tag}")
        for j in range(nch):
            nc.tensor.matmul(ps[:, j:j + 1], lhsT=gv[:, :, j], rhs=ones8,
                             start=True, stop=True)
        g = singles.tile([128, nch], f32, name=f"glob_{tag}")
        nc.vector.tensor_copy(out=g, in_=ps)
        return g

    def bn_scale_bias(glob, soff, qoff, gamma, beta, nch, tag):
        """glob: [128, *] with sums at soff and sum-sq at qoff.
        Returns (s, t): s = gamma*rsqrt(var+eps), t = beta - mean*s."""
        m = singles.tile([128, nch], f32, name=f"m_{tag}")
        ey2 = singles.tile([128, nch], f32, name=f"ey2_{tag}")
        nc.vector.tensor_scalar_mul(m, glob[:, soff:soff + nch], 1.0 / ntot)
        nc.vector.tensor_scalar_mul(ey2, glob[:, qoff:qoff + nch], 1.0 / ntot)
        var = singles.tile([128, nch], f32, name=f"var_{tag}")
        nc.vector.tensor_tensor(out=var, in0=m, in1=m, op=ALU.mult)
        nc.vector.tensor_tensor(out=var, in0=ey2, in1=var, op=ALU.subtract)
        std = singles.tile([128, nch], f32, name=f"std_{tag}")
        nc.vector.tensor_scalar_add(var, var, EPS)
        nc.scalar.activation(out=std, in_=var, func=AF.Sqrt)
        rstd = singles.tile([128, nch], f32, name=f"rstd_{tag}")
        nc.vector.reciprocal(rstd, std)
        s = singles.tile([128, nch], f32, name=f"s_{tag}")
        t = singles.tile([128, nch], f32, name=f"t_{tag}")
        nc.vector.tensor_tensor(out=s, in0=gamma, in1=rstd, op=ALU.mult)
        nc.vector.tensor_tensor(out=t, in0=m, in1=s, op=ALU.mult)
        nc.vector.tensor_tensor(out=t, in0=beta, in1=t, op=ALU.subtract)
        return s, t

    # ---------------- phase 1: load x, conv1, bn1 stats ----------------
    for b in range(bl):
        for ko in range(8):
            if (b, ko) in xsts:
                xst = xsts[(b, ko)]
            else:
                xst = xload.tile([128, HWP], f32, tag="xst",
                                 name=f"xst_{b}_{ko}")
                eng = nc.sync if ko % 2 == 0 else nc.gpsimd
                eng.dma_start(out=xst,
                              in_=x_ext[b, ko * 128:(ko + 1) * 128, :])
            nc.vector.tensor_copy(out=xres[:, ko, b, :], in_=xst)
        pts = [psA.tile([128, 2, 512], f32, tag="mm", name=f"c1_{b}_{mo}")
               for mo in range(2)]
        for ko in range(8):
            for mo in range(2):
                for n in range(2):
                    nc.tensor.matmul(
                        pts[mo][:, n, :NH],
                        lhsT=w1s[:, ko, mo * 128:(mo + 1) * 128],
                        rhs=xres[:, ko, b, n * NH:(n + 1) * NH],
                        start=(ko == 0), stop=(ko == 7))
        for mo in range(2):
            # gated PSUM->SBUF into padded act1 interior, fused sum
            dst = act1[:, mo, b, :].rearrange("p (r c) -> p r c", c=30)[
                :, 1:29, 1:29].rearrange("p (u r) c -> p u r c", u=2)
            src = pts[mo][:, :, :NH].rearrange("p u (r c) -> p u r c", c=28)
            idx = mo * bl + b
            nc.vector.tensor_scalar(
                out=dst, in0=src, scalar1=g1s[:, mo, b:b + 1], scalar2=None,
                op0=ALU.mult, op1=ALU.add, accum_out=sum1[:, idx:idx + 1])
            sc = ostage_p.tile([128, HWP], bf16, tag="ost", name=f"sq1_{b}_{mo}")
            dst3 = act1[:, mo, b, :].rearrange("p (r c) -> p r c", c=30)[
                :, 1:29, 1:29]
            nc.scalar.activation(
                out=sc.rearrange("p (r c) -> p r c", c=28),
                in_=dst3, func=AF.Square,
                accum_out=sq1[:, idx:idx + 1])

    # bn1 sync
    p1 = singles.tile([128, 4], f32)
    nc.vector.tensor_reduce(out=p1[:, 0:2], in_=sum1.rearrange(
        "p (mo b) -> p mo b", b=bl), axis=X, op=ALU.add)
    nc.vector.tensor_reduce(out=p1[:, 2:4], in_=sq1.rearrange(
        "p (mo b) -> p mo b", b=bl), axis=X, op=ALU.add)
    glob1 = cross_core_sum("bn1", p1, 4, cc1_in, cc1_out)
    s1, t1 = bn_scale_bias(glob1, 0, 2, bn1g, bn1b, 2, "bn1")

    # bn1 apply (+ReLU), in place, chunked per sample
    for b in range(bl):
        for mo in range(2):
            v = act1[:, mo, b, :].rearrange("p (r c) -> p r c", c=30)[
                :, 1:29, 1:29]
            nc.scalar.activation(out=v, in_=v, func=AF.Relu,
                                 bias=t1[:, mo:mo + 1], scale=s1[:, mo:mo + 1])

    # ---------------- phase 2: conv2 3x3, bn2 stats ----------------
    for mo in range(2):
        for bp in range((bl + 1) // 2):
            pair = tuple(range(2 * bp, min(2 * bp + 2, bl)))
            pts = {b: psA.tile([128, 2, 512], f32, tag="mm",
                               name=f"c2_{b}_{mo}") for b in pair}
            a1v = {b: act1[:, :, b, :].rearrange("p k (r c) -> p k r c", c=30)
                   for b in pair}
            k = 0
            for ko in range(2):
                for tap in range(9):
                    dy, dx = tap // 3, tap % 3
                    for b in pair:
                        for rc in range(2):
                            rhs = a1v[b][:, ko,
                                         14 * rc + dy:14 * rc + dy + 14,
                                         dx:dx + 28]
                            nc.tensor.matmul(
                                pts[b][:, rc, :NH],
                                lhsT=w2s[:, ko, tap, mo * 128:(mo + 1) * 128],
                                rhs=rhs, start=(k == 0), stop=(k == 17))
                    k += 1
            for b in pair:
                dst = act2[:, mo, b, :].rearrange("p (u r c) -> p u r c",
                                                  u=2, c=28)
                src = pts[b][:, :, :NH].rearrange("p u (r c) -> p u r c", c=28)
                idx = mo * bl + b
                nc.vector.tensor_scalar(
                    out=dst, in0=src, scalar1=g2s[:, mo, b:b + 1],
                    scalar2=None, op0=ALU.mult, op1=ALU.add,
                    accum_out=sum2[:, idx:idx + 1])
                sc = ostage_p.tile([128, HWP], bf16, tag="ost",
                                    name=f"sq2_{b}_{mo}")
                nc.scalar.activation(
                    out=sc.rearrange("p (u c) -> p u c", u=2),
                    in_=act2[:, mo, b, :].rearrange("p (u c) -> p u c", u=2),
                    func=AF.Square, accum_out=sq2[:, idx:idx + 1])

    # bn2 sync
    p2 = singles.tile([128, 4], f32)
    nc.vector.tensor_reduce(out=p2[:, 0:2], in_=sum2.rearrange(
        "p (mo b) -> p mo b", b=bl), axis=X, op=ALU.add)
    nc.vector.tensor_reduce(out=p2[:, 2:4], in_=sq2.rearrange(
        "p (mo b) -> p mo b", b=bl), axis=X, op=ALU.add)
    glob2 = cross_core_sum("bn2", p2, 4, cc2_in, cc2_out)
    s2, t2 = bn_scale_bias(glob2, 0, 2, bn2g, bn2b, 2, "bn2")

    # bn2 apply (+ReLU) with fused S2 = sum(act2) accumulation
    for b in range(bl):
        for mo in range(2):
            v = act2[:, mo, b, :]
            idx = mo * bl + b
            nc.scalar.activation(out=v, in_=v, func=AF.Relu,
                                 bias=t2[:, mo:mo + 1], scale=s2[:, mo:mo + 1],
                                 accum_out=S2acc[:, idx:idx + 1])

    # ---------------- phase 3: conv3 pass 1 (stats only) ----------------
    for mo in range(8):
        for bp in range((bl + 1) // 2):
            pair = tuple(range(2 * bp, min(2 * bp + 2, bl)))
            pts = {b: psA.tile([128, 2, 512], f32, tag="mm",
                               name=f"c3a_{b}_{mo}") for b in pair}
            for ko in range(2):
                for b in pair:
                    for n in range(2):
                        nc.tensor.matmul(
                            pts[b][:, n, :NH],
                            lhsT=w3s[:, ko, mo * 128:(mo + 1) * 128],
                            rhs=act2[:, ko, b, n * NH:(n + 1) * NH],
                            start=(ko == 0), stop=(ko == 1))
            for b in pair:
                sc = ostage_p.tile([128, HWP], bf16, tag="ost",
                                    name=f"sq3_{b}_{mo}")
                idx = mo * bl + b
                nc.scalar.activation(
                    out=sc.rearrange("p (u c) -> p u c", u=2),
                    in_=pts[b][:, :, :NH], func=AF.Square,
                    accum_out=sq3[:, idx:idx + 1])

    # bn3 sync: sum(y^2) partials + S2 partials in one AllGather
    p3 = singles.tile([128, 10], f32)
    nc.vector.tensor_reduce(out=p3[:, 0:8], in_=sq3.rearrange(
        "p (mo b) -> p mo b", b=bl), axis=X, op=ALU.add)
    nc.vector.tensor_reduce(out=p3[:, 8:10], in_=S2acc.rearrange(
        "p (mo b) -> p mo b", b=bl), axis=X, op=ALU.add)
    glob3 = cross_core_sum("bn3", p3, 10, cc3_in, cc3_out)

    # mean3 via W3 @ S2  (exploits linearity of the 1x1 conv)
    S2b = singles.tile([128, 2], bf16)
    nc.vector.tensor_copy(out=S2b, in_=glob3[:, 8:10])
    pm = psB.tile([128, 8], f32, tag="small")
    for mo in range(8):
        for ko in range(2):
            nc.tensor.matmul(pm[:, mo:mo + 1],
                             lhsT=w3s[:, ko, mo * 128:(mo + 1) * 128],
                             rhs=S2b[:, ko:ko + 1],
                             start=(ko == 0), stop=(ko == 1))
    g3full = singles.tile([128, 16], f32)
    nc.vector.tensor_copy(out=g3full[:, 0:8], in_=pm)
    nc.vector.tensor_copy(out=g3full[:, 8:16], in_=glob3[:, 0:8])
    s3, t3 = bn_scale_bias(g3full, 0, 8, bn3g, bn3b, 8, "bn3")

    # diag(1/s3) in bf16 for the in-PSUM residual add
    invs3 = singles.tile([128, 8], f32)
    nc.vector.reciprocal(invs3, s3)
    diag3 = singles.tile([128, 8, 128], bf16)
    for mo in range(8):
        nc.vector.tensor_scalar_mul(diag3[:, mo, :], ident,
                                    invs3[:, mo:mo + 1])

    # ---------------- phase 4: conv3 pass 2 + residual + out ----------------
    for mo in range(8):
        for bp in range((bl + 1) // 2):
            pair = tuple(range(2 * bp, min(2 * bp + 2, bl)))
            pts = {b: psA.tile([128, 2, 512], f32, tag="mm",
                               name=f"c3b_{b}_{mo}") for b in pair}
            for ko in range(2):
                for b in pair:
                    for n in range(2):
                        nc.tensor.matmul(
                            pts[b][:, n, :NH],
                            lhsT=w3s[:, ko, mo * 128:(mo + 1) * 128],
                            rhs=act2[:, ko, b, n * NH:(n + 1) * NH],
                            start=(ko == 0), stop=False)
            for b in pair:
                for n in range(2):
                    nc.tensor.matmul(
                        pts[b][:, n, :NH],
                        lhsT=diag3[:, mo, :],
                        rhs=xres[:, mo, b, n * NH:(n + 1) * NH],
                        start=False, stop=True)
            for b in pair:
                ost = ostage_p.tile([128, HWP], f32, tag="ost",
                                    name=f"ost_{b}_{mo}")
                if (b * 8 + mo) % 2 == 0:
                    nc.scalar.activation(
                        out=ost.rearrange("p (u c) -> p u c", u=2),
                        in_=pts[b][:, :, :NH], func=AF.Relu,
                        scale=s3[:, mo:mo + 1], bias=t3[:, mo:mo + 1])
                else:
                    nc.vector.tensor_scalar(
                        out=ost.rearrange("p (u c) -> p u c", u=2),
                        in0=pts[b][:, :, :NH], scalar1=s3[:, mo:mo + 1],
                        scalar2=t3[:, mo:mo + 1], op0=ALU.mult, op1=ALU.add)
                    nc.vector.tensor_scalar_max(ost, ost, 0.0)
                nc.sync.dma_start(out=out_ext[b, mo * 128:(mo + 1) * 128, :],
                                  in_=ost)


def build(bl=BL):
    nc = bacc.Bacc("TRN2", target_bir_lowering=False, debug=False,
                   num_devices=N_CORES)
    from contextlib import ExitStack
    with tile.TileContext(nc) as tc, ExitStack() as ctx:
        _emit(nc, tc, ctx, bl)
    nc.compile()
    return nc


def prep_weights(inputs):
    """Host-side reshape/cast of the (small) replicated weights."""
    w1 = np.asarray(inputs["conv1_w"], np.float32).reshape(MID, CIN)
    w2 = np.asarray(inputs["conv2_w"], np.float32)
    w3 = np.asarray(inputs["conv3_w"], np.float32).reshape(COUT, MID)

    w1t = np.ascontiguousarray(
        w1.reshape(MID, 8, 128).transpose(2, 1, 0)).astype(bfnp)
    w2t = np.ascontiguousarray(
        w2.reshape(MID, 2, 128, 9).transpose(2, 1, 3, 0)).astype(bfnp)
    w3t = np.ascontiguousarray(
        w3.reshape(COUT, 2, 128).transpose(2, 1, 0)).astype(bfnp)

    def chan_tile(v, nch):
        return np.ascontiguousarray(
            np.asarray(v, np.float32).reshape(nch, 128).T)

    return {
        "w1t": w1t, "w2t": w2t, "w3t": w3t,
        "wg1": np.asarray(inputs["w_gate1"], np.float32).astype(bfnp),
        "wg2": np.asarray(inputs["w_gate2"], np.float32).astype(bfnp),
        "bg1": chan_tile(inputs["b_gate1"], 2),
        "bg2": chan_tile(inputs["b_gate2"], 2),
        "bn1g": chan_tile(inputs["bn1_g"], 2),
        "bn1b": chan_tile(inputs["bn1_b"], 2),
        "bn2g": chan_tile(inputs["bn2_g"], 2),
        "bn2b": chan_tile(inputs["bn2_b"], 2),
        "bn3g": chan_tile(inputs["bn3_g"], 8),
        "bn3b": chan_tile(inputs["bn3_b"], 8),
    }


def make_in_maps(inputs, bl=BL):
    x = np.asarray(inputs["x"], np.float32).reshape(B, CIN, HWP)
    emb = np.asarray(inputs["embeddings"], np.float32)
    w = prep_weights(inputs)
    in_maps = []
    for c in range(N_CORES):
        sl = slice(c * bl, (c + 1) * bl)
        m = dict(w)
        m["x"] = x[sl]
        m["embt"] = np.ascontiguousarray(emb[sl].T).astype(bfnp)
        in_maps.append(m)
    return in_maps


_built = {}


def _get_nc():
    if "nc" not in _built:
        _built["nc"] = build(BL)
    return _built["nc"]


def kernel(**inputs):
    from concourse.bass_utils import run_bass_kernel_spmd
    nc = _get_nc()
    in_maps = make_in_maps(inputs)
    res = run_bass_kernel_spmd(nc, in_maps, list(range(N_CORES)))
    outs = [r["out"].reshape(BL, COUT, H, W) for r in res.results]
    g1s = [r["g1"].reshape(BL, MID) for r in res.results]
    g2s = [r["g2"].reshape(BL, MID) for r in res.results]
    out = np.concatenate(outs, axis=0)
    g1 = np.concatenate(g1s, axis=0)
    g2 = np.concatenate(g2s, axis=0)
    return out, g1, g2


if __name__ == "__main__":
    nc = build(BL)
    print("built + compiled ok")


# revision 21
# speedup vs baseline: 1.2876x; 1.0065x over previous
"""Trainium2 Bass kernel for nn_MoEBottleneckA (MoE bottleneck block).

Strategy: data-parallel over batch (64 -> 8 samples per core, 8 cores),
weights replicated. Training-mode BatchNorm stats are synchronized with
small AllGather collectives (sync-BN). All matmuls in bf16 (fp32 PSUM
accumulation); BN statistics and normalization in fp32.

Per core:
  conv1 (1x1, 1024->256) as 8-chunk K-accumulated matmuls, gated by g1,
  BN1 partial stats fused into the PSUM->SBUF epilogues; AllGather;
  BN1 apply (+ReLU) in place; conv2 (3x3 SAME) as 18 shifted matmuls on a
  zero-padded 30x30 activation; gate g2 + BN2 stats; AllGather; BN2 apply
  with fused S2 = sum(act2) accumulation; conv3 (1x1, 256->1024) pass 1
  computes sum(y^2) partials (mean comes free via W3 @ S2); AllGather;
  conv3 pass 2 recomputes y, adds the residual inside PSUM via a
  diag(1/s3) matmul of x, and Relu(scale,bias) epilogues (split between
  the scalar and vector engines) write the final fp32 output.

Cross-core partial sums travel as [128, n] contiguous blocks; the
per-core reduction of the gathered [8, 128*n] block is done on the
tensor engine (ones-vector matmul), which keeps the sync window short.
"""
import sys

for _p in ("/opt/trn_rl_repo", "/root/.axon_site/_ro/trn_rl_repo"):
    if _p not in sys.path:
        sys.path.append(_p)

import numpy as np
import ml_dtypes

import concourse.bass as bass
import concourse.mybir as mybir
import concourse.tile as tile
from concourse import bacc
from concourse.masks import make_identity

N_CORES = 8
B, CIN, H, W = 64, 1024, 28, 28
MID, COUT, EMB = 256, 1024, 64
EPS = 1e-5
HWP = H * W          # 784
NH = 392             # matmul free-dim chunk (2 per 784)
BL = B // N_CORES    # samples per core
NTOT = float(B * HWP)

f32 = mybir.dt.float32
bf16 = mybir.dt.bfloat16
AF = mybir.ActivationFunctionType
ALU = mybir.AluOpType
X = mybir.AxisListType.X

bfnp = ml_dtypes.bfloat16


def _emit(nc, tc, ctx, bl):
    ntot = float(N_CORES * bl * HWP)
    rg = [list(range(N_CORES))]

    # ---------------- DRAM I/O ----------------
    x_ext = nc.dram_tensor("x", [bl, CIN, HWP], f32, kind="ExternalInput")
    embt_ext = nc.dram_tensor("embt", [EMB, bl], bf16, kind="ExternalInput")
    w1_ext = nc.dram_tensor("w1t", [128, 8, MID], bf16, kind="ExternalInput")
    w2_ext = nc.dram_tensor("w2t", [128, 2, 9, MID], bf16, kind="ExternalInput")
    w3_ext = nc.dram_tensor("w3t", [128, 2, COUT], bf16, kind="ExternalInput")
    wg1_ext = nc.dram_tensor("wg1", [EMB, MID], bf16, kind="ExternalInput")
    wg2_ext = nc.dram_tensor("wg2", [EMB, MID], bf16, kind="ExternalInput")
    bg1_ext = nc.dram_tensor("bg1", [128, 2], f32, kind="ExternalInput")
    bg2_ext = nc.dram_tensor("bg2", [128, 2], f32, kind="ExternalInput")
    bn1g_ext = nc.dram_tensor("bn1g", [128, 2], f32, kind="ExternalInput")
    bn1b_ext = nc.dram_tensor("bn1b", [128, 2], f32, kind="ExternalInput")
    bn2g_ext = nc.dram_tensor("bn2g", [128, 2], f32, kind="ExternalInput")
    bn2b_ext = nc.dram_tensor("bn2b", [128, 2], f32, kind="ExternalInput")
    bn3g_ext = nc.dram_tensor("bn3g", [128, 8], f32, kind="ExternalInput")
    bn3b_ext = nc.dram_tensor("bn3b", [128, 8], f32, kind="ExternalInput")

    out_ext = nc.dram_tensor("out", [bl, COUT, HWP], f32, kind="ExternalOutput")
    g1_ext = nc.dram_tensor("g1", [bl, MID], f32, kind="ExternalOutput")
    g2_ext = nc.dram_tensor("g2", [bl, MID], f32, kind="ExternalOutput")

    # internal DRAM bounce buffers for the sync-BN collectives
    ccw_in = nc.dram_tensor("ccw_in", [8], f32)
    ccw_out = nc.dram_tensor("ccw_out", [N_CORES, 8], f32, addr_space="Shared")
    cc1_in = nc.dram_tensor("cc1_in", [128, 4], f32)
    cc1_out = nc.dram_tensor("cc1_out", [N_CORES, 128 * 4], f32,
                             addr_space="Shared")
    cc2_in = nc.dram_tensor("cc2_in", [128, 4], f32)
    cc2_out = nc.dram_tensor("cc2_out", [N_CORES, 128 * 4], f32,
                             addr_space="Shared")
    cc3_in = nc.dram_tensor("cc3_in", [128, 10], f32)
    cc3_out = nc.dram_tensor("cc3_out", [N_CORES, 128 * 10], f32,
                             addr_space="Shared")

    # ---------------- pools ----------------
    singles = ctx.enter_context(tc.tile_pool(name="singles", bufs=1))
    xload = ctx.enter_context(tc.tile_pool(name="xload", bufs=5))
    ostage_p = ctx.enter_context(tc.tile_pool(name="ostage", bufs=4))
    scratch_p = ctx.enter_context(tc.tile_pool(name="scratch", bufs=2))
    psA = ctx.enter_context(tc.tile_pool(name="psA", bufs=3, space="PSUM"))
    psB = ctx.enter_context(tc.tile_pool(name="psB", bufs=2, space="PSUM"))

    # ---------------- persistent SBUF ----------------
    xres = singles.tile([128, 8, bl, HWP], bf16)
    act1 = singles.tile([128, 2, bl, 900], bf16)   # zero-padded 30x30
    act2 = singles.tile([128, 2, bl, HWP], bf16)

    # warm up the collectives firmware with a dummy 8-float AllGather,
    # issued before everything else so its data plane doesn't contend
    # with the x loads
    ones8 = singles.tile([8, 1], f32)
    nc.vector.memset(ones8, 1.0)
    nc.sync.dma_start(out=ccw_in[:], in_=ones8[:, 0])
    nc.gpsimd.collective_compute(
        "AllGather", ALU.bypass, replica_groups=rg,
        ins=[ccw_in[:]], outs=[ccw_out[:, :]])

    # x for the first samples first: conv1 starts as soon as possible
    w1s = singles.tile([128, 8, MID], bf16)
    xsts = {}
    for b in range(min(2, bl)):
        for ko in range(8):
            xst = xload.tile([128, HWP], f32, tag="xst", name=f"xst_{b}_{ko}")
            nc.sync.dma_start(out=xst, in_=x_ext[b, ko * 128:(ko + 1) * 128, :])
            xsts[(b, ko)] = xst
    nc.sync.dma_start(out=w1s, in_=w1_ext[:, :, :])

    w2s = singles.tile([128, 2, 9, MID], bf16)
    w3s = singles.tile([128, 2, COUT], bf16)
    wg1s = singles.tile([EMB, MID], bf16)
    wg2s = singles.tile([EMB, MID], bf16)
    embts = singles.tile([EMB, bl], bf16)
    nc.sync.dma_start(out=wg1s, in_=wg1_ext[:, :])
    nc.sync.dma_start(out=wg2s, in_=wg2_ext[:, :])
    nc.sync.dma_start(out=embts, in_=embt_ext[:, :])
    nc.sync.dma_start(out=w2s, in_=w2_ext[:, :, :, :])
    nc.sync.dma_start(out=w3s, in_=w3_ext[:, :, :])

    bg1 = singles.tile([128, 2], f32)
    bg2 = singles.tile([128, 2], f32)
    bn1g = singles.tile([128, 2], f32)
    bn1b = singles.tile([128, 2], f32)
    bn2g = singles.tile([128, 2], f32)
    bn2b = singles.tile([128, 2], f32)
    bn3g = singles.tile([128, 8], f32)
    bn3b = singles.tile([128, 8], f32)
    for t, e in ((bg1, bg1_ext), (bg2, bg2_ext), (bn1g, bn1g_ext),
                 (bn1b, bn1b_ext), (bn2g, bn2g_ext), (bn2b, bn2b_ext),
                 (bn3g, bn3g_ext), (bn3b, bn3b_ext)):
        nc.sync.dma_start(out=t, in_=e[:, :])

    nc.gpsimd.memset(act1, 0.0)

    ident = singles.tile([128, 128], bf16)
    make_identity(nc, ident)

    # stats accumulators
    sum1 = singles.tile([128, 2 * bl], f32)
    sq1 = singles.tile([128, 2 * bl], f32)
    sum2 = singles.tile([128, 2 * bl], f32)
    sq2 = singles.tile([128, 2 * bl], f32)
    sq3 = singles.tile([128, 8 * bl], f32)
    S2acc = singles.tile([128, 2 * bl], f32)

    # ---------------- gates ----------------
    g1s = singles.tile([128, 2, bl], f32)
    g2s = singles.tile([128, 2, bl], f32)
    for gs, wgs, bgs, gext in ((g1s, wg1s, bg1, g1_ext),
                               (g2s, wg2s, bg2, g2_ext)):
        for mo in range(2):
            gps = psB.tile([128, bl], f32, tag="small", name=f"gps_{mo}")
            nc.tensor.matmul(gps[:, :], lhsT=wgs[:, mo * 128:(mo + 1) * 128],
                             rhs=embts, start=True, stop=True)
            nc.scalar.activation(out=gs[:, mo, :], in_=gps[:, :], func=AF.Relu,
                                 bias=bgs[:, mo:mo + 1], scale=1.0)
        for mo in range(2):
            nc.sync.dma_start(
                out=gext[:, mo * 128:(mo + 1) * 128].rearrange("b ki -> ki b"),
                in_=gs[:, mo, :])

    def cross_core_sum(tag, part, nch, cc_in, cc_out):
        """AllGather a [128, nch] fp32 partial block, then sum over the 8
        cores with ones-vector matmuls on the tensor engine.
        Returns a [128, nch] fp32 tile of global sums."""
        nc.sync.dma_start(out=cc_in[:, :], in_=part)
        nc.gpsimd.collective_compute(
            "AllGather", ALU.bypass, replica_groups=rg,
            ins=[cc_in[:, :]], outs=[cc_out[:, :]])
        gath = scratch_p.tile([8, 128 * nch], f32, tag=f"gath{nch}",
                              name=f"gath_{tag}", bufs=1)
        nc.sync.dma_start(out=gath, in_=cc_out[:, :])
        gv = gath.rearrange("c (ki n) -> c ki n", n=nch)
        ps = psB.tile([128, nch], f32, tag="small", name=f"ccps_{tag}")
        for j in range(nch):
            nc.tensor.matmul(ps[:, j:j + 1], lhsT=gv[:, :, j], rhs=ones8,
                             start=True, stop=True)
        g = singles.tile([128, nch], f32, name=f"glob_{tag}")
        nc.vector.tensor_copy(out=g, in_=ps)
        return g

    def bn_scale_bias(glob, soff, qoff, gamma, beta, nch, tag):
        """glob: [128, *] with sums at soff and sum-sq at qoff.
        Returns (s, t): s = gamma*rsqrt(var+eps), t = beta - mean*s."""
        m = singles.tile([128, nch], f32, name=f"m_{tag}")
        ey2 = singles.tile([128, nch], f32, name=f"ey2_{tag}")
        nc.vector.tensor_scalar_mul(m, glob[:, soff:soff + nch], 1.0 / ntot)
        nc.vector.tensor_scalar_mul(ey2, glob[:, qoff:qoff + nch], 1.0 / ntot)
        var = singles.tile([128, nch], f32, name=f"var_{tag}")
        nc.vector.tensor_tensor(out=var, in0=m, in1=m, op=ALU.mult)
        nc.vector.tensor_tensor(out=var, in0=ey2, in1=var, op=ALU.subtract)
        std = singles.tile([128, nch], f32, name=f"std_{tag}")
        nc.vector.tensor_scalar_add(var, var, EPS)
        nc.scalar.activation(out=std, in_=var, func=AF.Sqrt)
        rstd = singles.tile([128, nch], f32, name=f"rstd_{tag}")
        nc.vector.reciprocal(rstd, std)
        s = singles.tile([128, nch], f32, name=f"s_{tag}")
        t = singles.tile([128, nch], f32, name=f"t_{tag}")
        nc.vector.tensor_tensor(out=s, in0=gamma, in1=rstd, op=ALU.mult)
        nc.vector.tensor_tensor(out=t, in0=m, in1=s, op=ALU.mult)
        nc.vector.tensor_tensor(out=t, in0=beta, in1=t, op=ALU.subtract)
        return s, t

    # ---------------- phase 1: load x, conv1, bn1 stats ----------------
    for b in range(bl):
        for ko in range(8):
            if (b, ko) in xsts:
                xst = xsts[(b, ko)]
            else:
                xst = xload.tile([128, HWP], f32, tag="xst",
                                 name=f"xst_{b}_{ko}")
                nc.sync.dma_start(out=xst,
                                  in_=x_ext[b, ko * 128:(ko + 1) * 128, :])
            nc.vector.tensor_copy(out=xres[:, ko, b, :], in_=xst)
        pts = [psA.tile([128, 2, 512], f32, tag="mm", name=f"c1_{b}_{mo}")
               for mo in range(2)]
        for ko in range(8):
            for mo in range(2):
                for n in range(2):
                    nc.tensor.matmul(
                        pts[mo][:, n, :NH],
                        lhsT=w1s[:, ko, mo * 128:(mo + 1) * 128],
                        rhs=xres[:, ko, b, n * NH:(n + 1) * NH],
                        start=(ko == 0), stop=(ko == 7))
        for mo in range(2):
            # gated PSUM->SBUF into padded act1 interior, fused sum
            dst = act1[:, mo, b, :].rearrange("p (r c) -> p r c", c=30)[
                :, 1:29, 1:29].rearrange("p (u r) c -> p u r c", u=2)
            src = pts[mo][:, :, :NH].rearrange("p u (r c) -> p u r c", c=28)
            idx = mo * bl + b
            nc.vector.tensor_scalar(
                out=dst, in0=src, scalar1=g1s[:, mo, b:b + 1], scalar2=None,
                op0=ALU.mult, op1=ALU.add, accum_out=sum1[:, idx:idx + 1])
            sc = ostage_p.tile([128, HWP], bf16, tag="ost", name=f"sq1_{b}_{mo}")
            dst3 = act1[:, mo, b, :].rearrange("p (r c) -> p r c", c=30)[
                :, 1:29, 1:29]
            nc.scalar.activation(
                out=sc.rearrange("p (r c) -> p r c", c=28),
                in_=dst3, func=AF.Square,
                accum_out=sq1[:, idx:idx + 1])

    # bn1 sync
    p1 = singles.tile([128, 4], f32)
    nc.vector.tensor_reduce(out=p1[:, 0:2], in_=sum1.rearrange(
        "p (mo b) -> p mo b", b=bl), axis=X, op=ALU.add)
    nc.vector.tensor_reduce(out=p1[:, 2:4], in_=sq1.rearrange(
        "p (mo b) -> p mo b", b=bl), axis=X, op=ALU.add)
    glob1 = cross_core_sum("bn1", p1, 4, cc1_in, cc1_out)
    s1, t1 = bn_scale_bias(glob1, 0, 2, bn1g, bn1b, 2, "bn1")

    # bn1 apply (+ReLU), in place, chunked per sample
    for b in range(bl):
        for mo in range(2):
            v = act1[:, mo, b, :].rearrange("p (r c) -> p r c", c=30)[
                :, 1:29, 1:29]
            nc.scalar.activation(out=v, in_=v, func=AF.Relu,
                                 bias=t1[:, mo:mo + 1], scale=s1[:, mo:mo + 1])

    # ---------------- phase 2: conv2 3x3, bn2 stats ----------------
    for mo in range(2):
        for bp in range((bl + 1) // 2):
            pair = tuple(range(2 * bp, min(2 * bp + 2, bl)))
            pts = {b: psA.tile([128, 2, 512], f32, tag="mm",
                               name=f"c2_{b}_{mo}") for b in pair}
            a1v = {b: act1[:, :, b, :].rearrange("p k (r c) -> p k r c", c=30)
                   for b in pair}
            k = 0
            for ko in range(2):
                for tap in range(9):
                    dy, dx = tap // 3, tap % 3
                    for b in pair:
                        for rc in range(2):
                            rhs = a1v[b][:, ko,
                                         14 * rc + dy:14 * rc + dy + 14,
                                         dx:dx + 28]
                            nc.tensor.matmul(
                                pts[b][:, rc, :NH],
                                lhsT=w2s[:, ko, tap, mo * 128:(mo + 1) * 128],
                                rhs=rhs, start=(k == 0), stop=(k == 17))
                    k += 1
            for b in pair:
                dst = act2[:, mo, b, :].rearrange("p (u r c) -> p u r c",
                                                  u=2, c=28)
                src = pts[b][:, :, :NH].rearrange("p u (r c) -> p u r c", c=28)
                idx = mo * bl + b
                nc.vector.tensor_scalar(
                    out=dst, in0=src, scalar1=g2s[:, mo, b:b + 1],
                    scalar2=None, op0=ALU.mult, op1=ALU.add,
                    accum_out=sum2[:, idx:idx + 1])
                sc = ostage_p.tile([128, HWP], bf16, tag="ost",
                                    name=f"sq2_{b}_{mo}")
                nc.scalar.activation(
                    out=sc.rearrange("p (u c) -> p u c", u=2),
                    in_=act2[:, mo, b, :].rearrange("p (u c) -> p u c", u=2),
                    func=AF.Square, accum_out=sq2[:, idx:idx + 1])

    # bn2 sync
    p2 = singles.tile([128, 4], f32)
    nc.vector.tensor_reduce(out=p2[:, 0:2], in_=sum2.rearrange(
        "p (mo b) -> p mo b", b=bl), axis=X, op=ALU.add)
    nc.vector.tensor_reduce(out=p2[:, 2:4], in_=sq2.rearrange(
        "p (mo b) -> p mo b", b=bl), axis=X, op=ALU.add)
    glob2 = cross_core_sum("bn2", p2, 4, cc2_in, cc2_out)
    s2, t2 = bn_scale_bias(glob2, 0, 2, bn2g, bn2b, 2, "bn2")

    # bn2 apply (+ReLU) with fused S2 = sum(act2) accumulation
    for b in range(bl):
        for mo in range(2):
            v = act2[:, mo, b, :]
            idx = mo * bl + b
            nc.scalar.activation(out=v, in_=v, func=AF.Relu,
                                 bias=t2[:, mo:mo + 1], scale=s2[:, mo:mo + 1],
                                 accum_out=S2acc[:, idx:idx + 1])

    # ---------------- phase 3: conv3 pass 1 (stats only) ----------------
    for mo in range(8):
        for bp in range((bl + 1) // 2):
            pair = tuple(range(2 * bp, min(2 * bp + 2, bl)))
            pts = {b: psA.tile([128, 2, 512], f32, tag="mm",
                               name=f"c3a_{b}_{mo}") for b in pair}
            for ko in range(2):
                for b in pair:
                    for n in range(2):
                        nc.tensor.matmul(
                            pts[b][:, n, :NH],
                            lhsT=w3s[:, ko, mo * 128:(mo + 1) * 128],
                            rhs=act2[:, ko, b, n * NH:(n + 1) * NH],
                            start=(ko == 0), stop=(ko == 1))
            for b in pair:
                sc = ostage_p.tile([128, HWP], bf16, tag="ost",
                                    name=f"sq3_{b}_{mo}")
                idx = mo * bl + b
                nc.scalar.activation(
                    out=sc.rearrange("p (u c) -> p u c", u=2),
                    in_=pts[b][:, :, :NH], func=AF.Square,
                    accum_out=sq3[:, idx:idx + 1])

    # bn3 sync: sum(y^2) partials + S2 partials in one AllGather
    p3 = singles.tile([128, 10], f32)
    nc.vector.tensor_reduce(out=p3[:, 0:8], in_=sq3.rearrange(
        "p (mo b) -> p mo b", b=bl), axis=X, op=ALU.add)
    nc.vector.tensor_reduce(out=p3[:, 8:10], in_=S2acc.rearrange(
        "p (mo b) -> p mo b", b=bl), axis=X, op=ALU.add)
    glob3 = cross_core_sum("bn3", p3, 10, cc3_in, cc3_out)

    # mean3 via W3 @ S2  (exploits linearity of the 1x1 conv)
    S2b = singles.tile([128, 2], bf16)
    nc.vector.tensor_copy(out=S2b, in_=glob3[:, 8:10])
    pm = psB.tile([128, 8], f32, tag="small")
    for mo in range(8):
        for ko in range(2):
            nc.tensor.matmul(pm[:, mo:mo + 1],
                             lhsT=w3s[:, ko, mo * 128:(mo + 1) * 128],
                             rhs=S2b[:, ko:ko + 1],
                             start=(ko == 0), stop=(ko == 1))
    g3full = singles.tile([128, 16], f32)
    nc.vector.tensor_copy(out=g3full[:, 0:8], in_=pm)
    nc.vector.tensor_copy(out=g3full[:, 8:16], in_=glob3[:, 0:8])
    s3, t3 = bn_scale_bias(g3full, 0, 8, bn3g, bn3b, 8, "bn3")

    # diag(1/s3) in bf16 for the in-PSUM residual add
    invs3 = singles.tile([128, 8], f32)
    nc.vector.reciprocal(invs3, s3)
    diag3 = singles.tile([128, 8, 128], bf16)
    for mo in range(8):
        nc.vector.tensor_scalar_mul(diag3[:, mo, :], ident,
                                    invs3[:, mo:mo + 1])

    # ---------------- phase 4: conv3 pass 2 + residual + out ----------------
    for mo in range(8):
        for bp in range((bl + 1) // 2):
            pair = tuple(range(2 * bp, min(2 * bp + 2, bl)))
            pts = {b: psA.tile([128, 2, 512], f32, tag="mm",
                               name=f"c3b_{b}_{mo}") for b in pair}
            for ko in range(2):
                for b in pair:
                    for n in range(2):
                        nc.tensor.matmul(
                            pts[b][:, n, :NH],
                            lhsT=w3s[:, ko, mo * 128:(mo + 1) * 128],
                            rhs=act2[:, ko, b, n * NH:(n + 1) * NH],
                            start=(ko == 0), stop=False)
            for b in pair:
                for n in range(2):
                    nc.tensor.matmul(
                        pts[b][:, n, :NH],
                        lhsT=diag3[:, mo, :],
                        rhs=xres[:, mo, b, n * NH:(n + 1) * NH],
                        start=False, stop=True)
            for b in pair:
                ost = ostage_p.tile([128, HWP], f32, tag="ost",
                                    name=f"ost_{b}_{mo}")
                if (b * 8 + mo) % 2 == 0:
                    nc.scalar.activation(
                        out=ost.rearrange("p (u c) -> p u c", u=2),
                        in_=pts[b][:, :, :NH], func=AF.Relu,
                        scale=s3[:, mo:mo + 1], bias=t3[:, mo:mo + 1])
                else:
                    nc.vector.tensor_scalar(
                        out=ost.rearrange("p (u c) -> p u c", u=2),
                        in0=pts[b][:, :, :NH], scalar1=s3[:, mo:mo + 1],
                        scalar2=t3[:, mo:mo + 1], op0=ALU.mult, op1=ALU.add)
                    nc.vector.tensor_scalar_max(ost, ost, 0.0)
                nc.sync.dma_start(out=out_ext[b, mo * 128:(mo + 1) * 128, :],
                                  in_=ost)


def build(bl=BL):
    nc = bacc.Bacc("TRN2", target_bir_lowering=False, debug=False,
                   num_devices=N_CORES)
    from contextlib import ExitStack
    with tile.TileContext(nc) as tc, ExitStack() as ctx:
        _emit(nc, tc, ctx, bl)
    nc.compile()
    return nc


def prep_weights(inputs):
    """Host-side reshape/cast of the (small) replicated weights."""
    w1 = np.asarray(inputs["conv1_w"], np.float32).reshape(MID, CIN)
    w2 = np.asarray(inputs["conv2_w"], np.float32)
    w3 = np.asarray(inputs["conv3_w"], np.float32).reshape(COUT, MID)

    w1t = np.ascontiguousarray(
        w1.reshape(MID, 8, 128).transpose(2, 1, 0)).astype(bfnp)
    w2t = np.ascontiguousarray(
        w2.reshape(MID, 2, 128, 9).transpose(2, 1, 3, 0)).astype(bfnp)
    w3t = np.ascontiguousarray(
        w3.reshape(COUT, 2, 128).transpose(2, 1, 0)).astype(bfnp)

    def chan_tile(v, nch):
        return np.ascontiguousarray(
            np.asarray(v, np.float32).reshape(nch, 128).T)

    return {
        "w1t": w1t, "w2t": w2t, "w3t": w3t,
        "wg1": np.asarray(inputs["w_gate1"], np.float32).astype(bfnp),
        "wg2": np.asarray(inputs["w_gate2"], np.float32).astype(bfnp),
        "bg1": chan_tile(inputs["b_gate1"], 2),
        "bg2": chan_tile(inputs["b_gate2"], 2),
        "bn1g": chan_tile(inputs["bn1_g"], 2),
        "bn1b": chan_tile(inputs["bn1_b"], 2),
        "bn2g": chan_tile(inputs["bn2_g"], 2),
        "bn2b": chan_tile(inputs["bn2_b"], 2),
        "bn3g": chan_tile(inputs["bn3_g"], 8),
        "bn3b": chan_tile(inputs["bn3_b"], 8),
    }


def make_in_maps(inputs, bl=BL):
    x = np.asarray(inputs["x"], np.float32).reshape(B, CIN, HWP)
    emb = np.asarray(inputs["embeddings"], np.float32)
    w = prep_weights(inputs)
    in_maps = []
    for c in range(N_CORES):
        sl = slice(c * bl, (c + 1) * bl)
        m = dict(w)
        m["x"] = x[sl]
        m["embt"] = np.ascontiguousarray(emb[sl].T).astype(bfnp)
        in_maps.append(m)
    return in_maps


_built = {}


def _get_nc():
    if "nc" not in _built:
        _built["nc"] = build(BL)
    return _built["nc"]


def kernel(**inputs):
    from concourse.bass_utils import run_bass_kernel_spmd
    nc = _get_nc()
    in_maps = make_in_maps(inputs)
    res = run_bass_kernel_spmd(nc, in_maps, list(range(N_CORES)))
    outs = [r["out"].reshape(BL, COUT, H, W) for r in res.results]
    g1s = [r["g1"].reshape(BL, MID) for r in res.results]
    g2s = [r["g2"].reshape(BL, MID) for r in res.results]
    out = np.concatenate(outs, axis=0)
    g1 = np.concatenate(g1s, axis=0)
    g2 = np.concatenate(g2s, axis=0)
    return out, g1, g2


if __name__ == "__main__":
    nc = build(BL)
    print("built + compiled ok")


# revision 24
# speedup vs baseline: 1.2899x; 1.0017x over previous
"""Trainium2 Bass kernel for nn_MoEBottleneckA (MoE bottleneck block).

Strategy: data-parallel over batch (64 -> 8 samples per core, 8 cores),
weights replicated. Training-mode BatchNorm stats are synchronized with
small AllGather collectives (sync-BN). All matmuls in bf16 (fp32 PSUM
accumulation); BN statistics and normalization in fp32.

Per core:
  conv1 (1x1, 1024->256) as 8-chunk K-accumulated matmuls, gated by g1,
  BN1 partial stats fused into the PSUM->SBUF epilogues; AllGather;
  BN1 apply (+ReLU) in place; conv2 (3x3 SAME) as 18 shifted matmuls on a
  zero-padded 30x30 activation; gate g2 + BN2 stats; AllGather; BN2 apply
  with fused S2 = sum(act2) accumulation; conv3 (1x1, 256->1024) pass 1
  computes sum(y^2) partials (mean comes free via W3 @ S2); AllGather;
  conv3 pass 2 recomputes y, adds the residual inside PSUM via a
  diag(1/s3) matmul of x, and Relu(scale,bias) epilogues (split between
  the scalar and vector engines) write the final fp32 output.

Cross-core partial sums travel as [128, n] contiguous blocks; the
per-core reduction of the gathered [8, 128*n] block is done on the
tensor engine (ones-vector matmul), which keeps the sync window short.
"""
import sys

for _p in ("/opt/trn_rl_repo", "/root/.axon_site/_ro/trn_rl_repo"):
    if _p not in sys.path:
        sys.path.append(_p)

import numpy as np
import ml_dtypes

import concourse.bass as bass
import concourse.mybir as mybir
import concourse.tile as tile
from concourse import bacc
from concourse.masks import make_identity

N_CORES = 8
B, CIN, H, W = 64, 1024, 28, 28
MID, COUT, EMB = 256, 1024, 64
EPS = 1e-5
HWP = H * W          # 784
NH = 392             # matmul free-dim chunk (2 per 784)
BL = B // N_CORES    # samples per core
NTOT = float(B * HWP)

f32 = mybir.dt.float32
bf16 = mybir.dt.bfloat16
AF = mybir.ActivationFunctionType
ALU = mybir.AluOpType
X = mybir.AxisListType.X

bfnp = ml_dtypes.bfloat16


def _emit(nc, tc, ctx, bl):
    ntot = float(N_CORES * bl * HWP)
    rg = [list(range(N_CORES))]

    # ---------------- DRAM I/O ----------------
    x_ext = nc.dram_tensor("x", [bl, CIN, HWP], f32, kind="ExternalInput")
    embt_ext = nc.dram_tensor("embt", [EMB, bl], bf16, kind="ExternalInput")
    w1_ext = nc.dram_tensor("w1t", [128, 8, MID], bf16, kind="ExternalInput")
    w2_ext = nc.dram_tensor("w2t", [128, 2, 9, MID], bf16, kind="ExternalInput")
    w3_ext = nc.dram_tensor("w3t", [128, 2, COUT], bf16, kind="ExternalInput")
    wg1_ext = nc.dram_tensor("wg1", [EMB, MID], bf16, kind="ExternalInput")
    wg2_ext = nc.dram_tensor("wg2", [EMB, MID], bf16, kind="ExternalInput")
    bg1_ext = nc.dram_tensor("bg1", [128, 2], f32, kind="ExternalInput")
    bg2_ext = nc.dram_tensor("bg2", [128, 2], f32, kind="ExternalInput")
    bn1g_ext = nc.dram_tensor("bn1g", [128, 2], f32, kind="ExternalInput")
    bn1b_ext = nc.dram_tensor("bn1b", [128, 2], f32, kind="ExternalInput")
    bn2g_ext = nc.dram_tensor("bn2g", [128, 2], f32, kind="ExternalInput")
    bn2b_ext = nc.dram_tensor("bn2b", [128, 2], f32, kind="ExternalInput")
    bn3g_ext = nc.dram_tensor("bn3g", [128, 8], f32, kind="ExternalInput")
    bn3b_ext = nc.dram_tensor("bn3b", [128, 8], f32, kind="ExternalInput")

    out_ext = nc.dram_tensor("out", [bl, COUT, HWP], f32, kind="ExternalOutput")
    g1_ext = nc.dram_tensor("g1", [bl, MID], f32, kind="ExternalOutput")
    g2_ext = nc.dram_tensor("g2", [bl, MID], f32, kind="ExternalOutput")

    # internal DRAM bounce buffers for the sync-BN collectives
    ccw_in = nc.dram_tensor("ccw_in", [8], f32)
    ccw_out = nc.dram_tensor("ccw_out", [N_CORES, 8], f32, addr_space="Shared")
    cc1_in = nc.dram_tensor("cc1_in", [128, 4], f32)
    cc1_out = nc.dram_tensor("cc1_out", [N_CORES, 128 * 4], f32,
                             addr_space="Shared")
    cc2_in = nc.dram_tensor("cc2_in", [128, 4], f32)
    cc2_out = nc.dram_tensor("cc2_out", [N_CORES, 128 * 4], f32,
                             addr_space="Shared")
    cc3_in = nc.dram_tensor("cc3_in", [128, 10], f32)
    cc3_out = nc.dram_tensor("cc3_out", [N_CORES, 128 * 10], f32,
                             addr_space="Shared")

    # ---------------- pools ----------------
    singles = ctx.enter_context(tc.tile_pool(name="singles", bufs=1))
    xload = ctx.enter_context(tc.tile_pool(name="xload", bufs=5))
    ostage_p = ctx.enter_context(tc.tile_pool(name="ostage", bufs=4))
    scratch_p = ctx.enter_context(tc.tile_pool(name="scratch", bufs=2))
    psA = ctx.enter_context(tc.tile_pool(name="psA", bufs=3, space="PSUM"))
    psB = ctx.enter_context(tc.tile_pool(name="psB", bufs=2, space="PSUM"))

    # ---------------- persistent SBUF ----------------
    xres = singles.tile([128, 8, bl, HWP], bf16)
    act1 = singles.tile([128, 2, bl, 900], bf16)   # zero-padded 30x30
    act2 = singles.tile([128, 2, bl, HWP], bf16)

    # warm up the collectives firmware with a dummy 8-float AllGather,
    # issued before everything else so its data plane doesn't contend
    # with the x loads
    ones8 = singles.tile([8, 1], f32)
    nc.vector.memset(ones8, 1.0)
    nc.sync.dma_start(out=ccw_in[:], in_=ones8[:, 0])
    nc.gpsimd.collective_compute(
        "AllGather", ALU.bypass, replica_groups=rg,
        ins=[ccw_in[:]], outs=[ccw_out[:, :]])

    # x for the first samples first: conv1 starts as soon as possible
    w1s = singles.tile([128, 8, MID], bf16)
    xsts = {}
    for b in range(min(2, bl)):
        for ko in range(8):
            xst = xload.tile([128, HWP], f32, tag="xst", name=f"xst_{b}_{ko}")
            nc.sync.dma_start(out=xst, in_=x_ext[b, ko * 128:(ko + 1) * 128, :])
            xsts[(b, ko)] = xst
    nc.sync.dma_start(out=w1s, in_=w1_ext[:, :, :])

    w2s = singles.tile([128, 2, 9, MID], bf16)
    w3s = singles.tile([128, 2, COUT], bf16)
    wg1s = singles.tile([EMB, MID], bf16)
    wg2s = singles.tile([EMB, MID], bf16)
    embts = singles.tile([EMB, bl], bf16)
    nc.sync.dma_start(out=wg1s, in_=wg1_ext[:, :])
    nc.sync.dma_start(out=wg2s, in_=wg2_ext[:, :])
    nc.sync.dma_start(out=embts, in_=embt_ext[:, :])
    nc.sync.dma_start(out=w2s, in_=w2_ext[:, :, :, :])
    nc.sync.dma_start(out=w3s, in_=w3_ext[:, :, :])

    bg1 = singles.tile([128, 2], f32)
    bg2 = singles.tile([128, 2], f32)
    bn1g = singles.tile([128, 2], f32)
    bn1b = singles.tile([128, 2], f32)
    bn2g = singles.tile([128, 2], f32)
    bn2b = singles.tile([128, 2], f32)
    bn3g = singles.tile([128, 8], f32)
    bn3b = singles.tile([128, 8], f32)
    for t, e in ((bg1, bg1_ext), (bg2, bg2_ext), (bn1g, bn1g_ext),
                 (bn1b, bn1b_ext), (bn2g, bn2g_ext), (bn2b, bn2b_ext),
                 (bn3g, bn3g_ext), (bn3b, bn3b_ext)):
        nc.sync.dma_start(out=t, in_=e[:, :])

    nc.gpsimd.memset(act1, 0.0)

    ident = singles.tile([128, 128], bf16)
    make_identity(nc, ident)

    # stats accumulators
    sum1 = singles.tile([128, 2 * bl], f32)
    sq1 = singles.tile([128, 2 * bl], f32)
    sum2 = singles.tile([128, 2 * bl], f32)
    sq2 = singles.tile([128, 2 * bl], f32)
    sq3 = singles.tile([128, 8 * bl], f32)
    S2acc = singles.tile([128, 2 * bl], f32)

    # ---------------- gates ----------------
    g1s = singles.tile([128, 2, bl], f32)
    g2s = singles.tile([128, 2, bl], f32)
    for gs, wgs, bgs, gext in ((g1s, wg1s, bg1, g1_ext),
                               (g2s, wg2s, bg2, g2_ext)):
        for mo in range(2):
            gps = psB.tile([128, bl], f32, tag="small", name=f"gps_{mo}")
            nc.tensor.matmul(gps[:, :], lhsT=wgs[:, mo * 128:(mo + 1) * 128],
                             rhs=embts, start=True, stop=True)
            nc.scalar.activation(out=gs[:, mo, :], in_=gps[:, :], func=AF.Relu,
                                 bias=bgs[:, mo:mo + 1], scale=1.0)
        for mo in range(2):
            nc.sync.dma_start(
                out=gext[:, mo * 128:(mo + 1) * 128].rearrange("b ki -> ki b"),
                in_=gs[:, mo, :])

    def cross_core_sum(tag, part, nch, cc_in, cc_out):
        """AllGather a [128, nch] fp32 partial block, then sum over the 8
        cores with ones-vector matmuls on the tensor engine.
        Returns a [128, nch] fp32 tile of global sums."""
        nc.sync.dma_start(out=cc_in[:, :], in_=part)
        nc.gpsimd.collective_compute(
            "AllGather", ALU.bypass, replica_groups=rg,
            ins=[cc_in[:, :]], outs=[cc_out[:, :]])
        gath = scratch_p.tile([8, 128 * nch], f32, tag=f"gath{nch}",
                              name=f"gath_{tag}", bufs=1)
        nc.sync.dma_start(out=gath, in_=cc_out[:, :])
        gv = gath.rearrange("c (ki n) -> c ki n", n=nch)
        ps = psB.tile([128, nch], f32, tag="small", name=f"ccps_{tag}")
        for j in range(nch):
            nc.tensor.matmul(ps[:, j:j + 1], lhsT=gv[:, :, j], rhs=ones8,
                             start=True, stop=True)
        g = singles.tile([128, nch], f32, name=f"glob_{tag}")
        nc.vector.tensor_copy(out=g, in_=ps)
        return g

    def bn_scale_bias(glob, soff, qoff, gamma, beta, nch, tag):
        """glob: [128, *] with sums at soff and sum-sq at qoff.
        Returns (s, t): s = gamma*rsqrt(var+eps), t = beta - mean*s."""
        m = singles.tile([128, nch], f32, name=f"m_{tag}")
        ey2 = singles.tile([128, nch], f32, name=f"ey2_{tag}")
        nc.vector.tensor_scalar_mul(m, glob[:, soff:soff + nch], 1.0 / ntot)
        nc.vector.tensor_scalar_mul(ey2, glob[:, qoff:qoff + nch], 1.0 / ntot)
        var = singles.tile([128, nch], f32, name=f"var_{tag}")
        nc.vector.tensor_tensor(out=var, in0=m, in1=m, op=ALU.mult)
        nc.vector.tensor_tensor(out=var, in0=ey2, in1=var, op=ALU.subtract)
        std = singles.tile([128, nch], f32, name=f"std_{tag}")
        nc.vector.tensor_scalar_add(var, var, EPS)
        nc.scalar.activation(out=std, in_=var, func=AF.Sqrt)
        rstd = singles.tile([128, nch], f32, name=f"rstd_{tag}")
        nc.vector.reciprocal(rstd, std)
        s = singles.tile([128, nch], f32, name=f"s_{tag}")
        t = singles.tile([128, nch], f32, name=f"t_{tag}")
        nc.vector.tensor_tensor(out=s, in0=gamma, in1=rstd, op=ALU.mult)
        nc.vector.tensor_tensor(out=t, in0=m, in1=s, op=ALU.mult)
        nc.vector.tensor_tensor(out=t, in0=beta, in1=t, op=ALU.subtract)
        return s, t

    # ---------------- phase 1: load x, conv1, bn1 stats ----------------
    for b in range(bl):
        bts = {}
        for ko in range(8):
            if (b, ko) in xsts:
                bts[ko] = xsts[(b, ko)]
            else:
                xst = xload.tile([128, HWP], f32, tag="xst",
                                 name=f"xst_{b}_{ko}")
                nc.sync.dma_start(out=xst,
                                  in_=x_ext[b, ko * 128:(ko + 1) * 128, :])
                bts[ko] = xst
        pts = [psA.tile([128, 2, 512], f32, tag="mm", name=f"c1_{b}_{mo}")
               for mo in range(2)]
        for ko in range(8):
            nc.vector.tensor_copy(out=xres[:, ko, b, :], in_=bts[ko])
            for mo in range(2):
                for n in range(2):
                    nc.tensor.matmul(
                        pts[mo][:, n, :NH],
                        lhsT=w1s[:, ko, mo * 128:(mo + 1) * 128],
                        rhs=xres[:, ko, b, n * NH:(n + 1) * NH],
                        start=(ko == 0), stop=(ko == 7))
        for mo in range(2):
            # gated PSUM->SBUF into padded act1 interior, fused sum;
            # the square pass reads the (pre-gating) PSUM in parallel and
            # the g^2 factor is applied at partial-reduction time
            dst = act1[:, mo, b, :].rearrange("p (r c) -> p r c", c=30)[
                :, 1:29, 1:29].rearrange("p (u r) c -> p u r c", u=2)
            src = pts[mo][:, :, :NH].rearrange("p u (r c) -> p u r c", c=28)
            idx = mo * bl + b
            nc.vector.tensor_scalar(
                out=dst, in0=src, scalar1=g1s[:, mo, b:b + 1], scalar2=None,
                op0=ALU.mult, op1=ALU.add, accum_out=sum1[:, idx:idx + 1])
            sc = ostage_p.tile([128, HWP], bf16, tag="ost", name=f"sq1_{b}_{mo}")
            nc.scalar.activation(
                out=sc.rearrange("p (u c) -> p u c", u=2),
                in_=pts[mo][:, :, :NH], func=AF.Square,
                accum_out=sq1[:, idx:idx + 1])

    # bn1 sync (sq1 holds sums of the pre-gating squares: weight by g^2)
    p1 = singles.tile([128, 4], f32)
    g1sq = singles.tile([128, 2 * bl], f32)
    g1f = g1s.rearrange("p m b -> p (m b)")
    nc.vector.tensor_tensor(out=g1sq, in0=g1f, in1=g1f, op=ALU.mult)
    nc.vector.tensor_tensor(out=g1sq, in0=g1sq, in1=sq1, op=ALU.mult)
    nc.vector.tensor_reduce(out=p1[:, 0:2], in_=sum1.rearrange(
        "p (mo b) -> p mo b", b=bl), axis=X, op=ALU.add)
    nc.vector.tensor_reduce(out=p1[:, 2:4], in_=g1sq.rearrange(
        "p (mo b) -> p mo b", b=bl), axis=X, op=ALU.add)
    glob1 = cross_core_sum("bn1", p1, 4, cc1_in, cc1_out)
    s1, t1 = bn_scale_bias(glob1, 0, 2, bn1g, bn1b, 2, "bn1")

    # bn1 apply (+ReLU), in place, chunked per sample
    for b in range(bl):
        for mo in range(2):
            v = act1[:, mo, b, :].rearrange("p (r c) -> p r c", c=30)[
                :, 1:29, 1:29]
            nc.scalar.activation(out=v, in_=v, func=AF.Relu,
                                 bias=t1[:, mo:mo + 1], scale=s1[:, mo:mo + 1])

    # ---------------- phase 2: conv2 3x3, bn2 stats ----------------
    for mo in range(2):
        for bp in range((bl + 1) // 2):
            pair = tuple(range(2 * bp, min(2 * bp + 2, bl)))
            pts = {b: psA.tile([128, 2, 512], f32, tag="mm",
                               name=f"c2_{b}_{mo}") for b in pair}
            a1v = {b: act1[:, :, b, :].rearrange("p k (r c) -> p k r c", c=30)
                   for b in pair}
            k = 0
            for ko in range(2):
                for tap in range(9):
                    dy, dx = tap // 3, tap % 3
                    for b in pair:
                        for rc in range(2):
                            rhs = a1v[b][:, ko,
                                         14 * rc + dy:14 * rc + dy + 14,
                                         dx:dx + 28]
                            nc.tensor.matmul(
                                pts[b][:, rc, :NH],
                                lhsT=w2s[:, ko, tap, mo * 128:(mo + 1) * 128],
                                rhs=rhs, start=(k == 0), stop=(k == 17))
                    k += 1
            for b in pair:
                dst = act2[:, mo, b, :].rearrange("p (u r c) -> p u r c",
                                                  u=2, c=28)
                src = pts[b][:, :, :NH].rearrange("p u (r c) -> p u r c", c=28)
                idx = mo * bl + b
                nc.vector.tensor_scalar(
                    out=dst, in0=src, scalar1=g2s[:, mo, b:b + 1],
                    scalar2=None, op0=ALU.mult, op1=ALU.add,
                    accum_out=sum2[:, idx:idx + 1])
                sc = ostage_p.tile([128, HWP], bf16, tag="ost",
                                    name=f"sq2_{b}_{mo}")
                nc.scalar.activation(
                    out=sc.rearrange("p (u c) -> p u c", u=2),
                    in_=pts[b][:, :, :NH], func=AF.Square,
                    accum_out=sq2[:, idx:idx + 1])

    # bn2 sync (sq2 holds sums of the pre-gating squares: weight by g^2)
    p2 = singles.tile([128, 4], f32)
    g2sq = singles.tile([128, 2 * bl], f32)
    g2f = g2s.rearrange("p m b -> p (m b)")
    nc.vector.tensor_tensor(out=g2sq, in0=g2f, in1=g2f, op=ALU.mult)
    nc.vector.tensor_tensor(out=g2sq, in0=g2sq, in1=sq2, op=ALU.mult)
    nc.vector.tensor_reduce(out=p2[:, 0:2], in_=sum2.rearrange(
        "p (mo b) -> p mo b", b=bl), axis=X, op=ALU.add)
    nc.vector.tensor_reduce(out=p2[:, 2:4], in_=g2sq.rearrange(
        "p (mo b) -> p mo b", b=bl), axis=X, op=ALU.add)
    glob2 = cross_core_sum("bn2", p2, 4, cc2_in, cc2_out)
    s2, t2 = bn_scale_bias(glob2, 0, 2, bn2g, bn2b, 2, "bn2")

    # bn2 apply (+ReLU) with fused S2 = sum(act2) accumulation
    for b in range(bl):
        for mo in range(2):
            v = act2[:, mo, b, :]
            idx = mo * bl + b
            nc.scalar.activation(out=v, in_=v, func=AF.Relu,
                                 bias=t2[:, mo:mo + 1], scale=s2[:, mo:mo + 1],
                                 accum_out=S2acc[:, idx:idx + 1])

    # ---------------- phase 3: conv3 pass 1 (stats only) ----------------
    for mo in range(8):
        for bp in range((bl + 1) // 2):
            pair = tuple(range(2 * bp, min(2 * bp + 2, bl)))
            pts = {b: psA.tile([128, 2, 512], f32, tag="mm",
                               name=f"c3a_{b}_{mo}") for b in pair}
            for ko in range(2):
                for b in pair:
                    for n in range(2):
                        nc.tensor.matmul(
                            pts[b][:, n, :NH],
                            lhsT=w3s[:, ko, mo * 128:(mo + 1) * 128],
                            rhs=act2[:, ko, b, n * NH:(n + 1) * NH],
                            start=(ko == 0), stop=(ko == 1))
            for b in pair:
                sc = ostage_p.tile([128, HWP], bf16, tag="ost",
                                    name=f"sq3_{b}_{mo}")
                idx = mo * bl + b
                nc.scalar.activation(
                    out=sc.rearrange("p (u c) -> p u c", u=2),
                    in_=pts[b][:, :, :NH], func=AF.Square,
                    accum_out=sq3[:, idx:idx + 1])

    # bn3 sync: sum(y^2) partials + S2 partials in one AllGather
    p3 = singles.tile([128, 10], f32)
    nc.vector.tensor_reduce(out=p3[:, 0:8], in_=sq3.rearrange(
        "p (mo b) -> p mo b", b=bl), axis=X, op=ALU.add)
    nc.vector.tensor_reduce(out=p3[:, 8:10], in_=S2acc.rearrange(
        "p (mo b) -> p mo b", b=bl), axis=X, op=ALU.add)
    glob3 = cross_core_sum("bn3", p3, 10, cc3_in, cc3_out)

    # mean3 via W3 @ S2  (exploits linearity of the 1x1 conv)
    S2b = singles.tile([128, 2], bf16)
    nc.vector.tensor_copy(out=S2b, in_=glob3[:, 8:10])
    pm = psB.tile([128, 8], f32, tag="small")
    for mo in range(8):
        for ko in range(2):
            nc.tensor.matmul(pm[:, mo:mo + 1],
                             lhsT=w3s[:, ko, mo * 128:(mo + 1) * 128],
                             rhs=S2b[:, ko:ko + 1],
                             start=(ko == 0), stop=(ko == 1))
    g3full = singles.tile([128, 16], f32)
    nc.vector.tensor_copy(out=g3full[:, 0:8], in_=pm)
    nc.vector.tensor_copy(out=g3full[:, 8:16], in_=glob3[:, 0:8])
    s3, t3 = bn_scale_bias(g3full, 0, 8, bn3g, bn3b, 8, "bn3")

    # diag(1/s3) in bf16 for the in-PSUM residual add
    invs3 = singles.tile([128, 8], f32)
    nc.vector.reciprocal(invs3, s3)
    diag3 = singles.tile([128, 8, 128], bf16)
    for mo in range(8):
        nc.vector.tensor_scalar_mul(diag3[:, mo, :], ident,
                                    invs3[:, mo:mo + 1])

    # ---------------- phase 4: conv3 pass 2 + residual + out ----------------
    for mo in range(8):
        for bp in range((bl + 1) // 2):
            pair = tuple(range(2 * bp, min(2 * bp + 2, bl)))
            pts = {b: psA.tile([128, 2, 512], f32, tag="mm",
                               name=f"c3b_{b}_{mo}") for b in pair}
            for ko in range(2):
                for b in pair:
                    for n in range(2):
                        nc.tensor.matmul(
                            pts[b][:, n, :NH],
                            lhsT=w3s[:, ko, mo * 128:(mo + 1) * 128],
                            rhs=act2[:, ko, b, n * NH:(n + 1) * NH],
                            start=(ko == 0), stop=False)
            for b in pair:
                for n in range(2):
                    nc.tensor.matmul(
                        pts[b][:, n, :NH],
                        lhsT=diag3[:, mo, :],
                        rhs=xres[:, mo, b, n * NH:(n + 1) * NH],
                        start=False, stop=True)
            for b in pair:
                ost = ostage_p.tile([128, HWP], f32, tag="ost",
                                    name=f"ost_{b}_{mo}")
                if (b * 8 + mo) % 2 == 0:
                    nc.scalar.activation(
                        out=ost.rearrange("p (u c) -> p u c", u=2),
                        in_=pts[b][:, :, :NH], func=AF.Relu,
                        scale=s3[:, mo:mo + 1], bias=t3[:, mo:mo + 1])
                else:
                    nc.vector.tensor_scalar(
                        out=ost.rearrange("p (u c) -> p u c", u=2),
                        in0=pts[b][:, :, :NH], scalar1=s3[:, mo:mo + 1],
                        scalar2=t3[:, mo:mo + 1], op0=ALU.mult, op1=ALU.add)
                    nc.vector.tensor_scalar_max(ost, ost, 0.0)
                nc.sync.dma_start(out=out_ext[b, mo * 128:(mo + 1) * 128, :],
                                  in_=ost)


def build(bl=BL):
    nc = bacc.Bacc("TRN2", target_bir_lowering=False, debug=False,
                   num_devices=N_CORES)
    from contextlib import ExitStack
    with tile.TileContext(nc) as tc, ExitStack() as ctx:
        _emit(nc, tc, ctx, bl)
    nc.compile()
    return nc


def prep_weights(inputs):
    """Host-side reshape/cast of the (small) replicated weights."""
    w1 = np.asarray(inputs["conv1_w"], np.float32).reshape(MID, CIN)
    w2 = np.asarray(inputs["conv2_w"], np.float32)
    w3 = np.asarray(inputs["conv3_w"], np.float32).reshape(COUT, MID)

    w1t = np.ascontiguousarray(
        w1.reshape(MID, 8, 128).transpose(2, 1, 0)).astype(bfnp)
    w2t = np.ascontiguousarray(
        w2.reshape(MID, 2, 128, 9).transpose(2, 1, 3, 0)).astype(bfnp)
    w3t = np.ascontiguousarray(
        w3.reshape(COUT, 2, 128).transpose(2, 1, 0)).astype(bfnp)

    def chan_tile(v, nch):
        return np.ascontiguousarray(
            np.asarray(v, np.float32).reshape(nch, 128).T)

    return {
        "w1t": w1t, "w2t": w2t, "w3t": w3t,
        "wg1": np.asarray(inputs["w_gate1"], np.float32).astype(bfnp),
        "wg2": np.asarray(inputs["w_gate2"], np.float32).astype(bfnp),
        "bg1": chan_tile(inputs["b_gate1"], 2),
        "bg2": chan_tile(inputs["b_gate2"], 2),
        "bn1g": chan_tile(inputs["bn1_g"], 2),
        "bn1b": chan_tile(inputs["bn1_b"], 2),
        "bn2g": chan_tile(inputs["bn2_g"], 2),
        "bn2b": chan_tile(inputs["bn2_b"], 2),
        "bn3g": chan_tile(inputs["bn3_g"], 8),
        "bn3b": chan_tile(inputs["bn3_b"], 8),
    }


def make_in_maps(inputs, bl=BL):
    x = np.asarray(inputs["x"], np.float32).reshape(B, CIN, HWP)
    emb = np.asarray(inputs["embeddings"], np.float32)
    w = prep_weights(inputs)
    in_maps = []
    for c in range(N_CORES):
        sl = slice(c * bl, (c + 1) * bl)
        m = dict(w)
        m["x"] = x[sl]
        m["embt"] = np.ascontiguousarray(emb[sl].T).astype(bfnp)
        in_maps.append(m)
    return in_maps


_built = {}


def _get_nc():
    if "nc" not in _built:
        _built["nc"] = build(BL)
    return _built["nc"]


def kernel(**inputs):
    from concourse.bass_utils import run_bass_kernel_spmd
    nc = _get_nc()
    in_maps = make_in_maps(inputs)
    res = run_bass_kernel_spmd(nc, in_maps, list(range(N_CORES)))
    outs = [r["out"].reshape(BL, COUT, H, W) for r in res.results]
    g1s = [r["g1"].reshape(BL, MID) for r in res.results]
    g2s = [r["g2"].reshape(BL, MID) for r in res.results]
    out = np.concatenate(outs, axis=0)
    g1 = np.concatenate(g1s, axis=0)
    g2 = np.concatenate(g2s, axis=0)
    return out, g1, g2


if __name__ == "__main__":
    nc = build(BL)
    print("built + compiled ok")


# revision 25
# speedup vs baseline: 1.2947x; 1.0038x over previous
"""Trainium2 Bass kernel for nn_MoEBottleneckA (MoE bottleneck block).

Strategy: data-parallel over batch (64 -> 8 samples per core, 8 cores),
weights replicated. Training-mode BatchNorm stats are synchronized with
small AllGather collectives (sync-BN). All matmuls in bf16 (fp32 PSUM
accumulation); BN statistics and normalization in fp32.

Per core:
  conv1 (1x1, 1024->256) as 8-chunk K-accumulated matmuls, gated by g1,
  BN1 partial stats fused into the PSUM->SBUF epilogues; AllGather;
  BN1 apply (+ReLU) in place; conv2 (3x3 SAME) as 18 shifted matmuls on a
  zero-padded 30x30 activation; gate g2 + BN2 stats; AllGather; BN2 apply
  with fused S2 = sum(act2) accumulation; conv3 (1x1, 256->1024) pass 1
  computes sum(y^2) partials (mean comes free via W3 @ S2); AllGather;
  conv3 pass 2 recomputes y, adds the residual inside PSUM via a
  diag(1/s3) matmul of x, and Relu(scale,bias) epilogues (split between
  the scalar and vector engines) write the final fp32 output.

Cross-core partial sums travel as [128, n] contiguous blocks; the
per-core reduction of the gathered [8, 128*n] block is done on the
tensor engine (ones-vector matmul), which keeps the sync window short.
"""
import sys

for _p in ("/opt/trn_rl_repo", "/root/.axon_site/_ro/trn_rl_repo"):
    if _p not in sys.path:
        sys.path.append(_p)

import numpy as np
import ml_dtypes

import concourse.bass as bass
import concourse.mybir as mybir
import concourse.tile as tile
from concourse import bacc
from concourse.masks import make_identity

N_CORES = 8
B, CIN, H, W = 64, 1024, 28, 28
MID, COUT, EMB = 256, 1024, 64
EPS = 1e-5
HWP = H * W          # 784
NH = 392             # matmul free-dim chunk (2 per 784)
BL = B // N_CORES    # samples per core
NTOT = float(B * HWP)

f32 = mybir.dt.float32
bf16 = mybir.dt.bfloat16
AF = mybir.ActivationFunctionType
ALU = mybir.AluOpType
X = mybir.AxisListType.X

bfnp = ml_dtypes.bfloat16


def _emit(nc, tc, ctx, bl):
    ntot = float(N_CORES * bl * HWP)
    rg = [list(range(N_CORES))]

    # ---------------- DRAM I/O ----------------
    x_ext = nc.dram_tensor("x", [bl, CIN, HWP], f32, kind="ExternalInput")
    embt_ext = nc.dram_tensor("embt", [EMB, bl], bf16, kind="ExternalInput")
    w1_ext = nc.dram_tensor("w1t", [128, 8, MID], bf16, kind="ExternalInput")
    w2_ext = nc.dram_tensor("w2t", [128, 2, 9, MID], bf16, kind="ExternalInput")
    w3_ext = nc.dram_tensor("w3t", [128, 2, COUT], bf16, kind="ExternalInput")
    wg1_ext = nc.dram_tensor("wg1", [EMB, MID], bf16, kind="ExternalInput")
    wg2_ext = nc.dram_tensor("wg2", [EMB, MID], bf16, kind="ExternalInput")
    bg1_ext = nc.dram_tensor("bg1", [128, 2], f32, kind="ExternalInput")
    bg2_ext = nc.dram_tensor("bg2", [128, 2], f32, kind="ExternalInput")
    bn1g_ext = nc.dram_tensor("bn1g", [128, 2], f32, kind="ExternalInput")
    bn1b_ext = nc.dram_tensor("bn1b", [128, 2], f32, kind="ExternalInput")
    bn2g_ext = nc.dram_tensor("bn2g", [128, 2], f32, kind="ExternalInput")
    bn2b_ext = nc.dram_tensor("bn2b", [128, 2], f32, kind="ExternalInput")
    bn3g_ext = nc.dram_tensor("bn3g", [128, 8], f32, kind="ExternalInput")
    bn3b_ext = nc.dram_tensor("bn3b", [128, 8], f32, kind="ExternalInput")

    out_ext = nc.dram_tensor("out", [bl, COUT, HWP], f32, kind="ExternalOutput")
    g1_ext = nc.dram_tensor("g1", [bl, MID], f32, kind="ExternalOutput")
    g2_ext = nc.dram_tensor("g2", [bl, MID], f32, kind="ExternalOutput")

    # internal DRAM bounce buffers for the sync-BN collectives
    ccw_in = nc.dram_tensor("ccw_in", [8], f32)
    ccw_out = nc.dram_tensor("ccw_out", [N_CORES, 8], f32, addr_space="Shared")
    cc1_in = nc.dram_tensor("cc1_in", [128, 4], f32)
    cc1_out = nc.dram_tensor("cc1_out", [N_CORES, 128 * 4], f32,
                             addr_space="Shared")
    cc2_in = nc.dram_tensor("cc2_in", [128, 4], f32)
    cc2_out = nc.dram_tensor("cc2_out", [N_CORES, 128 * 4], f32,
                             addr_space="Shared")
    cc3_in = nc.dram_tensor("cc3_in", [128, 10], f32)
    cc3_out = nc.dram_tensor("cc3_out", [N_CORES, 128 * 10], f32,
                             addr_space="Shared")

    # ---------------- pools ----------------
    singles = ctx.enter_context(tc.tile_pool(name="singles", bufs=1))
    xload = ctx.enter_context(tc.tile_pool(name="xload", bufs=5))
    ostage_p = ctx.enter_context(tc.tile_pool(name="ostage", bufs=4))
    scratch_p = ctx.enter_context(tc.tile_pool(name="scratch", bufs=2))
    psA = ctx.enter_context(tc.tile_pool(name="psA", bufs=3, space="PSUM"))
    psB = ctx.enter_context(tc.tile_pool(name="psB", bufs=2, space="PSUM"))

    # ---------------- persistent SBUF ----------------
    xres = singles.tile([128, 8, bl, HWP], bf16)
    act1 = singles.tile([128, 2, bl, 900], bf16)   # zero-padded 30x30
    act2 = singles.tile([128, 2, bl, HWP], bf16)

    # warm up the collectives firmware with a dummy 8-float AllGather,
    # issued before everything else so its data plane doesn't contend
    # with the x loads
    ones8 = singles.tile([8, 1], f32)
    nc.vector.memset(ones8, 1.0)
    nc.sync.dma_start(out=ccw_in[:], in_=ones8[:, 0])
    nc.gpsimd.collective_compute(
        "AllGather", ALU.bypass, replica_groups=rg,
        ins=[ccw_in[:]], outs=[ccw_out[:, :]])

    # x for the first samples first: conv1 starts as soon as possible
    w1s = singles.tile([128, 8, MID], bf16)
    xsts = {}
    for b in range(min(2, bl)):
        for ko in range(8):
            xst = xload.tile([128, HWP], f32, tag="xst", name=f"xst_{b}_{ko}")
            nc.sync.dma_start(out=xst, in_=x_ext[b, ko * 128:(ko + 1) * 128, :])
            xsts[(b, ko)] = xst
    nc.sync.dma_start(out=w1s, in_=w1_ext[:, :, :])

    w2s = singles.tile([128, 2, 9, MID], bf16)
    w3s = singles.tile([128, 2, COUT], bf16)
    wg1s = singles.tile([EMB, MID], bf16)
    wg2s = singles.tile([EMB, MID], bf16)
    embts = singles.tile([EMB, bl], bf16)
    nc.sync.dma_start(out=wg1s, in_=wg1_ext[:, :])
    nc.sync.dma_start(out=wg2s, in_=wg2_ext[:, :])
    nc.sync.dma_start(out=embts, in_=embt_ext[:, :])
    nc.sync.dma_start(out=w2s, in_=w2_ext[:, :, :, :])
    nc.sync.dma_start(out=w3s, in_=w3_ext[:, :, :])

    bg1 = singles.tile([128, 2], f32)
    bg2 = singles.tile([128, 2], f32)
    bn1g = singles.tile([128, 2], f32)
    bn1b = singles.tile([128, 2], f32)
    bn2g = singles.tile([128, 2], f32)
    bn2b = singles.tile([128, 2], f32)
    bn3g = singles.tile([128, 8], f32)
    bn3b = singles.tile([128, 8], f32)
    for t, e in ((bg1, bg1_ext), (bg2, bg2_ext), (bn1g, bn1g_ext),
                 (bn1b, bn1b_ext), (bn2g, bn2g_ext), (bn2b, bn2b_ext),
                 (bn3g, bn3g_ext), (bn3b, bn3b_ext)):
        nc.sync.dma_start(out=t, in_=e[:, :])

    nc.gpsimd.memset(act1, 0.0)

    ident = singles.tile([128, 128], bf16)
    make_identity(nc, ident)

    # stats accumulators
    sum1 = singles.tile([128, 2 * bl], f32)
    sq1 = singles.tile([128, 2 * bl], f32)
    sum2 = singles.tile([128, 2 * bl], f32)
    sq2 = singles.tile([128, 2 * bl], f32)
    sq3 = singles.tile([128, 8 * bl], f32)
    S2acc = singles.tile([128, 2 * bl], f32)

    # ---------------- gates ----------------
    g1s = singles.tile([128, 2, bl], f32)
    g2s = singles.tile([128, 2, bl], f32)
    for gs, wgs, bgs, gext in ((g1s, wg1s, bg1, g1_ext),
                               (g2s, wg2s, bg2, g2_ext)):
        for mo in range(2):
            gps = psB.tile([128, bl], f32, tag="small", name=f"gps_{mo}")
            nc.tensor.matmul(gps[:, :], lhsT=wgs[:, mo * 128:(mo + 1) * 128],
                             rhs=embts, start=True, stop=True)
            nc.scalar.activation(out=gs[:, mo, :], in_=gps[:, :], func=AF.Relu,
                                 bias=bgs[:, mo:mo + 1], scale=1.0)
        for mo in range(2):
            nc.sync.dma_start(
                out=gext[:, mo * 128:(mo + 1) * 128].rearrange("b ki -> ki b"),
                in_=gs[:, mo, :])

    def cross_core_sum(tag, part, nch, cc_in, cc_out):
        """AllGather a [128, nch] fp32 partial block, then sum over the 8
        cores with ones-vector matmuls on the tensor engine.
        Returns a [128, nch] fp32 tile of global sums."""
        nc.sync.dma_start(out=cc_in[:, :], in_=part)
        nc.gpsimd.collective_compute(
            "AllGather", ALU.bypass, replica_groups=rg,
            ins=[cc_in[:, :]], outs=[cc_out[:, :]])
        gath = scratch_p.tile([8, 128 * nch], f32, tag=f"gath{nch}",
                              name=f"gath_{tag}", bufs=1)
        nc.sync.dma_start(out=gath, in_=cc_out[:, :])
        gv = gath.rearrange("c (ki n) -> c ki n", n=nch)
        ps = psB.tile([128, nch], f32, tag="small", name=f"ccps_{tag}")
        for j in range(nch):
            nc.tensor.matmul(ps[:, j:j + 1], lhsT=gv[:, :, j], rhs=ones8,
                             start=True, stop=True)
        g = singles.tile([128, nch], f32, name=f"glob_{tag}")
        nc.vector.tensor_copy(out=g, in_=ps)
        return g

    def bn_scale_bias(glob, soff, qoff, gamma, beta, nch, tag):
        """glob: [128, *] with sums at soff and sum-sq at qoff.
        Returns (s, t): s = gamma*rsqrt(var+eps), t = beta - mean*s."""
        m = singles.tile([128, nch], f32, name=f"m_{tag}")
        ey2 = singles.tile([128, nch], f32, name=f"ey2_{tag}")
        nc.vector.tensor_scalar_mul(m, glob[:, soff:soff + nch], 1.0 / ntot)
        nc.vector.tensor_scalar_mul(ey2, glob[:, qoff:qoff + nch], 1.0 / ntot)
        var = singles.tile([128, nch], f32, name=f"var_{tag}")
        nc.vector.tensor_tensor(out=var, in0=m, in1=m, op=ALU.mult)
        nc.vector.tensor_tensor(out=var, in0=ey2, in1=var, op=ALU.subtract)
        std = singles.tile([128, nch], f32, name=f"std_{tag}")
        nc.vector.tensor_scalar_add(var, var, EPS)
        nc.scalar.activation(out=std, in_=var, func=AF.Sqrt)
        rstd = singles.tile([128, nch], f32, name=f"rstd_{tag}")
        nc.vector.reciprocal(rstd, std)
        s = singles.tile([128, nch], f32, name=f"s_{tag}")
        t = singles.tile([128, nch], f32, name=f"t_{tag}")
        nc.vector.tensor_tensor(out=s, in0=gamma, in1=rstd, op=ALU.mult)
        nc.vector.tensor_tensor(out=t, in0=m, in1=s, op=ALU.mult)
        nc.vector.tensor_tensor(out=t, in0=beta, in1=t, op=ALU.subtract)
        return s, t

    # ---------------- phase 1: load x, conv1, bn1 stats ----------------
    # casts for the two prefetched samples
    for b in range(min(2, bl)):
        for ko in range(8):
            nc.vector.tensor_copy(out=xres[:, ko, b, :], in_=xsts[(b, ko)])
    for b in range(bl):
        # prefetch loads + casts for sample b+2 before b's epilogues are
        # emitted, so the DVE cast stream never stalls behind them
        bn = b + 2
        if bn < bl:
            for ko in range(8):
                xst = xload.tile([128, HWP], f32, tag="xst",
                                 name=f"xst_{bn}_{ko}")
                nc.sync.dma_start(out=xst,
                                  in_=x_ext[bn, ko * 128:(ko + 1) * 128, :])
                nc.vector.tensor_copy(out=xres[:, ko, bn, :], in_=xst)
        pts = [psA.tile([128, 2, 512], f32, tag="mm", name=f"c1_{b}_{mo}")
               for mo in range(2)]
        for ko in range(8):
            for mo in range(2):
                for n in range(2):
                    nc.tensor.matmul(
                        pts[mo][:, n, :NH],
                        lhsT=w1s[:, ko, mo * 128:(mo + 1) * 128],
                        rhs=xres[:, ko, b, n * NH:(n + 1) * NH],
                        start=(ko == 0), stop=(ko == 7))
        for mo in range(2):
            # gated PSUM->SBUF into padded act1 interior, fused sum;
            # the square pass reads the (pre-gating) PSUM in parallel and
            # the g^2 factor is applied at partial-reduction time
            dst = act1[:, mo, b, :].rearrange("p (r c) -> p r c", c=30)[
                :, 1:29, 1:29].rearrange("p (u r) c -> p u r c", u=2)
            src = pts[mo][:, :, :NH].rearrange("p u (r c) -> p u r c", c=28)
            idx = mo * bl + b
            nc.vector.tensor_scalar(
                out=dst, in0=src, scalar1=g1s[:, mo, b:b + 1], scalar2=None,
                op0=ALU.mult, op1=ALU.add, accum_out=sum1[:, idx:idx + 1])
            sc = ostage_p.tile([128, HWP], bf16, tag="ost", name=f"sq1_{b}_{mo}")
            nc.scalar.activation(
                out=sc.rearrange("p (u c) -> p u c", u=2),
                in_=pts[mo][:, :, :NH], func=AF.Square,
                accum_out=sq1[:, idx:idx + 1])

    # bn1 sync (sq1 holds sums of the pre-gating squares: weight by g^2)
    p1 = singles.tile([128, 4], f32)
    g1sq = singles.tile([128, 2 * bl], f32)
    g1f = g1s.rearrange("p m b -> p (m b)")
    nc.vector.tensor_tensor(out=g1sq, in0=g1f, in1=g1f, op=ALU.mult)
    nc.vector.tensor_tensor(out=g1sq, in0=g1sq, in1=sq1, op=ALU.mult)
    nc.vector.tensor_reduce(out=p1[:, 0:2], in_=sum1.rearrange(
        "p (mo b) -> p mo b", b=bl), axis=X, op=ALU.add)
    nc.vector.tensor_reduce(out=p1[:, 2:4], in_=g1sq.rearrange(
        "p (mo b) -> p mo b", b=bl), axis=X, op=ALU.add)
    glob1 = cross_core_sum("bn1", p1, 4, cc1_in, cc1_out)
    s1, t1 = bn_scale_bias(glob1, 0, 2, bn1g, bn1b, 2, "bn1")

    # bn1 apply (+ReLU), in place, chunked per sample
    for b in range(bl):
        for mo in range(2):
            v = act1[:, mo, b, :].rearrange("p (r c) -> p r c", c=30)[
                :, 1:29, 1:29]
            nc.scalar.activation(out=v, in_=v, func=AF.Relu,
                                 bias=t1[:, mo:mo + 1], scale=s1[:, mo:mo + 1])

    # ---------------- phase 2: conv2 3x3, bn2 stats ----------------
    for mo in range(2):
        for bp in range((bl + 1) // 2):
            pair = tuple(range(2 * bp, min(2 * bp + 2, bl)))
            pts = {b: psA.tile([128, 2, 512], f32, tag="mm",
                               name=f"c2_{b}_{mo}") for b in pair}
            a1v = {b: act1[:, :, b, :].rearrange("p k (r c) -> p k r c", c=30)
                   for b in pair}
            k = 0
            for ko in range(2):
                for tap in range(9):
                    dy, dx = tap // 3, tap % 3
                    for b in pair:
                        for rc in range(2):
                            rhs = a1v[b][:, ko,
                                         14 * rc + dy:14 * rc + dy + 14,
                                         dx:dx + 28]
                            nc.tensor.matmul(
                                pts[b][:, rc, :NH],
                                lhsT=w2s[:, ko, tap, mo * 128:(mo + 1) * 128],
                                rhs=rhs, start=(k == 0), stop=(k == 17))
                    k += 1
            for b in pair:
                dst = act2[:, mo, b, :].rearrange("p (u r c) -> p u r c",
                                                  u=2, c=28)
                src = pts[b][:, :, :NH].rearrange("p u (r c) -> p u r c", c=28)
                idx = mo * bl + b
                nc.vector.tensor_scalar(
                    out=dst, in0=src, scalar1=g2s[:, mo, b:b + 1],
                    scalar2=None, op0=ALU.mult, op1=ALU.add,
                    accum_out=sum2[:, idx:idx + 1])
                sc = ostage_p.tile([128, HWP], bf16, tag="ost",
                                    name=f"sq2_{b}_{mo}")
                nc.scalar.activation(
                    out=sc.rearrange("p (u c) -> p u c", u=2),
                    in_=pts[b][:, :, :NH], func=AF.Square,
                    accum_out=sq2[:, idx:idx + 1])

    # bn2 sync (sq2 holds sums of the pre-gating squares: weight by g^2)
    p2 = singles.tile([128, 4], f32)
    g2sq = singles.tile([128, 2 * bl], f32)
    g2f = g2s.rearrange("p m b -> p (m b)")
    nc.vector.tensor_tensor(out=g2sq, in0=g2f, in1=g2f, op=ALU.mult)
    nc.vector.tensor_tensor(out=g2sq, in0=g2sq, in1=sq2, op=ALU.mult)
    nc.vector.tensor_reduce(out=p2[:, 0:2], in_=sum2.rearrange(
        "p (mo b) -> p mo b", b=bl), axis=X, op=ALU.add)
    nc.vector.tensor_reduce(out=p2[:, 2:4], in_=g2sq.rearrange(
        "p (mo b) -> p mo b", b=bl), axis=X, op=ALU.add)
    glob2 = cross_core_sum("bn2", p2, 4, cc2_in, cc2_out)
    s2, t2 = bn_scale_bias(glob2, 0, 2, bn2g, bn2b, 2, "bn2")

    # bn2 apply (+ReLU) with fused S2 = sum(act2) accumulation
    for b in range(bl):
        for mo in range(2):
            v = act2[:, mo, b, :]
            idx = mo * bl + b
            nc.scalar.activation(out=v, in_=v, func=AF.Relu,
                                 bias=t2[:, mo:mo + 1], scale=s2[:, mo:mo + 1],
                                 accum_out=S2acc[:, idx:idx + 1])

    # ---------------- phase 3: conv3 pass 1 (stats only) ----------------
    for mo in range(8):
        for bp in range((bl + 1) // 2):
            pair = tuple(range(2 * bp, min(2 * bp + 2, bl)))
            pts = {b: psA.tile([128, 2, 512], f32, tag="mm",
                               name=f"c3a_{b}_{mo}") for b in pair}
            for ko in range(2):
                for b in pair:
                    for n in range(2):
                        nc.tensor.matmul(
                            pts[b][:, n, :NH],
                            lhsT=w3s[:, ko, mo * 128:(mo + 1) * 128],
                            rhs=act2[:, ko, b, n * NH:(n + 1) * NH],
                            start=(ko == 0), stop=(ko == 1))
            for b in pair:
                sc = ostage_p.tile([128, HWP], bf16, tag="ost",
                                    name=f"sq3_{b}_{mo}")
                idx = mo * bl + b
                if (b + mo) % 2 == 0:
                    nc.scalar.activation(
                        out=sc.rearrange("p (u c) -> p u c", u=2),
                        in_=pts[b][:, :, :NH], func=AF.Square,
                        accum_out=sq3[:, idx:idx + 1])
                else:
                    nc.vector.tensor_copy(
                        out=sc.rearrange("p (u c) -> p u c", u=2),
                        in_=pts[b][:, :, :NH])
                    sc2 = ostage_p.tile([128, HWP], bf16, tag="ost",
                                        name=f"sq3d_{b}_{mo}")
                    nc.vector.affine_mul_reduce(
                        out=sc2, accum_out=sq3[:, idx:idx + 1],
                        in0=sc, in1=sc, scale=1.0, bias=0.0)

    # bn3 sync: sum(y^2) partials + S2 partials in one AllGather
    p3 = singles.tile([128, 10], f32)
    nc.vector.tensor_reduce(out=p3[:, 0:8], in_=sq3.rearrange(
        "p (mo b) -> p mo b", b=bl), axis=X, op=ALU.add)
    nc.vector.tensor_reduce(out=p3[:, 8:10], in_=S2acc.rearrange(
        "p (mo b) -> p mo b", b=bl), axis=X, op=ALU.add)
    glob3 = cross_core_sum("bn3", p3, 10, cc3_in, cc3_out)

    # mean3 via W3 @ S2  (exploits linearity of the 1x1 conv)
    S2b = singles.tile([128, 2], bf16)
    nc.vector.tensor_copy(out=S2b, in_=glob3[:, 8:10])
    pm = psB.tile([128, 8], f32, tag="small")
    for mo in range(8):
        for ko in range(2):
            nc.tensor.matmul(pm[:, mo:mo + 1],
                             lhsT=w3s[:, ko, mo * 128:(mo + 1) * 128],
                             rhs=S2b[:, ko:ko + 1],
                             start=(ko == 0), stop=(ko == 1))
    g3full = singles.tile([128, 16], f32)
    nc.vector.tensor_copy(out=g3full[:, 0:8], in_=pm)
    nc.vector.tensor_copy(out=g3full[:, 8:16], in_=glob3[:, 0:8])
    s3, t3 = bn_scale_bias(g3full, 0, 8, bn3g, bn3b, 8, "bn3")

    # diag(1/s3) in bf16 for the in-PSUM residual add
    invs3 = singles.tile([128, 8], f32)
    nc.vector.reciprocal(invs3, s3)
    diag3 = singles.tile([128, 8, 128], bf16)
    for mo in range(8):
        nc.vector.tensor_scalar_mul(diag3[:, mo, :], ident,
                                    invs3[:, mo:mo + 1])

    # ---------------- phase 4: conv3 pass 2 + residual + out ----------------
    for mo in range(8):
        for bp in range((bl + 1) // 2):
            pair = tuple(range(2 * bp, min(2 * bp + 2, bl)))
            pts = {b: psA.tile([128, 2, 512], f32, tag="mm",
                               name=f"c3b_{b}_{mo}") for b in pair}
            for ko in range(2):
                for b in pair:
                    for n in range(2):
                        nc.tensor.matmul(
                            pts[b][:, n, :NH],
                            lhsT=w3s[:, ko, mo * 128:(mo + 1) * 128],
                            rhs=act2[:, ko, b, n * NH:(n + 1) * NH],
                            start=(ko == 0), stop=False)
            for b in pair:
                for n in range(2):
                    nc.tensor.matmul(
                        pts[b][:, n, :NH],
                        lhsT=diag3[:, mo, :],
                        rhs=xres[:, mo, b, n * NH:(n + 1) * NH],
                        start=False, stop=True)
            for b in pair:
                ost = ostage_p.tile([128, HWP], f32, tag="ost",
                                    name=f"ost_{b}_{mo}")
                if (b * 8 + mo) % 2 == 0:
                    nc.scalar.activation(
                        out=ost.rearrange("p (u c) -> p u c", u=2),
                        in_=pts[b][:, :, :NH], func=AF.Relu,
                        scale=s3[:, mo:mo + 1], bias=t3[:, mo:mo + 1])
                else:
                    nc.vector.tensor_scalar(
                        out=ost.rearrange("p (u c) -> p u c", u=2),
                        in0=pts[b][:, :, :NH], scalar1=s3[:, mo:mo + 1],
                        scalar2=t3[:, mo:mo + 1], op0=ALU.mult, op1=ALU.add)
                    nc.vector.tensor_scalar_max(ost, ost, 0.0)
                nc.sync.dma_start(out=out_ext[b, mo * 128:(mo + 1) * 128, :],
                                  in_=ost)


def build(bl=BL):
    nc = bacc.Bacc("TRN2", target_bir_lowering=False, debug=False,
                   num_devices=N_CORES)
    from contextlib import ExitStack
    with tile.TileContext(nc) as tc, ExitStack() as ctx:
        _emit(nc, tc, ctx, bl)
    nc.compile()
    return nc


def prep_weights(inputs):
    """Host-side reshape/cast of the (small) replicated weights."""
    w1 = np.asarray(inputs["conv1_w"], np.float32).reshape(MID, CIN)
    w2 = np.asarray(inputs["conv2_w"], np.float32)
    w3 = np.asarray(inputs["conv3_w"], np.float32).reshape(COUT, MID)

    w1t = np.ascontiguousarray(
        w1.reshape(MID, 8, 128).transpose(2, 1, 0)).astype(bfnp)
    w2t = np.ascontiguousarray(
        w2.reshape(MID, 2, 128, 9).transpose(2, 1, 3, 0)).astype(bfnp)
    w3t = np.ascontiguousarray(
        w3.reshape(COUT, 2, 128).transpose(2, 1, 0)).astype(bfnp)

    def chan_tile(v, nch):
        return np.ascontiguousarray(
            np.asarray(v, np.float32).reshape(nch, 128).T)

    return {
        "w1t": w1t, "w2t": w2t, "w3t": w3t,
        "wg1": np.asarray(inputs["w_gate1"], np.float32).astype(bfnp),
        "wg2": np.asarray(inputs["w_gate2"], np.float32).astype(bfnp),
        "bg1": chan_tile(inputs["b_gate1"], 2),
        "bg2": chan_tile(inputs["b_gate2"], 2),
        "bn1g": chan_tile(inputs["bn1_g"], 2),
        "bn1b": chan_tile(inputs["bn1_b"], 2),
        "bn2g": chan_tile(inputs["bn2_g"], 2),
        "bn2b": chan_tile(inputs["bn2_b"], 2),
        "bn3g": chan_tile(inputs["bn3_g"], 8),
        "bn3b": chan_tile(inputs["bn3_b"], 8),
    }


def make_in_maps(inputs, bl=BL):
    x = np.asarray(inputs["x"], np.float32).reshape(B, CIN, HWP)
    emb = np.asarray(inputs["embeddings"], np.float32)
    w = prep_weights(inputs)
    in_maps = []
    for c in range(N_CORES):
        sl = slice(c * bl, (c + 1) * bl)
        m = dict(w)
        m["x"] = x[sl]
        m["embt"] = np.ascontiguousarray(emb[sl].T).astype(bfnp)
        in_maps.append(m)
    return in_maps


_built = {}


def _get_nc():
    if "nc" not in _built:
        _built["nc"] = build(BL)
    return _built["nc"]


def kernel(**inputs):
    from concourse.bass_utils import run_bass_kernel_spmd
    nc = _get_nc()
    in_maps = make_in_maps(inputs)
    res = run_bass_kernel_spmd(nc, in_maps, list(range(N_CORES)))
    outs = [r["out"].reshape(BL, COUT, H, W) for r in res.results]
    g1s = [r["g1"].reshape(BL, MID) for r in res.results]
    g2s = [r["g2"].reshape(BL, MID) for r in res.results]
    out = np.concatenate(outs, axis=0)
    g1 = np.concatenate(g1s, axis=0)
    g2 = np.concatenate(g2s, axis=0)
    return out, g1, g2


if __name__ == "__main__":
    nc = build(BL)
    print("built + compiled ok")
